# revision 1
# baseline (speedup 1.0000x reference)
"""Trainium2 Bass kernel for nn_NeuralQKM: K[i,j] = |<psi_i|psi_j>|^2.

Math. The circuit's only per-sample gates are last-layer RY rotations, so
S[b] = (prod_q RY_q^T(X[b,q])) psi' with psi' fixed (all shared gates; the
final CNOT chain is a common permutation and drops out of the Gram).
Expanding the tensor-product rotation in the product-feature basis
Phi_b[u] = prod_q (cos(X/2) if u_q=0 else sin(X/2)):

    S[b,j] = sum_u Phi_b[u] * (-1)^{|j&u|} * psi'[j^u]

Split psi' = psi'_0 e_0 + r (||r|| ~ 0.04 since params are tiny):

    S = psi'_0 * (sgn . Phi)  +  Phi @ W_r,   W_r[u,j] = (-1)^{|j&u|} r[j^u]

The main term is exact host math (O(B*DIM)); only the small tail needs a
device matmul, which tolerates fp8.

Device pass 1 (state-sharded): T^T = W_r^T Phi^T via fp8e4m3 DoubleRow
matmuls (K=256/instruction at 0.5 cycles/row). Core r computes 512 states x
4096 samples. Host assembles S = main + tail, normalizes per sample,
quantizes planes A=Re(S), B=Im(S), P=fp8(A+B), M=fp8(A-B) at scale LAM.

Device pass 2 (row-sharded, block-cyclic symmetric): 3-product Karatsuba
Gram in fp8 DoubleRow: M1 = A_r A_c^T, M2 = B_r B_c^T,
M3 = (A_r+B_r)(A_c-B_c)^T; Gre = M1+M2, -Gim = M1-M2-M3. Post-ops apply a
per-state norm correction K = (Gre^2+Gim^2)/(rho_i^2 rho_j^2) with
rho^2 = ||quantized state||^2 (host-known), which cancels the dominant fp8
quantization error on the large entries of K. Output per core is the
transposed block strip K[rows, cols].T in bf16; host mirrors the symmetric
blocks.
"""
import numpy as np
import ml_dtypes
import orjson

import concourse.bass as bass
import concourse.mybir as mybir
import concourse.tile as tile
from concourse.bass_utils import run_bass_kernel_spmd

N_QUBITS = 12
N_LAYERS = 5
DIM = 2 ** N_QUBITS          # 4096
B = 4096
NCORES = 8
BLK = B // NCORES            # 512 rows per core in pass 2
NDBLK = 5                    # diagonal + 4 off-diagonal column blocks
NB_COLS = NDBLK * BLK        # 2560 rhs columns per core
NBLK = NB_COLS // 128        # 20 column blocks of 128
KCH = DIM // 256             # 16 contraction chunks of K=256 (DoubleRow)
LAM = 64.0                   # fp8 quantization scale for state planes
WARMUP1 = 30                 # PE warmup matmuls, pass 1 (sim-tuned)
WARMUP2 = 48                 # PE warmup matmuls, pass 2 (sim-tuned)

f32 = mybir.dt.float32
f8 = mybir.dt.float8e4
bf16 = mybir.dt.bfloat16
npf8 = ml_dtypes.float8_e4m3
npbf = ml_dtypes.bfloat16

# ----------------------------------------------------------------------------
# walrus in this toolchain rejects >1 sync-wait per instruction; Tile emits
# several. Engines are serial, so an extra wait is equivalent to a standalone
# EventSemaphore wait right before the instruction on the same engine.
# ----------------------------------------------------------------------------


def _legalize_multiwait_json(bir: bytes) -> bytes:
    m = orjson.loads(bir)
    changed = False
    for func in m.get("functions", []):
        for blk in func.get("blocks", []):
            out = []
            for inst in blk.get("instructions", []):
                sync = inst.get("sync_info")
                waits = (sync or {}).get("on_wait") or []
                if len(waits) > 1:
                    changed = True
                    for i, w in enumerate(waits[:-1]):
                        out.append({
                            "debug": inst.get("debug", 0),
                            "engine": inst["engine"],
                            "ins": [],
                            "name": f"{inst['name']}-xw{i}",
                            "opcode": "EventSemaphore",
                            "outs": [],
                            "sync_info": {"on_update": [], "on_wait": [w]},
                        })
                    sync["on_wait"] = [waits[-1]]
                out.append(inst)
            blk["instructions"] = out
    return orjson.dumps(m) if changed else bir


_patched = False


def _install_waitfix():
    global _patched
    if _patched:
        return
    _patched = True
    orig = bass.Bass.to_json_bytes

    def patched(self):
        return _legalize_multiwait_json(orig(self))

    bass.Bass.to_json_bytes = patched


# ----------------------------------------------------------------------------
# Host math: psi' (state after all shared circuit parts), complex64 to track
# the reference's precision.
# ----------------------------------------------------------------------------


def _host_psi(params: np.ndarray) -> np.ndarray:
    params = np.asarray(params, np.float32)
    psi = np.zeros(DIM, np.complex64)
    psi[0] = 1.0
    for l in range(N_LAYERS):
        for q in range(N_QUBITS):
            phi, theta, lam = (np.complex64(params[l, q, i]) for i in range(3))
            rz_p = np.array([[np.exp(-0.5j * phi), 0], [0, np.exp(0.5j * phi)]],
                            np.complex64)
            rz_l = np.array([[np.exp(-0.5j * lam), 0], [0, np.exp(0.5j * lam)]],
                            np.complex64)
            c, s = np.cos(0.5 * theta), np.sin(0.5 * theta)
            ry = np.array([[c, -s], [s, c]], np.complex64)
            U = rz_l @ ry @ rz_p
            # reference einsum applies U^T
            st = psi.reshape(2 ** q, 2, -1)
            psi = np.einsum("st,lsr->ltr", U, st).astype(np.complex64).reshape(-1)
        if l < N_LAYERS - 1:
            for q in range(N_QUBITS - 1):
                st = psi.reshape(2 ** q, 2, 2, -1)
                st = np.stack([st[:, 0], np.flip(st[:, 1], axis=1)], axis=1)
                psi = st.reshape(-1)
    return psi


def _popcount_sign() -> np.ndarray:
    j = np.arange(DIM)
    pop = np.zeros(DIM, np.int64)
    for q in range(N_QUBITS):
        pop += (j >> q) & 1
    return np.where(pop % 2 == 0, 1.0, -1.0).astype(np.float32)


def _features(X: np.ndarray) -> np.ndarray:
    """Phi[b, u] = prod_q (cos(X/2) if bit(11-q) of u is 0 else sin(X/2))."""
    c = np.cos(0.5 * X).astype(np.float32)
    s = np.sin(0.5 * X).astype(np.float32)
    phi = np.ones((B, 1), np.float32)
    for q in range(N_QUBITS):
        phi = np.stack([phi * c[:, q:q + 1], phi * s[:, q:q + 1]],
                       axis=2).reshape(B, -1)
    return phi


# ----------------------------------------------------------------------------
# Pass 1: tail states T^T = W_r^T Phi^T, fp8 DoubleRow.
# Core r computes states [512r, 512r+512) x all 4096 samples.
# ----------------------------------------------------------------------------


def _build_pass1() -> bass.Bass:
    nc = bass.Bass("TRN2", target_bir_lowering=False, debug=False,
                   num_devices=NCORES)
    # w8[p, pl, kc, i, blk, c] = plane pl of W_r[kc*256+i*128+p, 512r+blk*128+c]
    w_d = nc.dram_tensor("w8", [128, 2, KCH, 2, 4, 128], f8,
                         kind="ExternalInput").ap()
    # phi[n, p, kc, i, b] = Phi8^T[kc*256+i*128+p, n*512+b]
    phi_d = nc.dram_tensor("phi", [8, 128, KCH, 2, 512], f8,
                           kind="ExternalInput").ap()
    # t[n, pl, p, blk, b] = lamP*lamW * T^T[pl, 512r+blk*128+p, n*512+b]
    t_d = nc.dram_tensor("t", [8, 2, 128, 4, 512], bf16,
                         kind="ExternalOutput").ap()

    with tile.TileContext(nc) as tc:
        with (
            tc.tile_pool(name="wpool", bufs=1) as wpool,
            tc.tile_pool(name="ppool", bufs=2) as phipool,
            tc.tile_pool(name="spool", bufs=2) as spool,
            tc.tile_pool(name="psum", bufs=1, space="PSUM") as psum,
        ):
            # PE warmup: dummy fp8 DoubleRow matmuls (production pattern)
            # bridge the opening DMA so the real matmuls start p-state-warm
            wa = wpool.tile([128, 2, 128], f8, tag="wa")
            wb = wpool.tile([128, 2, 512], f8, tag="wb")
            nc.vector.memset(wa[:], 0.0)
            nc.gpsimd.memset(wb[:], 0.0)
            wps = psum.tile([128, 512], f32, tag="ps00", name="warm")
            for i in range(WARMUP1):
                nc.tensor.matmul(wps[:], wa[:], wb[:], start=True, stop=True,
                                 perf_mode=mybir.MatmulPerfMode.DoubleRow)
            wrd = wpool.tile([128, 128], f32, tag="wrd")
            nc.vector.tensor_copy(wrd[:], wps[:, :128])

            w8 = wpool.tile([128, 2, KCH, 2, 4, 128], f8, tag="w8")

            for n in range(8):
                phi = phipool.tile([128, KCH, 2, 512], f8, tag="phi")
                # SP stream order: phi0, w-pl0, w-pl1, phi1, phi2... — PE
                # starts gapless after phi0+w-pl0; w-pl1 streams under the
                # pl0 groups of chunk 0 and phi1 under its pl1 groups. Any
                # PE bubble resets the p-state ramp (~1.7us extra), so a
                # coarse gapless stream beats a fine-grained early start.
                if n == 0:
                    nc.sync.dma_start(phi[:, :8], phi_d[0, :, :8])
                    nc.sync.dma_start(w8[:, 0, :8], w_d[:, 0, :8])
                    nc.sync.dma_start(phi[:, 8:], phi_d[0, :, 8:])
                    nc.sync.dma_start(w8[:, 0, 8:], w_d[:, 0, 8:])
                    nc.sync.dma_start(w8[:, 1, :8], w_d[:, 1, :8])
                    nc.sync.dma_start(w8[:, 1, 8:], w_d[:, 1, 8:])
                elif n == 1:
                    nc.sync.dma_start(phi[:, :8], phi_d[1, :, :8])
                    nc.sync.dma_start(phi[:, 8:], phi_d[1, :, 8:])
                else:
                    nc.sync.dma_start(phi[:], phi_d[n])
                for pl in range(2):
                    st = spool.tile([128, 4, 512], bf16, tag=f"st{pl}",
                                    name=f"st_{n}_{pl}")
                    for blk in range(4):
                        ps = psum.tile([128, 512], f32, tag=f"ps{pl}{blk}",
                                       name=f"ps_{n}_{pl}_{blk}")
                        for k in range(KCH):
                            nc.tensor.matmul(
                                ps[:], w8[:, pl, k, :, blk, :], phi[:, k],
                                start=(k == 0), stop=(k == KCH - 1),
                                perf_mode=mybir.MatmulPerfMode.DoubleRow)
                        # gpsimd cannot access PSUM; alternate DVE/ACT
                        if blk % 2 == 0:
                            nc.vector.tensor_copy(st[:, blk], ps[:])
                        else:
                            nc.scalar.copy(st[:, blk], ps[:])
                        if n == 7:
                            # last chunk: store per-block to shorten the tail
                            (nc.sync if blk % 2 == 0 else nc.scalar).dma_start(
                                t_d[n, pl, :, blk], st[:, blk])
                    if n < 7:
                        # one batched store per (n, pl), off the SP queue
                        nc.scalar.dma_start(t_d[n, pl], st[:])
    return nc


# ----------------------------------------------------------------------------
# Pass 2: Karatsuba Gram + norm-corrected |.|^2, fp8 DoubleRow.
# ----------------------------------------------------------------------------


def _build_pass2() -> bass.Bass:
    nc = bass.Bass("TRN2", target_bir_lowering=False, debug=False,
                   num_devices=NCORES)
    # mv[p, pl, kc, i, f]: planes (A, B, P=A+B) of own rows (moving operand)
    mv_d = nc.dram_tensor("mv8", [128, 3, KCH, 2, BLK], f8,
                          kind="ExternalInput").ap()
    # wt[n, p, pl, kc, i, c]: planes (A, B, M=A-B) of col block n (stationary)
    wt_d = nc.dram_tensor("wt8", [NBLK, 128, 3, KCH, 2, 128], f8,
                          kind="ExternalInput").ap()
    sig_d = nc.dram_tensor("sig", [128, NBLK], f32, kind="ExternalInput").ap()
    wrow_d = nc.dram_tensor("wrow", [1, BLK], f32, kind="ExternalInput").ap()
    # ko[g, p, j, f]: block n = 4g+j -> K[row 512r+f, col block n, col p].T
    ko_d = nc.dram_tensor("ko", [NBLK // 4, 128, 4, BLK], bf16,
                          kind="ExternalOutput").ap()

    with tile.TileContext(nc) as tc:
        with (
            tc.tile_pool(name="mv", bufs=1) as mpool,
            tc.tile_pool(name="wt", bufs=4) as wpool,
            tc.tile_pool(name="dwt", bufs=1) as dpool,
            tc.tile_pool(name="post", bufs=2) as qpool,
            tc.tile_pool(name="psum", bufs=2, space="PSUM") as ppool,
        ):
            mv = mpool.tile([128, 3, KCH, 2, BLK], f8, tag="mv")
            wt0 = wpool.tile([128, 3, KCH, 2, 128], f8, tag="wt", name="wt_0")
            # SP stream: block 0 operand planes interleaved with mv planes so
            # block 0's products start as early as each plane pair lands
            nc.sync.dma_start(mv[:, 0], mv_d[:, 0])
            nc.sync.dma_start(wt0[:, 0], wt_d[0, :, 0])
            sig = mpool.tile([128, NBLK], f32, tag="sig")
            nc.sync.dma_start(sig[:], sig_d)
            wrow = mpool.tile([128, BLK], f32, tag="wrow")
            nc.sync.dma_start(wrow[:], wrow_d[0].partition_broadcast(128))
            for pl in range(1, 3):
                nc.sync.dma_start(wt0[:, pl], wt_d[0, :, pl])
                # k-halves: block 0's products start accumulating on the
                # first half while the second streams
                nc.sync.dma_start(mv[:, pl, :8], mv_d[:, pl, :8])
                nc.sync.dma_start(mv[:, pl, 8:], mv_d[:, pl, 8:])

            # diagonal stationaries (blocks 16..19 = own cols): planes A, B
            # are SBUF-copied from mv (saves wt DMA); plane M is loaded
            # mid-stream. Copies are emitted in pieces inside the block loop
            # so they don't sit ahead of early post-ops in the engine queues.
            dwts = [dpool.tile([128, 3, KCH, 2, 128], f8, tag=f"dwt{d}",
                               name=f"dwt_{d}")
                    for d in range(4)]
            engs = [nc.vector, nc.scalar, nc.gpsimd]

            def _copy(eng, dst, src):
                if eng is nc.scalar:
                    nc.scalar.copy(dst, src)
                else:
                    eng.tensor_copy(dst, src)

            def emit_diag_copies(d):
                dwt = dwts[d]
                csl = slice(128 * d, 128 * (d + 1))
                for pl in range(2):
                    for h in range(2):
                        ks = slice(8 * h, 8 * h + 8)
                        _copy(engs[(2 * d + pl + h) % 3], dwt[:, pl, ks],
                              mv[:, pl, ks, :, csl])

            # dwt16 is prepared in the opening so block 16 can interleave
            # with block 0, filling its mvB/mvP wait stalls with PE work
            emit_diag_copies(0)
            for h in range(2):
                ks = slice(8 * h, 8 * h + 8)
                (nc.vector, nc.gpsimd)[h].tensor_tensor(
                    dwts[0][:, 2, ks], mv[:, 0, ks, :, slice(0, 128)],
                    mv[:, 1, ks, :, slice(0, 128)], mybir.AluOpType.subtract)

            # diag blocks (no wt DMA) spread through the order: each gives
            # the just-in-time wt stream a 5.1us breather, and the run ends
            # on a block whose weights arrived long before
            ORDER = [0, 16, 1, 2, 3, 17, 4, 5, 6, 18, 7, 8, 9, 19,
                     10, 11, 12, 13, 14, 15]
            kos = None

            def emit_matmuls(b, wt, prod):
                ps = ppool.tile([128, BLK], f32, tag=f"m{prod}",
                                name=f"m{prod}_{b}")
                for k in range(KCH):
                    nc.tensor.matmul(
                        ps[:], wt[:, prod, k], mv[:, prod, k],
                        start=(k == 0), stop=(k == KCH - 1),
                        perf_mode=mybir.MatmulPerfMode.DoubleRow)
                return ps

            def emit_post(b, ms, pos):
                nonlocal kos
                m1, m2, m3 = ms
                # only one PSUM operand allowed per instruction
                c2 = qpool.tile([128, BLK], f32, tag="c2", name=f"c2_{b}")
                nc.scalar.copy(c2[:], m2[:])
                t1 = qpool.tile([128, BLK], f32, tag="t1", name=f"t1_{b}")
                nc.vector.tensor_tensor(t1[:], m1[:], c2[:],
                                        mybir.AluOpType.add)
                t2 = qpool.tile([128, BLK], f32, tag="t2", name=f"t2_{b}")
                nc.vector.tensor_tensor(t2[:], m1[:], c2[:],
                                        mybir.AluOpType.subtract)
                t3 = qpool.tile([128, BLK], f32, tag="t3", name=f"t3_{b}")
                # gpsimd cannot access PSUM -> DVE for the m3 read
                nc.vector.scalar_tensor_tensor(t3[:], m3[:], -1.0, t2[:],
                                               mybir.AluOpType.mult,
                                               mybir.AluOpType.add)
                sq1 = qpool.tile([128, BLK], f32, tag="sq1", name=f"sq1_{b}")
                nc.scalar.activation(sq1[:], t1[:],
                                     mybir.ActivationFunctionType.Square,
                                     scale=sig[:, b:b + 1])
                sq3 = qpool.tile([128, BLK], f32, tag="sq3", name=f"sq3_{b}")
                nc.scalar.activation(sq3[:], t3[:],
                                     mybir.ActivationFunctionType.Square,
                                     scale=sig[:, b:b + 1])
                ss = qpool.tile([128, BLK], f32, tag="ss", name=f"ss_{b}")
                # Pool normally (keeps DVE free of Pool-waiting ops); DVE for
                # the final block where these sit on the exposed tail path
                eng = nc.vector if pos == NBLK - 1 else nc.gpsimd
                eng.tensor_tensor(ss[:], sq1[:], sq3[:],
                                  mybir.AluOpType.add)
                if pos % 4 == 0:
                    kos = qpool.tile([128, 4, BLK], bf16, tag="kos",
                                     name=f"kos_{pos // 4}")
                eng.tensor_tensor(kos[:, pos % 4], ss[:], wrow[:],
                                  mybir.AluOpType.mult)
                if pos >= 16:
                    # last group: per-block stores on the idle SP queue
                    nc.sync.dma_start(ko_d[pos // 4, :, pos % 4],
                                      kos[:, pos % 4])
                elif pos % 4 == 3:
                    nc.scalar.dma_start(ko_d[pos // 4], kos[:])

            # block 0 and block 16 product-interleaved
            ms0 = []
            ms16 = []
            for prod in range(3):
                ms0.append(emit_matmuls(0, wt0, prod))
                ms16.append(emit_matmuls(16, dwts[0], prod))
            emit_post(0, ms0, 0)
            emit_post(16, ms16, 1)

            for pos in range(2, NBLK):
                n = ORDER[pos]
                if n >= 16:
                    wt = dwts[n - 16]
                else:
                    wt = wpool.tile([128, 3, KCH, 2, 128], f8, tag="wt",
                                    name=f"wt_{n}")
                    nc.sync.dma_start(wt[:], wt_d[n])
                if pos in (2, 6, 10):
                    d = {2: 1, 6: 2, 10: 3}[pos]
                    emit_diag_copies(d)
                    csl = slice(128 * d, 128 * (d + 1))
                    for h in range(2):
                        ks = slice(8 * h, 8 * h + 8)
                        eng = (nc.vector, nc.gpsimd)[(d + h) % 2]
                        eng.tensor_tensor(dwts[d][:, 2, ks],
                                          mv[:, 0, ks, :, csl],
                                          mv[:, 1, ks, :, csl],
                                          mybir.AluOpType.subtract)

                ms = [emit_matmuls(n, wt, prod) for prod in range(3)]
                emit_post(n, ms, pos)
    return nc


_nc1 = None
_nc2 = None

PROFILE = False
LAST_PROFILE: dict = {}


def kernel(X: np.ndarray, params: np.ndarray) -> np.ndarray:
    global _nc1, _nc2
    _install_waitfix()
    X = np.asarray(X, np.float32)
    params = np.asarray(params, np.float32)

    # ---- host precompute -------------------------------------------------
    psi = _host_psi(params)
    psi0 = psi[0]
    r = psi.copy()
    r[0] = 0.0
    sgn = _popcount_sign()
    phi = _features(X)                       # (B, DIM) f32

    jj = np.arange(DIM)
    XORm = np.bitwise_xor.outer(jj, jj)      # (u, j)
    ANDm = np.bitwise_and.outer(jj, jj)
    sgn_uj = sgn[ANDm]
    w_re = sgn_uj * r.real[XORm]
    w_im = sgn_uj * r.imag[XORm]
    lam_w = float(224.0 / max(np.abs(w_re).max(), np.abs(w_im).max(), 1e-30))
    w8 = np.stack([(w_re * lam_w).astype(npf8),
                   (w_im * lam_w).astype(npf8)])      # (2, DIM u, DIM j)
    lam_p = 64.0
    phi8t = np.ascontiguousarray((phi.T * lam_p).astype(npf8))   # (u, b)

    # per-core pass-1 inputs
    phi_in = np.ascontiguousarray(
        phi8t.reshape(KCH, 2, 128, 8, 512).transpose(3, 2, 0, 1, 4))
    in_maps1 = []
    for cr in range(NCORES):
        wc = w8[:, :, cr * BLK:(cr + 1) * BLK]        # (2, DIM, 512)
        wc = wc.reshape(2, KCH, 2, 128, 4, 128).transpose(3, 0, 1, 2, 4, 5)
        in_maps1.append({"w8": np.ascontiguousarray(wc), "phi": phi_in})

    if _nc1 is None:
        _nc1 = _build_pass1()
    res1 = run_bass_kernel_spmd(_nc1, in_maps1, core_ids=list(range(NCORES)))

    # ---- host mid: assemble S, quantize planes ---------------------------
    inv_lw = 1.0 / (lam_p * lam_w)
    phiT = phi.T                                      # (j, b)
    A = np.empty((DIM, B), np.float32)
    Bp = np.empty((DIM, B), np.float32)
    for cr in range(NCORES):
        t = res1.results[cr]["t"].astype(np.float32) * inv_lw  # (8,2,128,4,512)
        rows = slice(cr * BLK, (cr + 1) * BLK)
        tt = t.transpose(1, 3, 2, 0, 4).reshape(2, BLK, B)
        A[rows] = tt[0]
        Bp[rows] = tt[1]
    A += psi0.real * sgn[:, None] * phiT
    Bp += psi0.imag * sgn[:, None] * phiT
    nrm = np.sqrt(np.einsum("jb,jb->b", A, A) + np.einsum("jb,jb->b", Bp, Bp))
    A *= (1.0 / nrm)[None, :]
    Bp *= (1.0 / nrm)[None, :]

    A8 = (A * LAM).astype(npf8)
    B8 = (Bp * LAM).astype(npf8)
    A8f = A8.astype(np.float32)
    B8f = B8.astype(np.float32)
    P8 = (A8f + B8f).astype(npf8)
    M8 = (A8f - B8f).astype(npf8)
    rho2 = (np.einsum("jb,jb->b", A8f, A8f)
            + np.einsum("jb,jb->b", B8f, B8f)) / (LAM * LAM)    # (B,)

    pl_mv = np.stack([A8, B8, P8])    # (3, j, b)
    pl_wt = np.stack([A8, B8, M8])
    sig_all = (1.0 / (LAM * LAM * np.sqrt(rho2))).astype(np.float32)
    wrow_all = (1.0 / rho2).astype(np.float32)

    # strip layout: 16 off-diagonal col blocks first (strip offsets
    # 512..2560), the 4 diagonal blocks (offsets 0..512) last — the device
    # fills the diagonal stationary tiles by SBUF copies from mv.
    colrel = np.concatenate([np.arange(BLK, NB_COLS), np.arange(0, BLK)])
    in_maps2 = []
    for cr in range(NCORES):
        cols = (cr * BLK + colrel) % B
        mvc = pl_mv[:, :, cr * BLK:(cr + 1) * BLK]    # (3, DIM, 512)
        mvc = mvc.reshape(3, KCH, 2, 128, BLK).transpose(3, 0, 1, 2, 4)
        wtc = pl_wt[:, :, cols]                       # (3, DIM, 2560)
        wtc = (wtc.reshape(3, KCH, 2, 128, NBLK, 128)
               .transpose(4, 3, 0, 1, 2, 5))
        sig = sig_all[cols].reshape(NBLK, 128).T      # (128, NBLK)
        wrow = wrow_all[cr * BLK:(cr + 1) * BLK][None, :]
        in_maps2.append({
            "mv8": np.ascontiguousarray(mvc),
            "wt8": np.ascontiguousarray(wtc),
            "sig": np.ascontiguousarray(sig),
            "wrow": np.ascontiguousarray(wrow),
        })

    if _nc2 is None:
        _nc2 = _build_pass2()
    res2 = run_bass_kernel_spmd(_nc2, in_maps2, core_ids=list(range(NCORES)))

    # ---- assemble K (with symmetric mirroring) ---------------------------
    K = np.empty((B, B), np.float32)
    for cr in range(NCORES):
        # (NBLK//4, 128, 4, BLK) -> (NBLK, 128, BLK)
        ko = (res2.results[cr]["ko"].astype(np.float32)
              .transpose(0, 2, 1, 3).reshape(NBLK, 128, BLK))
        rows = slice(cr * BLK, (cr + 1) * BLK)
        order = [0, 16, 1, 2, 3, 17, 4, 5, 6, 18, 7, 8, 9, 19,
                 10, 11, 12, 13, 14, 15]
        for pos in range(NBLK):
            n = order[pos]
            gs = (cr * BLK + int(colrel[n * 128])) % B
            colsl = slice(gs, gs + 128)
            K[rows, colsl] = ko[pos].T
            d = 1 + n // 4 if n < 16 else 0
            if 0 < d < 4 or (d == 4 and cr < 4):
                K[colsl, rows] = ko[pos]
    return K



# revision 5
# speedup vs baseline: 3.0122x; 3.0122x over previous
"""Trainium2 Bass kernel for nn_NeuralQKM: K[i,j] = |<psi_i|psi_j>|^2.

Math. States factor as S = Phi C with product features
Phi_b[u] = prod_q (cos(X/2) if u_q=0 else sin(X/2)) and a fixed complex
matrix C[u,j] = (-1)^{|j&u|} psi'[j^u] (psi' = state after all shared
gates; the final CNOT chain is a common permutation and drops out).
The Gram G = S S^H = Phi (C C^H) Phi^T where

    (C C^H)[u,u'] = (-1)^{|u&d|} rho(d),  d = u^u',
    rho(d) = sum_k (-1)^{|k&d|} psi'[k] conj(psi'[k^d]),

so Re G = Phi Wsym Phi^T with Wsym real symmetric PSD, and Re rho(d) = 0
for odd |d| makes Wsym parity-block-diagonal. Im G vanishes on the
diagonal and contributes O(1e-6) to ||K||_F: K ~= (Re G)^2 elementwise.

Cholesky per parity block, Wsym = L L^T, gives Re G = Z Z^T with
Z = Phi L of exactly unit row norm. W = L - I is small (params are
tiny), so Z = Phi + Phi W: the main term is exact host math and only the
tail needs the device, which tolerates fp8.

Device pass 1 (4 batch-groups x 2 parities): tail^T = W^T Phi^T per
parity block, fp8 DoubleRow, skipping the strictly-upper-triangular
chunks of W (53% of the dense work). lam_w is sized so psum values fit
fp8 range directly: the tail streams out as fp8 with a plain copy.

Device pass 2 (row-sharded, block-cyclic symmetric): single-product Gram
ps = Z8_cols . Z8_rows^T; post-ops square with a per-state norm
correction K = ps^2/(LAM^4 rho_c^2 rho_r^2) (rho^2 = ||quantized Z||^2,
host-known), which cancels the dominant fp8 radial error. Diagonal
column blocks reuse mv directly as the stationary operand (no wt DMA).
Output per core is the transposed block strip K[rows, cols].T in bf16;
host mirrors the symmetric blocks.
"""
import numpy as np
import ml_dtypes
import orjson

import concourse.bass as bass
import concourse.mybir as mybir
import concourse.tile as tile
from concourse.bass_utils import run_bass_kernel_spmd

N_QUBITS = 12
N_LAYERS = 5
DIM = 2 ** N_QUBITS          # 4096
HDIM = DIM // 2              # 2048 per parity block
B = 4096
NCORES = 8
BLK = B // NCORES            # 512 rows per core in pass 2
NDBLK = 5                    # diagonal + 4 off-diagonal column blocks
NB_COLS = NDBLK * BLK        # 2560 rhs columns per core
NBLK = NB_COLS // 128        # 20 column blocks of 128
KCH = DIM // 256             # 16 contraction chunks of K=256 (DoubleRow)
KCH1 = HDIM // 256           # 8 contraction chunks in pass 1
NJB = HDIM // 128            # 16 output column blocks in pass 1
BG = B // 4                  # 1024 samples per pass-1 batch-group
LAM = 64.0                   # fp8 quantization scale for state planes
WARMUP1 = 12                 # PE warmup matmuls, pass 1 (sim-tuned)
WARMUP2 = 24                 # PE warmup matmuls, pass 2 (sim-tuned)

# pass-1 chunk schedule: stage s handles j-blocks (15-2s, 14-2s), each
# needing k-chunks 7-s..7 of the triangular W
P1_SCHED = []
for _s in range(8):
    for _c in (NJB - 1 - 2 * _s, NJB - 2 - 2 * _s):
        P1_SCHED.append((_c, list(range(_c // 2, KCH1))))
P1_NCHUNK = sum(len(ks) for _, ks in P1_SCHED)   # 72

f32 = mybir.dt.float32
f8 = mybir.dt.float8e4
bf16 = mybir.dt.bfloat16
npf8 = ml_dtypes.float8_e4m3
npbf = ml_dtypes.bfloat16

# ----------------------------------------------------------------------------
# walrus in this toolchain rejects >1 sync-wait per instruction; Tile emits
# several. Engines are serial, so an extra wait is equivalent to a standalone
# EventSemaphore wait right before the instruction on the same engine.
# ----------------------------------------------------------------------------


def _legalize_multiwait_json(bir: bytes) -> bytes:
    m = orjson.loads(bir)
    changed = False
    for func in m.get("functions", []):
        for blk in func.get("blocks", []):
            out = []
            for inst in blk.get("instructions", []):
                sync = inst.get("sync_info")
                waits = (sync or {}).get("on_wait") or []
                if len(waits) > 1:
                    changed = True
                    for i, w in enumerate(waits[:-1]):
                        out.append({
                            "debug": inst.get("debug", 0),
                            "engine": inst["engine"],
                            "ins": [],
                            "name": f"{inst['name']}-xw{i}",
                            "opcode": "EventSemaphore",
                            "outs": [],
                            "sync_info": {"on_update": [], "on_wait": [w]},
                        })
                    sync["on_wait"] = [waits[-1]]
                out.append(inst)
            blk["instructions"] = out
    return orjson.dumps(m) if changed else bir


_patched = False


def _install_waitfix():
    global _patched
    if _patched:
        return
    _patched = True
    orig = bass.Bass.to_json_bytes

    def patched(self):
        return _legalize_multiwait_json(orig(self))

    bass.Bass.to_json_bytes = patched


# ----------------------------------------------------------------------------
# Host math: psi' (state after all shared circuit parts), complex64 to track
# the reference's precision.
# ----------------------------------------------------------------------------


def _host_psi(params: np.ndarray) -> np.ndarray:
    params = np.asarray(params, np.float32)
    psi = np.zeros(DIM, np.complex64)
    psi[0] = 1.0
    for l in range(N_LAYERS):
        for q in range(N_QUBITS):
            phi, theta, lam = (np.complex64(params[l, q, i]) for i in range(3))
            rz_p = np.array([[np.exp(-0.5j * phi), 0], [0, np.exp(0.5j * phi)]],
                            np.complex64)
            rz_l = np.array([[np.exp(-0.5j * lam), 0], [0, np.exp(0.5j * lam)]],
                            np.complex64)
            c, s = np.cos(0.5 * theta), np.sin(0.5 * theta)
            ry = np.array([[c, -s], [s, c]], np.complex64)
            U = rz_l @ ry @ rz_p
            # reference einsum applies U^T
            st = psi.reshape(2 ** q, 2, -1)
            psi = np.einsum("st,lsr->ltr", U, st).astype(np.complex64).reshape(-1)
        if l < N_LAYERS - 1:
            for q in range(N_QUBITS - 1):
                st = psi.reshape(2 ** q, 2, 2, -1)
                st = np.stack([st[:, 0], np.flip(st[:, 1], axis=1)], axis=1)
                psi = st.reshape(-1)
    return psi


def _features(X: np.ndarray) -> np.ndarray:
    """Phi[b, u] = prod_q (cos(X/2) if bit(11-q) of u is 0 else sin(X/2))."""
    c = np.cos(0.5 * X).astype(np.float32)
    s = np.sin(0.5 * X).astype(np.float32)
    phi = np.ones((B, 1), np.float32)
    for q in range(N_QUBITS):
        phi = np.stack([phi * c[:, q:q + 1], phi * s[:, q:q + 1]],
                       axis=2).reshape(B, -1)
    return phi


def _host_factor(psi: np.ndarray):
    """rho -> Wsym -> parity-ordered Cholesky. Returns (perm, W0, W1) with
    W = L - I per parity block (f32, strictly small)."""
    jj = np.arange(DIM)
    XORm = np.bitwise_xor.outer(jj, jj).astype(np.int32)
    ANDm = np.bitwise_and.outer(jj, jj).astype(np.int32)
    popand = np.zeros((DIM, DIM), np.int8)
    t = ANDm
    for q in range(N_QUBITS):
        popand += (t & 1).astype(np.int8)
        t = t >> 1
    del ANDm, t
    sgn_and = np.where(popand % 2 == 0, np.float32(1), np.float32(-1))
    del popand
    pop = np.zeros(DIM, np.int64)
    for q in range(N_QUBITS):
        pop += (jj >> q) & 1
    sgn = np.where(pop % 2 == 0, np.float32(1), np.float32(-1))
    par = (pop & 1).astype(np.int8)

    M = sgn_and * np.conj(psi)[XORm]          # M[d,k] = sgn(d&k) psi*_{d^k}
    rho = np.real(M @ psi).astype(np.float32)
    del M
    Wsym = (sgn[:, None] * sgn_and) * rho[XORm]
    del sgn_and, XORm

    perm = np.argsort(par, kind="stable")
    Wp = Wsym[np.ix_(perm, perm)]
    del Wsym
    L0 = np.linalg.cholesky(Wp[:HDIM, :HDIM].astype(np.float64))
    L1 = np.linalg.cholesky(Wp[HDIM:, HDIM:].astype(np.float64))
    W0 = (L0 - np.eye(HDIM)).astype(np.float32)
    W1 = (L1 - np.eye(HDIM)).astype(np.float32)
    return perm, W0, W1


# ----------------------------------------------------------------------------
# Pass 1: tail^T = W^T Phi^T per parity block, triangular fp8 DoubleRow.
# Core cr = 2*bg + p handles batch-group bg (1024 samples), parity p.
# ----------------------------------------------------------------------------


def _build_pass1() -> bass.Bass:
    nc = bass.Bass("TRN2", target_bir_lowering=False, debug=False,
                   num_devices=NCORES)
    # w8[p, idx, i, c]: chunk list in P1_SCHED order; chunk (k, cblk) holds
    # lam_w * W[k*256 + i*128 + p, cblk*128 + c]
    w_d = nc.dram_tensor("w8", [128, P1_NCHUNK, 2, 128], f8,
                         kind="ExternalInput").ap()
    # phi[k, p, i, b] = lam_p * Phi^T[k*256 + i*128 + p, bg*1024 + b]
    phi_d = nc.dram_tensor("phi", [KCH1, 128, 2, BG], f8,
                           kind="ExternalInput").ap()
    # t[cblk, p, b] = lam_p*lam_w * tail^T[cblk*128 + p, bg*1024 + b]
    t_d = nc.dram_tensor("t", [NJB, 128, BG], f8, kind="ExternalOutput").ap()

    with tile.TileContext(nc) as tc:
        with (
            tc.tile_pool(name="wpool", bufs=1) as wpool,
            tc.tile_pool(name="spool", bufs=2) as spool,
            tc.tile_pool(name="psum", bufs=2, space="PSUM") as psum,
        ):
            # PE warmup: dummy fp8 DoubleRow matmuls bridge the opening DMA
            # so the real matmuls start p-state-warm
            wa = wpool.tile([128, 2, 128], f8, tag="wa")
            wb = wpool.tile([128, 2, 512], f8, tag="wb")
            nc.vector.memset(wa[:], 0.0)
            nc.gpsimd.memset(wb[:], 0.0)
            wps = psum.tile([128, 512], f32, tag="ps0", name="warm")
            for i in range(WARMUP1):
                nc.tensor.matmul(wps[:], wa[:], wb[:], start=True, stop=True,
                                 perf_mode=mybir.MatmulPerfMode.DoubleRow)

            w8 = wpool.tile([128, P1_NCHUNK, 2, 128], f8, tag="w8")
            phis = [wpool.tile([128, 2, BG], f8, tag=f"phi{k}",
                               name=f"phi_{k}")
                    for k in range(KCH1)]

            # SP stream: phi chunk 7-s, then the W chunks of stage s; each
            # j-block's matmuls start as soon as its last-needed chunk lands
            idx = 0
            order = []
            for s in range(8):
                nc.sync.dma_start(phis[KCH1 - 1 - s][:], phi_d[KCH1 - 1 - s])
                n = 2 * (s + 1)
                nc.sync.dma_start(w8[:, idx:idx + n], w_d[:, idx:idx + n])
                idx += n

            pos = 0
            idx = 0
            for c, ks in P1_SCHED:
                ps0 = psum.tile([128, 512], f32, tag="psA", name=f"psA_{c}")
                ps1 = psum.tile([128, 512], f32, tag="psB", name=f"psB_{c}")
                for j, k in enumerate(ks):
                    st_mm = (j == 0)
                    sp_mm = (j == len(ks) - 1)
                    wch = w8[:, idx + j]
                    nc.tensor.matmul(ps0[:], wch, phis[k][:, :, :512],
                                     start=st_mm, stop=sp_mm,
                                     perf_mode=mybir.MatmulPerfMode.DoubleRow)
                    nc.tensor.matmul(ps1[:], wch, phis[k][:, :, 512:],
                                     start=st_mm, stop=sp_mm,
                                     perf_mode=mybir.MatmulPerfMode.DoubleRow)
                idx += len(ks)
                st = spool.tile([128, BG], f8, tag="st", name=f"st_{c}")
                # psum values are bounded by fp8 range via lam_w: plain copy
                if pos % 2 == 0:
                    nc.vector.tensor_copy(st[:, :512], ps0[:])
                    nc.scalar.copy(st[:, 512:], ps1[:])
                else:
                    nc.scalar.copy(st[:, :512], ps0[:])
                    nc.vector.tensor_copy(st[:, 512:], ps1[:])
                nc.scalar.dma_start(t_d[c], st[:])
                pos += 1
    return nc


# ----------------------------------------------------------------------------
# Pass 2: single-product Gram + norm-corrected square, fp8 DoubleRow.
# ----------------------------------------------------------------------------


def _build_pass2() -> bass.Bass:
    nc = bass.Bass("TRN2", target_bir_lowering=False, debug=False,
                   num_devices=NCORES)
    # mv[p, kc, i, f]: Z8^T chunk of own rows (moving operand; also the
    # stationary operand for the 4 diagonal column blocks)
    mv_d = nc.dram_tensor("mv8", [128, KCH, 2, BLK], f8,
                          kind="ExternalInput").ap()
    # wt[n, p, kc, i, c]: Z8^T of off-diagonal column block n (stationary)
    wt_d = nc.dram_tensor("wt8", [NBLK - 4, 128, KCH, 2, 128], f8,
                          kind="ExternalInput").ap()
    sig_d = nc.dram_tensor("sig", [128, NBLK], f32, kind="ExternalInput").ap()
    wrow_d = nc.dram_tensor("wrow", [1, BLK], f32, kind="ExternalInput").ap()
    # ko[g, p, j, f]: position pos = 4g+j -> K[rows, col block, col p].T
    ko_d = nc.dram_tensor("ko", [NBLK // 4, 128, 4, BLK], bf16,
                          kind="ExternalOutput").ap()

    with tile.TileContext(nc) as tc:
        with (
            tc.tile_pool(name="mv", bufs=1) as mpool,
            tc.tile_pool(name="wt", bufs=4) as wpool,
            tc.tile_pool(name="post", bufs=2) as qpool,
            tc.tile_pool(name="psum", bufs=2, space="PSUM") as ppool,
        ):
            mv = mpool.tile([128, KCH, 2, BLK], f8, tag="mv")
            wt0 = wpool.tile([128, KCH, 2, 128], f8, tag="wt", name="wt_0")
            # SP stream: first halves of mv and block 0 land, matmuls start;
            # second halves stream under the first-half products
            nc.sync.dma_start(mv[:, :8], mv_d[:, :8])
            nc.sync.dma_start(wt0[:, :8], wt_d[0, :, :8])
            sig = mpool.tile([128, NBLK], f32, tag="sig")
            nc.sync.dma_start(sig[:], sig_d)
            wrow = mpool.tile([128, BLK], f32, tag="wrow")
            nc.sync.dma_start(wrow[:], wrow_d[0].partition_broadcast(128))
            nc.sync.dma_start(mv[:, 8:], mv_d[:, 8:])
            nc.sync.dma_start(wt0[:, 8:], wt_d[0, :, 8:])

            # warmup on PE under the opening DMA
            wa = mpool.tile([128, 2, 128], f8, tag="wa")
            wb = mpool.tile([128, 2, 512], f8, tag="wb")
            nc.vector.memset(wa[:], 0.0)
            nc.gpsimd.memset(wb[:], 0.0)
            wps = ppool.tile([128, BLK], f32, tag="mA", name="warm")
            for i in range(WARMUP2):
                nc.tensor.matmul(wps[:], wa[:], wb[:], start=True, stop=True,
                                 perf_mode=mybir.MatmulPerfMode.DoubleRow)

            # diag blocks (no wt DMA: stationary slices straight out of mv)
            # spread through the order: each gives the just-in-time wt stream
            # a breather, and the run ends on a block whose weights arrived
            # long before. Position -> block n (diag blocks are n >= 16,
            # using mv cols (n-16)*128..): col offsets per colrel below.
            ORDER = [0, 16, 1, 2, 3, 17, 4, 5, 6, 18, 7, 8, 9, 19,
                     10, 11, 12, 13, 14, 15]
            kos = None
            for pos in range(NBLK):
                n = ORDER[pos]
                if pos == 0:
                    wt = wt0
                elif n >= 16:
                    d = n - 16
                    wt = None                  # slices of mv
                else:
                    wt = wpool.tile([128, KCH, 2, 128], f8, tag="wt",
                                    name=f"wt_{n}")
                    nc.sync.dma_start(wt[:], wt_d[n])
                ps = ppool.tile([128, BLK], f32, tag=f"m{pos % 2}",
                                name=f"m_{pos}")
                for k in range(KCH):
                    stat = (wt[:, k] if wt is not None
                            else mv[:, k, :, d * 128:(d + 1) * 128])
                    nc.tensor.matmul(ps[:], stat, mv[:, k],
                                     start=(k == 0), stop=(k == KCH - 1),
                                     perf_mode=mybir.MatmulPerfMode.DoubleRow)
                sq = qpool.tile([128, BLK], f32, tag="sq", name=f"sq_{pos}")
                nc.scalar.activation(sq[:], ps[:],
                                     mybir.ActivationFunctionType.Square,
                                     scale=sig[:, pos:pos + 1])
                if pos % 4 == 0:
                    kos = qpool.tile([128, 4, BLK], bf16, tag="kos",
                                     name=f"kos_{pos // 4}")
                nc.vector.tensor_tensor(kos[:, pos % 4], sq[:], wrow[:],
                                        mybir.AluOpType.mult)
                if pos >= 16:
                    # last group: per-block stores on the idle SP queue
                    nc.sync.dma_start(ko_d[pos // 4, :, pos % 4],
                                      kos[:, pos % 4])
                elif pos % 4 == 3:
                    nc.scalar.dma_start(ko_d[pos // 4], kos[:])
    return nc


_nc1 = None
_nc2 = None

PROFILE = False
LAST_PROFILE: dict = {}


def kernel(X: np.ndarray, params: np.ndarray) -> np.ndarray:
    global _nc1, _nc2
    _install_waitfix()
    X = np.asarray(X, np.float32)
    params = np.asarray(params, np.float32)

    # ---- host precompute -------------------------------------------------
    psi = _host_psi(params)
    phi = _features(X)                           # (B, DIM) f32
    perm, W0, W1 = _host_factor(psi)
    phiT = np.ascontiguousarray(phi[:, perm].T)  # (DIM parity-ordered, B)

    lam_p = 64.0
    # bound |tail| <= max column norm of W so psum fits fp8 range directly
    bnd0 = float(np.sqrt((W0.astype(np.float64) ** 2).sum(axis=0).max()))
    bnd1 = float(np.sqrt((W1.astype(np.float64) ** 2).sum(axis=0).max()))
    lam_w0 = 400.0 / (lam_p * max(bnd0, 1e-30))
    lam_w1 = 400.0 / (lam_p * max(bnd1, 1e-30))
    phi8 = (phiT * lam_p).astype(npf8)           # (DIM, B)

    def pack_w(W, lam_w):
        W8 = (W * lam_w).astype(npf8)            # (HDIM u, HDIM j)
        out = np.empty((128, P1_NCHUNK, 2, 128), npf8)
        idx = 0
        for c, ks in P1_SCHED:
            for k in ks:
                ch = W8[k * 256:(k + 1) * 256, c * 128:(c + 1) * 128]
                out[:, idx] = ch.reshape(2, 128, 128).transpose(1, 0, 2)
                idx += 1
        return out

    w_par = [pack_w(W0, lam_w0), pack_w(W1, lam_w1)]
    phi_par = []
    for p in range(2):
        rows = phi8[p * HDIM:(p + 1) * HDIM]     # (HDIM, B)
        phi_par.append(rows.reshape(KCH1, 2, 128, B).transpose(0, 2, 1, 3))

    in_maps1 = []
    for cr in range(NCORES):
        bg, p = divmod(cr, 2)
        in_maps1.append({
            "w8": w_par[p],
            "phi": np.ascontiguousarray(phi_par[p][:, :, :,
                                                   bg * BG:(bg + 1) * BG]),
        })

    if _nc1 is None:
        _nc1 = _build_pass1()
    res1 = run_bass_kernel_spmd(_nc1, in_maps1, core_ids=list(range(NCORES)))

    # ---- host mid: assemble Z, quantize ----------------------------------
    ZT = phiT                                    # reuse buffer (DIM, B)
    inv = [1.0 / (lam_p * lam_w0), 1.0 / (lam_p * lam_w1)]
    for cr in range(NCORES):
        bg, p = divmod(cr, 2)
        t = res1.results[cr]["t"].astype(np.float32) * inv[p]   # (16,128,1024)
        ZT[p * HDIM:(p + 1) * HDIM, bg * BG:(bg + 1) * BG] += t.reshape(
            HDIM, BG)

    Z8 = (ZT * LAM).astype(npf8)                 # (DIM, B)
    Z8f32 = Z8.astype(np.float32)
    rho2 = np.einsum("jb,jb->b", Z8f32, Z8f32) / (LAM * LAM)    # (B,)
    del Z8f32
    sig_all = (1.0 / (LAM * LAM * np.sqrt(rho2))).astype(np.float32)
    wrow_all = (1.0 / rho2).astype(np.float32)

    # strip layout: 16 off-diagonal col blocks (strip offsets 512..2560)
    # DMA'd as wt; the 4 diagonal blocks (offsets 0..512) slice mv.
    colrel = np.concatenate([np.arange(BLK, NB_COLS), np.arange(0, BLK)])
    ORDER = [0, 16, 1, 2, 3, 17, 4, 5, 6, 18, 7, 8, 9, 19,
             10, 11, 12, 13, 14, 15]
    Z8c = Z8.reshape(KCH, 2, 128, B)
    in_maps2 = []
    for cr in range(NCORES):
        cols = (cr * BLK + colrel) % B
        mvc = Z8c[:, :, :, cr * BLK:(cr + 1) * BLK].transpose(2, 0, 1, 3)
        wtc = Z8c[:, :, :, cols[:16 * 128]].reshape(
            KCH, 2, 128, 16, 128).transpose(3, 2, 0, 1, 4)
        sig_blocks = sig_all[cols].reshape(NBLK, 128)    # by block n
        sig = sig_blocks[ORDER].T                        # (128, pos)
        wrow = wrow_all[cr * BLK:(cr + 1) * BLK][None, :]
        in_maps2.append({
            "mv8": np.ascontiguousarray(mvc),
            "wt8": np.ascontiguousarray(wtc),
            "sig": np.ascontiguousarray(sig),
            "wrow": np.ascontiguousarray(wrow),
        })

    if _nc2 is None:
        _nc2 = _build_pass2()
    res2 = run_bass_kernel_spmd(_nc2, in_maps2, core_ids=list(range(NCORES)))

    # ---- assemble K (with symmetric mirroring) ---------------------------
    K = np.empty((B, B), np.float32)
    for cr in range(NCORES):
        # (NBLK//4, 128, 4, BLK) -> (pos, 128, BLK)
        ko = (res2.results[cr]["ko"].astype(np.float32)
              .transpose(0, 2, 1, 3).reshape(NBLK, 128, BLK))
        rows = slice(cr * BLK, (cr + 1) * BLK)
        for pos in range(NBLK):
            n = ORDER[pos]
            gs = (cr * BLK + int(colrel[n * 128])) % B
            colsl = slice(gs, gs + 128)
            K[rows, colsl] = ko[pos].T
            d = 1 + n // 4 if n < 16 else 0
            if 0 < d < 4 or (d == 4 and cr < 4):
                K[colsl, rows] = ko[pos]
    return K


# revision 16
# speedup vs baseline: 3.7248x; 1.2366x over previous
"""Trainium2 Bass kernel for nn_NeuralQKM: K[i,j] = |<psi_i|psi_j>|^2.

Math. States factor as S = Phi C with product features
Phi_b[u] = prod_q (cos(X/2) if u_q=0 else sin(X/2)) and a fixed complex
matrix C[u,j] = (-1)^{|j&u|} psi'[j^u] (psi' = state after all shared
gates; the final CNOT chain is a common permutation and drops out).
The Gram G = S S^H = Phi (C C^H) Phi^T where

    (C C^H)[u,u'] = (-1)^{|u&d|} rho(d),  d = u^u',
    rho(d) = sum_k (-1)^{|k&d|} psi'[k] conj(psi'[k^d]),

so Re G = Phi Wsym Phi^T with Wsym real symmetric PSD, and Re rho(d) = 0
for odd |d| makes Wsym parity-block-diagonal. Im G vanishes on the
diagonal and contributes O(1e-6) to ||K||_F: K ~= (Re G)^2 elementwise.

Cholesky per parity block, Wsym = L L^T, gives Re G = Z Z^T with
Z = Phi L of exactly unit row norm. W = L - I is small (params are
tiny), so Z = Phi + Phi W: the main term is exact host math and only the
tail needs the device, which tolerates fp8.

Device pass 1 (4 batch-groups x 2 parities): tail^T = W^T Phi^T per
parity block, fp8 DoubleRow, keeping only the lower-triangular W chunks
whose Frobenius mass matters (~40 of 128). lam_w is sized so psum values
fit fp8 range directly: the tail streams out as fp8 with a plain copy.
Output stores ride the in-order SP queue after every input DMA so they
never preempt the input stream on the shared DMA engines.

Device pass 2 (row-sharded, block-cyclic symmetric): single-product Gram
ps = Z8_cols . Z8_rows^T; post-ops square with a per-state norm
correction K = ps^2/(LAM^4 rho_c^2 rho_r^2) (rho^2 = ||quantized Z||^2,
host-known), which cancels the dominant fp8 radial error. Diagonal
column blocks slice mv directly as the stationary operand (no wt DMA)
and two of them open the pass so compute starts after one mv chunk.
Output per core is the transposed block strip K[rows, cols].T in bf16;
host mirrors the symmetric blocks.
"""
import numpy as np
import ml_dtypes
import orjson

import concourse.bass as bass
import concourse.mybir as mybir
import concourse.tile as tile
from concourse.bass_utils import run_bass_kernel_spmd

N_QUBITS = 12
N_LAYERS = 5
DIM = 2 ** N_QUBITS          # 4096
HDIM = DIM // 2              # 2048 per parity block
B = 4096
NCORES = 8
BLK = B // NCORES            # 512 rows per core in pass 2
NDBLK = 5                    # diagonal + 4 off-diagonal column blocks
NB_COLS = NDBLK * BLK        # 2560 rhs columns per core
NBLK = NB_COLS // 128        # 20 column blocks of 128
KCH = DIM // 256             # 16 contraction chunks of K=256 (DoubleRow)
KCH1 = HDIM // 256           # 8 contraction chunks in pass 1
NJB = HDIM // 128            # 16 output column blocks in pass 1
BG = B // 4                  # 1024 samples per pass-1 batch-group
LAM = 64.0                   # fp8 quantization scale for state planes
WARMUP1 = 15                 # PE warmup matmuls, pass 1 (sim-tuned)
WARMUP2 = 11                 # PE warmup matmuls, pass 2 (sim-tuned)

# pass-2 block order: two free-stationary diagonal blocks open the pass
# (only mv chunks needed), the other two give the wt stream breathers
# where output stores contend for the DMA engines
ORDER2 = [16, 17, 0, 1, 2, 3, 4, 5, 6, 7, 18, 8, 9, 10, 11, 12, 19,
          13, 14, 15]

f32 = mybir.dt.float32
f8 = mybir.dt.float8e4
bf16 = mybir.dt.bfloat16
npf8 = ml_dtypes.float8_e4m3
npbf = ml_dtypes.bfloat16

# ----------------------------------------------------------------------------
# walrus in this toolchain rejects >1 sync-wait per instruction; Tile emits
# several. Engines are serial, so an extra wait is equivalent to a standalone
# EventSemaphore wait right before the instruction on the same engine.
# ----------------------------------------------------------------------------


def _legalize_multiwait_json(bir: bytes) -> bytes:
    m = orjson.loads(bir)
    changed = False
    for func in m.get("functions", []):
        for blk in func.get("blocks", []):
            out = []
            for inst in blk.get("instructions", []):
                sync = inst.get("sync_info")
                waits = (sync or {}).get("on_wait") or []
                if len(waits) > 1:
                    changed = True
                    for i, w in enumerate(waits[:-1]):
                        out.append({
                            "debug": inst.get("debug", 0),
                            "engine": inst["engine"],
                            "ins": [],
                            "name": f"{inst['name']}-xw{i}",
                            "opcode": "EventSemaphore",
                            "outs": [],
                            "sync_info": {"on_update": [], "on_wait": [w]},
                        })
                    sync["on_wait"] = [waits[-1]]
                out.append(inst)
            blk["instructions"] = out
    return orjson.dumps(m) if changed else bir


_patched = False


def _install_waitfix():
    global _patched
    if _patched:
        return
    _patched = True
    orig = bass.Bass.to_json_bytes

    def patched(self):
        return _legalize_multiwait_json(orig(self))

    bass.Bass.to_json_bytes = patched


# ----------------------------------------------------------------------------
# Host math: psi' (state after all shared circuit parts), complex64 to track
# the reference's precision.
# ----------------------------------------------------------------------------


def _host_psi(params: np.ndarray) -> np.ndarray:
    params = np.asarray(params, np.float32)
    psi = np.zeros(DIM, np.complex64)
    psi[0] = 1.0
    for l in range(N_LAYERS):
        for q in range(N_QUBITS):
            phi, theta, lam = (np.complex64(params[l, q, i]) for i in range(3))
            rz_p = np.array([[np.exp(-0.5j * phi), 0], [0, np.exp(0.5j * phi)]],
                            np.complex64)
            rz_l = np.array([[np.exp(-0.5j * lam), 0], [0, np.exp(0.5j * lam)]],
                            np.complex64)
            c, s = np.cos(0.5 * theta), np.sin(0.5 * theta)
            ry = np.array([[c, -s], [s, c]], np.complex64)
            U = rz_l @ ry @ rz_p
            # reference einsum applies U^T
            st = psi.reshape(2 ** q, 2, -1)
            psi = np.einsum("st,lsr->ltr", U, st).astype(np.complex64).reshape(-1)
        if l < N_LAYERS - 1:
            for q in range(N_QUBITS - 1):
                st = psi.reshape(2 ** q, 2, 2, -1)
                st = np.stack([st[:, 0], np.flip(st[:, 1], axis=1)], axis=1)
                psi = st.reshape(-1)
    return psi


def _features(X: np.ndarray) -> np.ndarray:
    """Phi[b, u] = prod_q (cos(X/2) if bit(11-q) of u is 0 else sin(X/2))."""
    c = np.cos(0.5 * X).astype(np.float32)
    s = np.sin(0.5 * X).astype(np.float32)
    phi = np.ones((B, 1), np.float32)
    for q in range(N_QUBITS):
        phi = np.stack([phi * c[:, q:q + 1], phi * s[:, q:q + 1]],
                       axis=2).reshape(B, -1)
    return phi


def _host_factor(psi: np.ndarray):
    """rho -> Wsym -> parity-ordered Cholesky. Returns (perm, W0, W1) with
    W = L - I per parity block (f32, strictly small)."""
    jj = np.arange(DIM)
    XORm = np.bitwise_xor.outer(jj, jj).astype(np.int32)
    ANDm = np.bitwise_and.outer(jj, jj).astype(np.int32)
    popand = np.zeros((DIM, DIM), np.int8)
    t = ANDm
    for q in range(N_QUBITS):
        popand += (t & 1).astype(np.int8)
        t = t >> 1
    del ANDm, t
    sgn_and = np.where(popand % 2 == 0, np.float32(1), np.float32(-1))
    del popand
    pop = np.zeros(DIM, np.int64)
    for q in range(N_QUBITS):
        pop += (jj >> q) & 1
    sgn = np.where(pop % 2 == 0, np.float32(1), np.float32(-1))
    par = (pop & 1).astype(np.int8)

    M = sgn_and * np.conj(psi)[XORm]          # M[d,k] = sgn(d&k) psi*_{d^k}
    rho = np.real(M @ psi).astype(np.float32)
    del M
    Wsym = (sgn[:, None] * sgn_and) * rho[XORm]
    del sgn_and, XORm

    perm = np.argsort(par, kind="stable")
    Wp = Wsym[np.ix_(perm, perm)]
    del Wsym
    L0 = np.linalg.cholesky(Wp[:HDIM, :HDIM].astype(np.float64))
    L1 = np.linalg.cholesky(Wp[HDIM:, HDIM:].astype(np.float64))
    W0 = (L0 - np.eye(HDIM)).astype(np.float32)
    W1 = (L1 - np.eye(HDIM)).astype(np.float32)
    return perm, W0, W1


def _prune_schedule(W0, W1, budget=0.09):
    """Triangular chunk list per j-block, dropping chunks whose total
    Frobenius mass stays under sqrt(budget) in both parities (the tail
    error this adds is ~1% of the fp8 noise, in quadrature). Blocks are
    scheduled descending so the earliest need the fewest phi chunks."""
    masses = []
    for c in range(NJB):
        for k in range(c // 2, KCH1):
            s0 = float((W0[k * 256:(k + 1) * 256,
                           c * 128:(c + 1) * 128].astype(np.float64) ** 2).sum())
            s1 = float((W1[k * 256:(k + 1) * 256,
                           c * 128:(c + 1) * 128].astype(np.float64) ** 2).sum())
            masses.append((max(s0, s1), c, k, s0, s1))
    masses.sort()
    drop = set()
    a0 = a1 = 0.0
    for mx, c, k, s0, s1 in masses:
        if k == c // 2 or a0 + s0 > budget or a1 + s1 > budget:
            continue
        a0 += s0
        a1 += s1
        drop.add((c, k))
    sched = []
    for c in range(NJB - 1, -1, -1):
        ks = [k for k in range(c // 2, KCH1) if (c, k) not in drop]
        sched.append((c, ks))
    return sched


# ----------------------------------------------------------------------------
# Pass 1: tail^T = W^T Phi^T per parity block, triangular fp8 DoubleRow.
# Core cr = 2*bg + p handles batch-group bg (1024 samples), parity p.
# ----------------------------------------------------------------------------


def _build_pass1(sched) -> bass.Bass:
    nchunk = sum(len(ks) for _, ks in sched)
    nc = bass.Bass("TRN2", target_bir_lowering=False, debug=False,
                   num_devices=NCORES)
    # w8[p, idx, i, c]: chunk list in sched order; chunk (k, cblk) holds
    # lam_w * W[k*256 + i*128 + p, cblk*128 + c]
    w_d = nc.dram_tensor("w8", [128, nchunk, 2, 128], f8,
                         kind="ExternalInput").ap()
    # phi[p, k, i, b] = lam_p * Phi^T[k*256 + i*128 + p, bg*1024 + b]
    phi_d = nc.dram_tensor("phi", [128, KCH1, 2, BG], f8,
                           kind="ExternalInput").ap()
    # t[p, pos, b] = lam_p*lam_w * tail^T[cblk(pos)*128 + p, bg*1024 + b]
    t_d = nc.dram_tensor("t", [128, NJB, BG], f8, kind="ExternalOutput").ap()

    # group blocks in fours for phi/W DMA batching and output batching
    gstart = [0]
    for c, ks in sched:
        gstart.append(gstart[-1] + len(ks))

    with tile.TileContext(nc) as tc:
        with (
            tc.tile_pool(name="wpool", bufs=1) as wpool,
            tc.tile_pool(name="spool", bufs=1) as spool,
            tc.tile_pool(name="psumw", bufs=1, space="PSUM") as psumw,
            tc.tile_pool(name="psum", bufs=3, space="PSUM") as psum,
        ):
            wa = wpool.tile([128, 2, 128], f8, tag="wa")
            wb = wpool.tile([128, 2, 512], f8, tag="wb")
            nc.vector.memset(wa[:], 0.0)
            nc.gpsimd.memset(wb[:], 0.0)
            wps = psumw.tile([128, 512], f32, tag="ps0", name="warm")
            for i in range(WARMUP1):
                nc.tensor.matmul(wps[:], wa[:], wb[:], start=True, stop=True,
                                 perf_mode=mybir.MatmulPerfMode.DoubleRow)

            w8 = wpool.tile([128, nchunk, 2, 128], f8, tag="w8")
            phi = wpool.tile([128, KCH1, 2, BG], f8, tag="phi")
            st = spool.tile([128, NJB, BG], f8, tag="st")

            # in-DMA stream: per group of 4 blocks, the two new phi chunks
            # then the group's W chunks; all on the in-order SP queue.
            # Output stores are emitted later (inside the block loop) on the
            # same queue: inputs always win the DMA engines.
            for g in range(4):
                klo = 6 - 2 * g
                nc.sync.dma_start(phi[:, klo:klo + 2], phi_d[:, klo:klo + 2])
                i0, i1 = gstart[4 * g], gstart[4 * g + 4]
                nc.sync.dma_start(w8[:, i0:i1], w_d[:, i0:i1])

            for pos, (c, ks) in enumerate(sched):
                ps0 = psum.tile([128, 512], f32, tag="psA", name=f"psA_{c}")
                ps1 = psum.tile([128, 512], f32, tag="psB", name=f"psB_{c}")
                i0 = gstart[pos]
                for j, k in enumerate(ks):
                    st_mm = (j == 0)
                    sp_mm = (j == len(ks) - 1)
                    wch = w8[:, i0 + j]
                    nc.tensor.matmul(ps0[:], wch, phi[:, k, :, :512],
                                     start=st_mm, stop=sp_mm,
                                     perf_mode=mybir.MatmulPerfMode.DoubleRow)
                    nc.tensor.matmul(ps1[:], wch, phi[:, k, :, 512:],
                                     start=st_mm, stop=sp_mm,
                                     perf_mode=mybir.MatmulPerfMode.DoubleRow)
                # psum values are bounded by fp8 range via lam_w: plain copy
                nc.scalar.copy(st[:, pos, :512], ps0[:])
                nc.vector.tensor_copy(st[:, pos, 512:], ps1[:])
                if pos % 4 == 3:
                    g = pos // 4
                    nc.sync.dma_start(t_d[:, 4 * g:4 * g + 4],
                                      st[:, 4 * g:4 * g + 4])
    return nc


# ----------------------------------------------------------------------------
# Pass 2: single-product Gram + norm-corrected square, fp8 DoubleRow.
# ----------------------------------------------------------------------------


def _build_pass2() -> bass.Bass:
    nc = bass.Bass("TRN2", target_bir_lowering=False, debug=False,
                   num_devices=NCORES)
    # mv[p, kc, i, f]: Z8^T chunk of own rows (moving operand; also the
    # stationary operand for the 4 diagonal column blocks)
    mv_d = nc.dram_tensor("mv8", [128, KCH, 2, BLK], f8,
                          kind="ExternalInput").ap()
    # wt[n, p, kc, i, c]: Z8^T of off-diagonal column block n (stationary)
    wt_d = nc.dram_tensor("wt8", [NBLK - 4, 128, KCH, 2, 128], f8,
                          kind="ExternalInput").ap()
    sig_d = nc.dram_tensor("sig", [128, NBLK], f32, kind="ExternalInput").ap()
    wrow_d = nc.dram_tensor("wrow", [1, BLK], f32, kind="ExternalInput").ap()
    # ko[pos, p, f]: K[rows, col block ORDER2[pos], col p].T
    ko_d = nc.dram_tensor("ko", [NBLK, 128, BLK], bf16,
                          kind="ExternalOutput").ap()

    with tile.TileContext(nc) as tc:
        with (
            tc.tile_pool(name="mv", bufs=1) as mpool,
            tc.tile_pool(name="wt", bufs=4) as wpool,
            tc.tile_pool(name="post", bufs=3) as qpool,
            tc.tile_pool(name="psum", bufs=2, space="PSUM") as ppool,
        ):
            mv = mpool.tile([128, KCH, 2, BLK], f8, tag="mv")
            sig = mpool.tile([128, NBLK], f32, tag="sig")
            wrow = mpool.tile([128, BLK], f32, tag="wrow")
            # mv streams in 8 chunks so the opening diagonal blocks can
            # chase it; wt panels follow just-in-time inside the block loop
            # (4-buffer lookahead), all on the in-order SP queue
            for h in range(8):
                nc.sync.dma_start(mv[:, 2 * h:2 * h + 2],
                                  mv_d[:, 2 * h:2 * h + 2])
                if h == 0:
                    nc.sync.dma_start(sig[:], sig_d)
                    nc.sync.dma_start(wrow[:],
                                      wrow_d[0].partition_broadcast(128))
            offdiag = [n for n in ORDER2 if n < 16]
            wts = {}

            def fetch_wt(i):
                if i >= len(offdiag):
                    return
                n = offdiag[i]
                wt = wpool.tile([128, KCH, 2, 128], f8, tag="wt",
                                name=f"wt_{n}")
                nc.sync.dma_start(wt[:], wt_d[n])
                wts[n] = wt

            for i in range(4):
                fetch_wt(i)

            wa = mpool.tile([128, 2, 128], f8, tag="wa")
            wb = mpool.tile([128, 2, 512], f8, tag="wb")
            nc.vector.memset(wa[:], 0.0)
            nc.gpsimd.memset(wb[:], 0.0)
            wps = ppool.tile([128, BLK], f32, tag="mw", name="warm")
            for i in range(WARMUP2):
                nc.tensor.matmul(wps[:], wa[:], wb[:], start=True, stop=True,
                                 perf_mode=mybir.MatmulPerfMode.DoubleRow)

            def post(ps, pos, fsl, fo):
                sq = qpool.tile([128, BLK], f32, tag="sq",
                                name=f"sq_{pos}_{fo}")
                nc.scalar.activation(sq[:, :fsl], ps[:, :fsl],
                                     mybir.ActivationFunctionType.Square,
                                     scale=sig[:, pos:pos + 1])
                ko = qpool.tile([128, BLK], bf16, tag="ko",
                                name=f"ko_{pos}_{fo}")
                nc.vector.tensor_tensor(ko[:, :fsl], sq[:, :fsl],
                                        wrow[:, fo:fo + fsl],
                                        mybir.AluOpType.mult)
                # store trigger rides the idle Pool queue (SWDGE): the SP
                # input stream is untouched and no busy engine stalls
                nc.gpsimd.dma_start(ko_d[pos, :, fo:fo + fsl], ko[:, :fsl])

            nwt = 4
            for pos in range(NBLK):
                n = ORDER2[pos]
                halves = ((0, BLK),) if pos < NBLK - 1 else ((0, 256),
                                                            (256, 256))
                for fo, fsl in halves:
                    ps = ppool.tile([128, BLK], f32, tag=f"m{pos % 2}",
                                    name=f"m_{pos}_{fo}")
                    for k in range(KCH):
                        stat = (wts[n][:, k] if n < 16
                                else mv[:, k, :, (n - 16) * 128:(n - 15) * 128])
                        nc.tensor.matmul(
                            ps[:, :fsl], stat, mv[:, k, :, fo:fo + fsl],
                            start=(k == 0), stop=(k == KCH - 1),
                            perf_mode=mybir.MatmulPerfMode.DoubleRow)
                    post(ps, pos, fsl, fo)
                if n < 16:
                    # refill the 4-deep wt pipeline now that this block's
                    # matmuls guard the recycled buffer
                    fetch_wt(nwt)
                    nwt += 1
    return nc


_nc1 = None
_nc2 = None

PROFILE = False
LAST_PROFILE: dict = {}


def kernel(X: np.ndarray, params: np.ndarray) -> np.ndarray:
    global _nc1, _nc2
    _install_waitfix()
    X = np.asarray(X, np.float32)
    params = np.asarray(params, np.float32)

    # ---- host precompute -------------------------------------------------
    psi = _host_psi(params)
    phi = _features(X)                           # (B, DIM) f32
    perm, W0, W1 = _host_factor(psi)
    sched = _prune_schedule(W0, W1)
    phiT = np.ascontiguousarray(phi[:, perm].T)  # (DIM parity-ordered, B)

    lam_p = 64.0
    # bound |tail| <= max column norm of W so psum fits fp8 range directly
    bnd0 = float(np.sqrt((W0.astype(np.float64) ** 2).sum(axis=0).max()))
    bnd1 = float(np.sqrt((W1.astype(np.float64) ** 2).sum(axis=0).max()))
    lam_w0 = 400.0 / (lam_p * max(bnd0, 1e-30))
    lam_w1 = 400.0 / (lam_p * max(bnd1, 1e-30))
    phi8 = (phiT * lam_p).astype(npf8)           # (DIM, B)

    nchunk = sum(len(ks) for _, ks in sched)

    def pack_w(W, lam_w):
        W8 = (W * lam_w).astype(npf8)            # (HDIM u, HDIM j)
        out = np.empty((128, nchunk, 2, 128), npf8)
        idx = 0
        for c, ks in sched:
            for k in ks:
                ch = W8[k * 256:(k + 1) * 256, c * 128:(c + 1) * 128]
                out[:, idx] = ch.reshape(2, 128, 128).transpose(1, 0, 2)
                idx += 1
        return out

    w_par = [pack_w(W0, lam_w0), pack_w(W1, lam_w1)]
    phi_par = []
    for p in range(2):
        rows = phi8[p * HDIM:(p + 1) * HDIM]     # (HDIM, B)
        phi_par.append(rows.reshape(KCH1, 2, 128, B).transpose(2, 0, 1, 3))

    in_maps1 = []
    for cr in range(NCORES):
        bg, p = divmod(cr, 2)
        in_maps1.append({
            "w8": w_par[p],
            "phi": np.ascontiguousarray(phi_par[p][:, :, :,
                                                   bg * BG:(bg + 1) * BG]),
        })

    if _nc1 is None:
        _nc1 = _build_pass1(sched)
    res1 = run_bass_kernel_spmd(_nc1, in_maps1, core_ids=list(range(NCORES)))

    # ---- host mid: assemble Z, quantize ----------------------------------
    ZT = phiT                                    # reuse buffer (DIM, B)
    inv = [1.0 / (lam_p * lam_w0), 1.0 / (lam_p * lam_w1)]
    pos2c = [c for c, _ in sched]
    for cr in range(NCORES):
        bg, p = divmod(cr, 2)
        t = res1.results[cr]["t"].astype(np.float32) * inv[p]   # (128,16,1024)
        for pos in range(NJB):
            c = pos2c[pos]
            ZT[p * HDIM + c * 128:p * HDIM + (c + 1) * 128,
               bg * BG:(bg + 1) * BG] += t[:, pos]

    Z8 = (ZT * LAM).astype(npf8)                 # (DIM, B)
    Z8f32 = Z8.astype(np.float32)
    rho2 = np.einsum("jb,jb->b", Z8f32, Z8f32) / (LAM * LAM)    # (B,)
    del Z8f32
    sig_all = (1.0 / (LAM * LAM * np.sqrt(rho2))).astype(np.float32)
    wrow_all = (1.0 / rho2).astype(np.float32)

    # strip layout: 16 off-diagonal col blocks (strip offsets 512..2560)
    # DMA'd as wt; the 4 diagonal blocks (offsets 0..512) slice mv.
    colrel = np.concatenate([np.arange(BLK, NB_COLS), np.arange(0, BLK)])
    Z8c = Z8.reshape(KCH, 2, 128, B)
    in_maps2 = []
    for cr in range(NCORES):
        cols = (cr * BLK + colrel) % B
        mvc = Z8c[:, :, :, cr * BLK:(cr + 1) * BLK].transpose(2, 0, 1, 3)
        wtc = Z8c[:, :, :, cols[:16 * 128]].reshape(
            KCH, 2, 128, 16, 128).transpose(3, 2, 0, 1, 4)
        sig_blocks = sig_all[cols].reshape(NBLK, 128)    # by block n
        sig = sig_blocks[ORDER2].T                       # (128, pos)
        wrow = wrow_all[cr * BLK:(cr + 1) * BLK][None, :]
        in_maps2.append({
            "mv8": np.ascontiguousarray(mvc),
            "wt8": np.ascontiguousarray(wtc),
            "sig": np.ascontiguousarray(sig),
            "wrow": np.ascontiguousarray(wrow),
        })

    if _nc2 is None:
        _nc2 = _build_pass2()
    res2 = run_bass_kernel_spmd(_nc2, in_maps2, core_ids=list(range(NCORES)))

    # ---- assemble K (with symmetric mirroring) ---------------------------
    K = np.empty((B, B), np.float32)
    for cr in range(NCORES):
        ko = res2.results[cr]["ko"].astype(np.float32)   # (pos, 128, BLK)
        rows = slice(cr * BLK, (cr + 1) * BLK)
        for pos in range(NBLK):
            n = ORDER2[pos]
            gs = (cr * BLK + int(colrel[n * 128])) % B
            colsl = slice(gs, gs + 128)
            K[rows, colsl] = ko[pos].T
            d = 1 + n // 4 if n < 16 else 0
            if 0 < d < 4 or (d == 4 and cr < 4):
                K[colsl, rows] = ko[pos]
    return K


# revision 20
# speedup vs baseline: 3.7652x; 1.0109x over previous
"""Trainium2 Bass kernel for nn_NeuralQKM: K[i,j] = |<psi_i|psi_j>|^2.

Math. States factor as S = Phi C with product features
Phi_b[u] = prod_q (cos(X/2) if u_q=0 else sin(X/2)) and a fixed complex
matrix C[u,j] = (-1)^{|j&u|} psi'[j^u] (psi' = state after all shared
gates; the final CNOT chain is a common permutation and drops out).
The Gram G = S S^H = Phi (C C^H) Phi^T where

    (C C^H)[u,u'] = (-1)^{|u&d|} rho(d),  d = u^u',
    rho(d) = sum_k (-1)^{|k&d|} psi'[k] conj(psi'[k^d]),

so Re G = Phi Wsym Phi^T with Wsym real symmetric PSD, and Re rho(d) = 0
for odd |d| makes Wsym parity-block-diagonal. Im G vanishes on the
diagonal and contributes O(1e-6) to ||K||_F: K ~= (Re G)^2 elementwise.

Cholesky per parity block, Wsym = L L^T, gives Re G = Z Z^T with
Z = Phi L of exactly unit row norm. W = L - I is small (params are
tiny), so Z = Phi + Phi W: the main term is exact host math and only the
tail needs the device, which tolerates fp8.

Device pass 1 (4 batch-groups x 2 parities): tail^T = W^T Phi^T per
parity block, fp8 DoubleRow, keeping only the lower-triangular W chunks
whose Frobenius mass matters (~40 of 128). lam_w is sized so psum values
fit fp8 range directly: the tail streams out as fp8 with a plain copy.
Output stores ride the in-order SP queue after every input DMA so they
never preempt the input stream on the shared DMA engines.

Device pass 2 (row-sharded, block-cyclic symmetric): single-product Gram
ps = Z8_cols . Z8_rows^T; post-ops square with a per-state norm
correction K = ps^2/(LAM^4 rho_c^2 rho_r^2) (rho^2 = ||quantized Z||^2,
host-known), which cancels the dominant fp8 radial error. Diagonal
column blocks slice mv directly as the stationary operand (no wt DMA)
and two of them open the pass so compute starts after one mv chunk.
Output per core is the transposed block strip K[rows, cols].T in bf16;
host mirrors the symmetric blocks.
"""
import numpy as np
import ml_dtypes
import orjson

import concourse.bass as bass
import concourse.mybir as mybir
import concourse.tile as tile
from concourse.bass_utils import run_bass_kernel_spmd

N_QUBITS = 12
N_LAYERS = 5
DIM = 2 ** N_QUBITS          # 4096
HDIM = DIM // 2              # 2048 per parity block
B = 4096
NCORES = 8
BLK = B // NCORES            # 512 rows per core in pass 2
NDBLK = 5                    # diagonal + 4 off-diagonal column blocks
NB_COLS = NDBLK * BLK        # 2560 rhs columns per core
NBLK = NB_COLS // 128        # 20 column blocks of 128
KCH = DIM // 256             # 16 contraction chunks of K=256 (DoubleRow)
KCH1 = HDIM // 256           # 8 contraction chunks in pass 1
NJB = HDIM // 128            # 16 output column blocks in pass 1
BG = B // 4                  # 1024 samples per pass-1 batch-group
LAM = 64.0                   # fp8 quantization scale for state planes
WARMUP1 = 15                 # PE warmup matmuls, pass 1 (sim-tuned)
WARMUP2 = 11                 # PE warmup matmuls, pass 2 (sim-tuned)

# pass-2 block order: all four free-stationary diagonal blocks open the
# pass, chunk-interleaved so they chase the streaming mv chunks — PE has
# ~7us of work before the first wt panel can possibly arrive
ORDER2 = [16, 17, 18, 19] + list(range(16))

f32 = mybir.dt.float32
f8 = mybir.dt.float8e4
bf16 = mybir.dt.bfloat16
npf8 = ml_dtypes.float8_e4m3
npbf = ml_dtypes.bfloat16

# ----------------------------------------------------------------------------
# walrus in this toolchain rejects >1 sync-wait per instruction; Tile emits
# several. Engines are serial, so an extra wait is equivalent to a standalone
# EventSemaphore wait right before the instruction on the same engine.
# ----------------------------------------------------------------------------


def _legalize_multiwait_json(bir: bytes) -> bytes:
    m = orjson.loads(bir)
    changed = False
    for func in m.get("functions", []):
        for blk in func.get("blocks", []):
            out = []
            for inst in blk.get("instructions", []):
                sync = inst.get("sync_info")
                waits = (sync or {}).get("on_wait") or []
                if len(waits) > 1:
                    changed = True
                    for i, w in enumerate(waits[:-1]):
                        out.append({
                            "debug": inst.get("debug", 0),
                            "engine": inst["engine"],
                            "ins": [],
                            "name": f"{inst['name']}-xw{i}",
                            "opcode": "EventSemaphore",
                            "outs": [],
                            "sync_info": {"on_update": [], "on_wait": [w]},
                        })
                    sync["on_wait"] = [waits[-1]]
                out.append(inst)
            blk["instructions"] = out
    return orjson.dumps(m) if changed else bir


_patched = False


def _install_waitfix():
    global _patched
    if _patched:
        return
    _patched = True
    orig = bass.Bass.to_json_bytes

    def patched(self):
        return _legalize_multiwait_json(orig(self))

    bass.Bass.to_json_bytes = patched


# ----------------------------------------------------------------------------
# Host math: psi' (state after all shared circuit parts), complex64 to track
# the reference's precision.
# ----------------------------------------------------------------------------


def _host_psi(params: np.ndarray) -> np.ndarray:
    params = np.asarray(params, np.float32)
    psi = np.zeros(DIM, np.complex64)
    psi[0] = 1.0
    for l in range(N_LAYERS):
        for q in range(N_QUBITS):
            phi, theta, lam = (np.complex64(params[l, q, i]) for i in range(3))
            rz_p = np.array([[np.exp(-0.5j * phi), 0], [0, np.exp(0.5j * phi)]],
                            np.complex64)
            rz_l = np.array([[np.exp(-0.5j * lam), 0], [0, np.exp(0.5j * lam)]],
                            np.complex64)
            c, s = np.cos(0.5 * theta), np.sin(0.5 * theta)
            ry = np.array([[c, -s], [s, c]], np.complex64)
            U = rz_l @ ry @ rz_p
            # reference einsum applies U^T
            st = psi.reshape(2 ** q, 2, -1)
            psi = np.einsum("st,lsr->ltr", U, st).astype(np.complex64).reshape(-1)
        if l < N_LAYERS - 1:
            for q in range(N_QUBITS - 1):
                st = psi.reshape(2 ** q, 2, 2, -1)
                st = np.stack([st[:, 0], np.flip(st[:, 1], axis=1)], axis=1)
                psi = st.reshape(-1)
    return psi


def _features(X: np.ndarray) -> np.ndarray:
    """Phi[b, u] = prod_q (cos(X/2) if bit(11-q) of u is 0 else sin(X/2))."""
    c = np.cos(0.5 * X).astype(np.float32)
    s = np.sin(0.5 * X).astype(np.float32)
    phi = np.ones((B, 1), np.float32)
    for q in range(N_QUBITS):
        phi = np.stack([phi * c[:, q:q + 1], phi * s[:, q:q + 1]],
                       axis=2).reshape(B, -1)
    return phi


def _host_factor(psi: np.ndarray):
    """rho -> Wsym -> parity-ordered Cholesky. Returns (perm, W0, W1) with
    W = L - I per parity block (f32, strictly small)."""
    jj = np.arange(DIM)
    XORm = np.bitwise_xor.outer(jj, jj).astype(np.int32)
    ANDm = np.bitwise_and.outer(jj, jj).astype(np.int32)
    popand = np.zeros((DIM, DIM), np.int8)
    t = ANDm
    for q in range(N_QUBITS):
        popand += (t & 1).astype(np.int8)
        t = t >> 1
    del ANDm, t
    sgn_and = np.where(popand % 2 == 0, np.float32(1), np.float32(-1))
    del popand
    pop = np.zeros(DIM, np.int64)
    for q in range(N_QUBITS):
        pop += (jj >> q) & 1
    sgn = np.where(pop % 2 == 0, np.float32(1), np.float32(-1))
    par = (pop & 1).astype(np.int8)

    M = sgn_and * np.conj(psi)[XORm]          # M[d,k] = sgn(d&k) psi*_{d^k}
    rho = np.real(M @ psi).astype(np.float32)
    del M
    Wsym = (sgn[:, None] * sgn_and) * rho[XORm]
    del sgn_and, XORm

    perm = np.argsort(par, kind="stable")
    Wp = Wsym[np.ix_(perm, perm)]
    del Wsym
    L0 = np.linalg.cholesky(Wp[:HDIM, :HDIM].astype(np.float64))
    L1 = np.linalg.cholesky(Wp[HDIM:, HDIM:].astype(np.float64))
    W0 = (L0 - np.eye(HDIM)).astype(np.float32)
    W1 = (L1 - np.eye(HDIM)).astype(np.float32)
    return perm, W0, W1


def _prune_schedule(W0, W1, budget=0.09):
    """Triangular chunk list per j-block, dropping chunks whose total
    Frobenius mass stays under sqrt(budget) in both parities (the tail
    error this adds is ~1% of the fp8 noise, in quadrature). Blocks are
    scheduled descending so the earliest need the fewest phi chunks."""
    masses = []
    for c in range(NJB):
        for k in range(c // 2, KCH1):
            s0 = float((W0[k * 256:(k + 1) * 256,
                           c * 128:(c + 1) * 128].astype(np.float64) ** 2).sum())
            s1 = float((W1[k * 256:(k + 1) * 256,
                           c * 128:(c + 1) * 128].astype(np.float64) ** 2).sum())
            masses.append((max(s0, s1), c, k, s0, s1))
    masses.sort()
    drop = set()
    a0 = a1 = 0.0
    for mx, c, k, s0, s1 in masses:
        if k == c // 2 or a0 + s0 > budget or a1 + s1 > budget:
            continue
        a0 += s0
        a1 += s1
        drop.add((c, k))
    sched = []
    for c in range(NJB - 1, -1, -1):
        ks = [k for k in range(c // 2, KCH1) if (c, k) not in drop]
        sched.append((c, ks))
    return sched


# ----------------------------------------------------------------------------
# Pass 1: tail^T = W^T Phi^T per parity block, triangular fp8 DoubleRow.
# Core cr = 2*bg + p handles batch-group bg (1024 samples), parity p.
# ----------------------------------------------------------------------------


def _build_pass1(sched) -> bass.Bass:
    nchunk = sum(len(ks) for _, ks in sched)
    nc = bass.Bass("TRN2", target_bir_lowering=False, debug=False,
                   num_devices=NCORES)
    # w8[p, idx, i, c]: chunk list in sched order; chunk (k, cblk) holds
    # lam_w * W[k*256 + i*128 + p, cblk*128 + c]
    w_d = nc.dram_tensor("w8", [128, nchunk, 2, 128], f8,
                         kind="ExternalInput").ap()
    # phi[p, k, i, b] = lam_p * Phi^T[k*256 + i*128 + p, bg*1024 + b]
    phi_d = nc.dram_tensor("phi", [128, KCH1, 2, BG], f8,
                           kind="ExternalInput").ap()
    # t[p, pos, b] = lam_p*lam_w * tail^T[cblk(pos)*128 + p, bg*1024 + b]
    t_d = nc.dram_tensor("t", [128, NJB, BG], f8, kind="ExternalOutput").ap()

    # group blocks in fours for phi/W DMA batching and output batching
    gstart = [0]
    for c, ks in sched:
        gstart.append(gstart[-1] + len(ks))

    with tile.TileContext(nc) as tc:
        with (
            tc.tile_pool(name="wpool", bufs=1) as wpool,
            tc.tile_pool(name="spool", bufs=1) as spool,
            tc.tile_pool(name="psumw", bufs=1, space="PSUM") as psumw,
            tc.tile_pool(name="psum", bufs=3, space="PSUM") as psum,
        ):
            wa = wpool.tile([128, 2, 128], f8, tag="wa")
            wb = wpool.tile([128, 2, 512], f8, tag="wb")
            nc.vector.memset(wa[:], 0.0)
            nc.gpsimd.memset(wb[:], 0.0)
            wps = psumw.tile([128, 512], f32, tag="ps0", name="warm")
            for i in range(WARMUP1):
                nc.tensor.matmul(wps[:], wa[:], wb[:], start=True, stop=True,
                                 perf_mode=mybir.MatmulPerfMode.DoubleRow)

            w8 = wpool.tile([128, nchunk, 2, 128], f8, tag="w8")
            phi = wpool.tile([128, KCH1, 2, BG], f8, tag="phi")
            st = spool.tile([128, NJB, BG], f8, tag="st")

            # in-DMA stream: per group of 4 blocks, each new phi chunk then
            # the W chunks of the two blocks it unlocks; all on the in-order
            # SP queue. Output stores are emitted later (inside the block
            # loop) on the same queue: inputs always win the DMA engines.
            for g in range(4):
                klo = 6 - 2 * g
                i0, im = gstart[4 * g], gstart[4 * g + 2]
                i1 = gstart[4 * g + 4]
                nc.sync.dma_start(phi[:, klo + 1], phi_d[:, klo + 1])
                nc.sync.dma_start(w8[:, i0:im], w_d[:, i0:im])
                nc.sync.dma_start(phi[:, klo], phi_d[:, klo])
                nc.sync.dma_start(w8[:, im:i1], w_d[:, im:i1])

            for pos, (c, ks) in enumerate(sched):
                ps0 = psum.tile([128, 512], f32, tag="psA", name=f"psA_{c}")
                ps1 = psum.tile([128, 512], f32, tag="psB", name=f"psB_{c}")
                i0 = gstart[pos]
                for j, k in enumerate(ks):
                    st_mm = (j == 0)
                    sp_mm = (j == len(ks) - 1)
                    wch = w8[:, i0 + j]
                    nc.tensor.matmul(ps0[:], wch, phi[:, k, :, :512],
                                     start=st_mm, stop=sp_mm,
                                     perf_mode=mybir.MatmulPerfMode.DoubleRow)
                    nc.tensor.matmul(ps1[:], wch, phi[:, k, :, 512:],
                                     start=st_mm, stop=sp_mm,
                                     perf_mode=mybir.MatmulPerfMode.DoubleRow)
                # psum values are bounded by fp8 range via lam_w: plain copy
                nc.scalar.copy(st[:, pos, :512], ps0[:])
                nc.vector.tensor_copy(st[:, pos, 512:], ps1[:])
                if pos >= 12:
                    # last group: per-block stores so the tail is one block
                    nc.sync.dma_start(t_d[:, pos:pos + 1], st[:, pos:pos + 1])
                elif pos % 4 == 3:
                    g = pos // 4
                    nc.sync.dma_start(t_d[:, 4 * g:4 * g + 4],
                                      st[:, 4 * g:4 * g + 4])
    return nc


# ----------------------------------------------------------------------------
# Pass 2: single-product Gram + norm-corrected square, fp8 DoubleRow.
# ----------------------------------------------------------------------------


def _build_pass2() -> bass.Bass:
    nc = bass.Bass("TRN2", target_bir_lowering=False, debug=False,
                   num_devices=NCORES)
    # mv[p, kc, i, f]: Z8^T chunk of own rows (moving operand; also the
    # stationary operand for the 4 diagonal column blocks)
    mv_d = nc.dram_tensor("mv8", [128, KCH, 2, BLK], f8,
                          kind="ExternalInput").ap()
    # wt[n, p, kc, i, c]: Z8^T of off-diagonal column block n (stationary)
    wt_d = nc.dram_tensor("wt8", [NBLK - 4, 128, KCH, 2, 128], f8,
                          kind="ExternalInput").ap()
    sig_d = nc.dram_tensor("sig", [128, NBLK], f32, kind="ExternalInput").ap()
    wrow_d = nc.dram_tensor("wrow", [1, BLK], f32, kind="ExternalInput").ap()
    # ko[pos, p, f]: K[rows, col block ORDER2[pos], col p].T
    ko_d = nc.dram_tensor("ko", [NBLK, 128, BLK], bf16,
                          kind="ExternalOutput").ap()

    with tile.TileContext(nc) as tc:
        with (
            tc.tile_pool(name="mv", bufs=1) as mpool,
            tc.tile_pool(name="wt", bufs=4) as wpool,
            tc.tile_pool(name="post", bufs=3) as qpool,
            tc.tile_pool(name="psumd", bufs=1, space="PSUM") as dpool,
            tc.tile_pool(name="psum", bufs=2, space="PSUM") as ppool,
        ):
            mv = mpool.tile([128, KCH, 2, BLK], f8, tag="mv")
            sig = mpool.tile([128, NBLK], f32, tag="sig")
            wrow = mpool.tile([128, BLK], f32, tag="wrow")
            # mv streams in 8 chunks so the opening diagonal blocks can
            # chase it; wt panels follow just-in-time inside the block loop
            # (4-buffer lookahead), all on the in-order SP queue
            for h in range(8):
                nc.sync.dma_start(mv[:, 2 * h:2 * h + 2],
                                  mv_d[:, 2 * h:2 * h + 2])
                if h == 0:
                    nc.sync.dma_start(sig[:], sig_d)
                    nc.sync.dma_start(wrow[:],
                                      wrow_d[0].partition_broadcast(128))
            wts = {}

            def fetch_wt(n):
                if n >= 16:
                    return
                wt = wpool.tile([128, KCH, 2, 128], f8, tag="wt",
                                name=f"wt_{n}")
                nc.sync.dma_start(wt[:], wt_d[n])
                wts[n] = wt

            for i in range(4):
                fetch_wt(i)

            wa = mpool.tile([128, 2, 128], f8, tag="wa")
            wb = mpool.tile([128, 2, 512], f8, tag="wb")
            nc.vector.memset(wa[:], 0.0)
            nc.gpsimd.memset(wb[:], 0.0)
            wps = ppool.tile([128, BLK], f32, tag="m0", name="warm")
            for i in range(WARMUP2):
                nc.tensor.matmul(wps[:], wa[:], wb[:], start=True, stop=True,
                                 perf_mode=mybir.MatmulPerfMode.DoubleRow)

            def post(ps, pos, fsl, fo):
                sq = qpool.tile([128, BLK], f32, tag="sq",
                                name=f"sq_{pos}_{fo}")
                nc.scalar.activation(sq[:, :fsl], ps[:, :fsl],
                                     mybir.ActivationFunctionType.Square,
                                     scale=sig[:, pos:pos + 1])
                ko = qpool.tile([128, BLK], bf16, tag="ko",
                                name=f"ko_{pos}_{fo}")
                nc.vector.tensor_tensor(ko[:, :fsl], sq[:, :fsl],
                                        wrow[:, fo:fo + fsl],
                                        mybir.AluOpType.mult)
                # store trigger rides the idle Pool queue (SWDGE): the SP
                # input stream is untouched and no busy engine stalls
                nc.gpsimd.dma_start(ko_d[pos, :, fo:fo + fsl], ko[:, :fsl])

            # positions 0-3: the four diagonal blocks, k-interleaved so the
            # whole opening chases the mv chunk stream
            dps = [dpool.tile([128, BLK], f32, tag=f"d{d}", name=f"dps_{d}")
                   for d in range(4)]
            for k in range(KCH):
                for d in range(4):
                    nc.tensor.matmul(
                        dps[d][:], mv[:, k, :, d * 128:(d + 1) * 128],
                        mv[:, k], start=(k == 0), stop=(k == KCH - 1),
                        perf_mode=mybir.MatmulPerfMode.DoubleRow)
            for d in range(4):
                post(dps[d], d, BLK, 0)

            # positions 4..19: off-diagonal blocks on the wt stream
            for pos in range(4, NBLK):
                n = ORDER2[pos]
                halves = ((0, BLK),) if pos < NBLK - 1 else ((0, 256),
                                                            (256, 256))
                for fo, fsl in halves:
                    ps = ppool.tile([128, BLK], f32, tag=f"m{pos % 2}",
                                    name=f"m_{pos}_{fo}")
                    for k in range(KCH):
                        nc.tensor.matmul(
                            ps[:, :fsl], wts[n][:, k], mv[:, k, :, fo:fo + fsl],
                            start=(k == 0), stop=(k == KCH - 1),
                            perf_mode=mybir.MatmulPerfMode.DoubleRow)
                    post(ps, pos, fsl, fo)
                # refill the 4-deep wt pipeline now that this block's
                # matmuls guard the recycled buffer
                fetch_wt(n + 4)
    return nc


_nc1 = None
_nc2 = None

PROFILE = False
LAST_PROFILE: dict = {}


def kernel(X: np.ndarray, params: np.ndarray) -> np.ndarray:
    global _nc1, _nc2
    _install_waitfix()
    X = np.asarray(X, np.float32)
    params = np.asarray(params, np.float32)

    # ---- host precompute -------------------------------------------------
    psi = _host_psi(params)
    phi = _features(X)                           # (B, DIM) f32
    perm, W0, W1 = _host_factor(psi)
    sched = _prune_schedule(W0, W1)
    phiT = np.ascontiguousarray(phi[:, perm].T)  # (DIM parity-ordered, B)

    lam_p = 64.0
    # bound |tail| <= max column norm of W so psum fits fp8 range directly
    bnd0 = float(np.sqrt((W0.astype(np.float64) ** 2).sum(axis=0).max()))
    bnd1 = float(np.sqrt((W1.astype(np.float64) ** 2).sum(axis=0).max()))
    lam_w0 = 400.0 / (lam_p * max(bnd0, 1e-30))
    lam_w1 = 400.0 / (lam_p * max(bnd1, 1e-30))
    phi8 = (phiT * lam_p).astype(npf8)           # (DIM, B)

    nchunk = sum(len(ks) for _, ks in sched)

    def pack_w(W, lam_w):
        W8 = (W * lam_w).astype(npf8)            # (HDIM u, HDIM j)
        out = np.empty((128, nchunk, 2, 128), npf8)
        idx = 0
        for c, ks in sched:
            for k in ks:
                ch = W8[k * 256:(k + 1) * 256, c * 128:(c + 1) * 128]
                out[:, idx] = ch.reshape(2, 128, 128).transpose(1, 0, 2)
                idx += 1
        return out

    w_par = [pack_w(W0, lam_w0), pack_w(W1, lam_w1)]
    phi_par = []
    for p in range(2):
        rows = phi8[p * HDIM:(p + 1) * HDIM]     # (HDIM, B)
        phi_par.append(rows.reshape(KCH1, 2, 128, B).transpose(2, 0, 1, 3))

    in_maps1 = []
    for cr in range(NCORES):
        bg, p = divmod(cr, 2)
        in_maps1.append({
            "w8": w_par[p],
            "phi": np.ascontiguousarray(phi_par[p][:, :, :,
                                                   bg * BG:(bg + 1) * BG]),
        })

    if _nc1 is None:
        _nc1 = _build_pass1(sched)
    res1 = run_bass_kernel_spmd(_nc1, in_maps1, core_ids=list(range(NCORES)))

    # ---- host mid: assemble Z, quantize ----------------------------------
    ZT = phiT                                    # reuse buffer (DIM, B)
    inv = [1.0 / (lam_p * lam_w0), 1.0 / (lam_p * lam_w1)]
    pos2c = [c for c, _ in sched]
    for cr in range(NCORES):
        bg, p = divmod(cr, 2)
        t = res1.results[cr]["t"].astype(np.float32) * inv[p]   # (128,16,1024)
        for pos in range(NJB):
            c = pos2c[pos]
            ZT[p * HDIM + c * 128:p * HDIM + (c + 1) * 128,
               bg * BG:(bg + 1) * BG] += t[:, pos]

    Z8 = (ZT * LAM).astype(npf8)                 # (DIM, B)
    Z8f32 = Z8.astype(np.float32)
    rho2 = np.einsum("jb,jb->b", Z8f32, Z8f32) / (LAM * LAM)    # (B,)
    del Z8f32
    sig_all = (1.0 / (LAM * LAM * np.sqrt(rho2))).astype(np.float32)
    wrow_all = (1.0 / rho2).astype(np.float32)

    # strip layout: 16 off-diagonal col blocks (strip offsets 512..2560)
    # DMA'd as wt; the 4 diagonal blocks (offsets 0..512) slice mv.
    colrel = np.concatenate([np.arange(BLK, NB_COLS), np.arange(0, BLK)])
    Z8c = Z8.reshape(KCH, 2, 128, B)
    in_maps2 = []
    for cr in range(NCORES):
        cols = (cr * BLK + colrel) % B
        mvc = Z8c[:, :, :, cr * BLK:(cr + 1) * BLK].transpose(2, 0, 1, 3)
        wtc = Z8c[:, :, :, cols[:16 * 128]].reshape(
            KCH, 2, 128, 16, 128).transpose(3, 2, 0, 1, 4)
        sig_blocks = sig_all[cols].reshape(NBLK, 128)    # by block n
        sig = sig_blocks[ORDER2].T                       # (128, pos)
        wrow = wrow_all[cr * BLK:(cr + 1) * BLK][None, :]
        in_maps2.append({
            "mv8": np.ascontiguousarray(mvc),
            "wt8": np.ascontiguousarray(wtc),
            "sig": np.ascontiguousarray(sig),
            "wrow": np.ascontiguousarray(wrow),
        })

    if _nc2 is None:
        _nc2 = _build_pass2()
    res2 = run_bass_kernel_spmd(_nc2, in_maps2, core_ids=list(range(NCORES)))

    # ---- assemble K (with symmetric mirroring) ---------------------------
    K = np.empty((B, B), np.float32)
    for cr in range(NCORES):
        ko = res2.results[cr]["ko"].astype(np.float32)   # (pos, 128, BLK)
        rows = slice(cr * BLK, (cr + 1) * BLK)
        for pos in range(NBLK):
            n = ORDER2[pos]
            gs = (cr * BLK + int(colrel[n * 128])) % B
            colsl = slice(gs, gs + 128)
            K[rows, colsl] = ko[pos].T
            d = 1 + n // 4 if n < 16 else 0
            if 0 < d < 4 or (d == 4 and cr < 4):
                K[colsl, rows] = ko[pos]
    return K


# revision 22
# speedup vs baseline: 3.7690x; 1.0010x over previous
"""Trainium2 Bass kernel for nn_NeuralQKM: K[i,j] = |<psi_i|psi_j>|^2.

Math. States factor as S = Phi C with product features
Phi_b[u] = prod_q (cos(X/2) if u_q=0 else sin(X/2)) and a fixed complex
matrix C[u,j] = (-1)^{|j&u|} psi'[j^u] (psi' = state after all shared
gates; the final CNOT chain is a common permutation and drops out).
The Gram G = S S^H = Phi (C C^H) Phi^T where

    (C C^H)[u,u'] = (-1)^{|u&d|} rho(d),  d = u^u',
    rho(d) = sum_k (-1)^{|k&d|} psi'[k] conj(psi'[k^d]),

so Re G = Phi Wsym Phi^T with Wsym real symmetric PSD, and Re rho(d) = 0
for odd |d| makes Wsym parity-block-diagonal. Im G vanishes on the
diagonal and contributes O(1e-6) to ||K||_F: K ~= (Re G)^2 elementwise.

Cholesky per parity block, Wsym = L L^T, gives Re G = Z Z^T with
Z = Phi L of exactly unit row norm. W = L - I is small (params are
tiny), so Z = Phi + Phi W: the main term is exact host math and only the
tail needs the device, which tolerates fp8.

Device pass 1 (4 batch-groups x 2 parities): tail^T = W^T Phi^T per
parity block, fp8 DoubleRow, keeping only the lower-triangular W chunks
whose Frobenius mass matters (~40 of 128). lam_w is sized so psum values
fit fp8 range directly: the tail streams out as fp8 with a plain copy.
Output stores ride the in-order SP queue after every input DMA so they
never preempt the input stream on the shared DMA engines.

Device pass 2 (row-sharded, block-cyclic symmetric): single-product Gram
ps = Z8_cols . Z8_rows^T; post-ops square with a per-state norm
correction K = ps^2/(LAM^4 rho_c^2 rho_r^2) (rho^2 = ||quantized Z||^2,
host-known), which cancels the dominant fp8 radial error. Diagonal
column blocks slice mv directly as the stationary operand (no wt DMA)
and two of them open the pass so compute starts after one mv chunk.
Output per core is the transposed block strip K[rows, cols].T in bf16;
host mirrors the symmetric blocks.
"""
import numpy as np
import ml_dtypes
import orjson

import concourse.bass as bass
import concourse.mybir as mybir
import concourse.tile as tile
from concourse.bass_utils import run_bass_kernel_spmd

N_QUBITS = 12
N_LAYERS = 5
DIM = 2 ** N_QUBITS          # 4096
HDIM = DIM // 2              # 2048 per parity block
B = 4096
NCORES = 8
BLK = B // NCORES            # 512 rows per core in pass 2
NDBLK = 5                    # diagonal + 4 off-diagonal column blocks
NB_COLS = NDBLK * BLK        # 2560 rhs columns per core
NBLK = NB_COLS // 128        # 20 column blocks of 128
KCH = DIM // 256             # 16 contraction chunks of K=256 (DoubleRow)
KCH1 = HDIM // 256           # 8 contraction chunks in pass 1
NJB = HDIM // 128            # 16 output column blocks in pass 1
BG = B // 4                  # 1024 samples per pass-1 batch-group
LAM = 64.0                   # fp8 quantization scale for state planes
WARMUP1 = 15                 # PE warmup matmuls, pass 1 (sim-tuned)
WARMUP2 = 11                 # PE warmup matmuls, pass 2 (sim-tuned)

# pass-2 block order: all four free-stationary diagonal blocks open the
# pass, chunk-interleaved so they chase the streaming mv chunks — PE has
# ~7us of work before the first wt panel can possibly arrive
ORDER2 = [16, 17, 18, 19] + list(range(16))

f32 = mybir.dt.float32
f8 = mybir.dt.float8e4
bf16 = mybir.dt.bfloat16
npf8 = ml_dtypes.float8_e4m3
npbf = ml_dtypes.bfloat16

# ----------------------------------------------------------------------------
# walrus in this toolchain rejects >1 sync-wait per instruction; Tile emits
# several. Engines are serial, so an extra wait is equivalent to a standalone
# EventSemaphore wait right before the instruction on the same engine.
# ----------------------------------------------------------------------------


def _legalize_multiwait_json(bir: bytes) -> bytes:
    m = orjson.loads(bir)
    changed = False
    for func in m.get("functions", []):
        for blk in func.get("blocks", []):
            out = []
            for inst in blk.get("instructions", []):
                sync = inst.get("sync_info")
                waits = (sync or {}).get("on_wait") or []
                if len(waits) > 1:
                    changed = True
                    for i, w in enumerate(waits[:-1]):
                        out.append({
                            "debug": inst.get("debug", 0),
                            "engine": inst["engine"],
                            "ins": [],
                            "name": f"{inst['name']}-xw{i}",
                            "opcode": "EventSemaphore",
                            "outs": [],
                            "sync_info": {"on_update": [], "on_wait": [w]},
                        })
                    sync["on_wait"] = [waits[-1]]
                out.append(inst)
            blk["instructions"] = out
    return orjson.dumps(m) if changed else bir


_patched = False


def _install_waitfix():
    global _patched
    if _patched:
        return
    _patched = True
    orig = bass.Bass.to_json_bytes

    def patched(self):
        return _legalize_multiwait_json(orig(self))

    bass.Bass.to_json_bytes = patched


# ----------------------------------------------------------------------------
# Host math: psi' (state after all shared circuit parts), complex64 to track
# the reference's precision.
# ----------------------------------------------------------------------------


def _host_psi(params: np.ndarray) -> np.ndarray:
    params = np.asarray(params, np.float32)
    psi = np.zeros(DIM, np.complex64)
    psi[0] = 1.0
    for l in range(N_LAYERS):
        for q in range(N_QUBITS):
            phi, theta, lam = (np.complex64(params[l, q, i]) for i in range(3))
            rz_p = np.array([[np.exp(-0.5j * phi), 0], [0, np.exp(0.5j * phi)]],
                            np.complex64)
            rz_l = np.array([[np.exp(-0.5j * lam), 0], [0, np.exp(0.5j * lam)]],
                            np.complex64)
            c, s = np.cos(0.5 * theta), np.sin(0.5 * theta)
            ry = np.array([[c, -s], [s, c]], np.complex64)
            U = rz_l @ ry @ rz_p
            # reference einsum applies U^T
            st = psi.reshape(2 ** q, 2, -1)
            psi = np.einsum("st,lsr->ltr", U, st).astype(np.complex64).reshape(-1)
        if l < N_LAYERS - 1:
            for q in range(N_QUBITS - 1):
                st = psi.reshape(2 ** q, 2, 2, -1)
                st = np.stack([st[:, 0], np.flip(st[:, 1], axis=1)], axis=1)
                psi = st.reshape(-1)
    return psi


def _features(X: np.ndarray) -> np.ndarray:
    """Phi[b, u] = prod_q (cos(X/2) if bit(11-q) of u is 0 else sin(X/2))."""
    c = np.cos(0.5 * X).astype(np.float32)
    s = np.sin(0.5 * X).astype(np.float32)
    phi = np.ones((B, 1), np.float32)
    for q in range(N_QUBITS):
        phi = np.stack([phi * c[:, q:q + 1], phi * s[:, q:q + 1]],
                       axis=2).reshape(B, -1)
    return phi


def _host_factor(psi: np.ndarray):
    """rho -> Wsym -> parity-ordered Cholesky. Returns (perm, W0, W1) with
    W = L - I per parity block (f32, strictly small)."""
    jj = np.arange(DIM)
    XORm = np.bitwise_xor.outer(jj, jj).astype(np.int32)
    ANDm = np.bitwise_and.outer(jj, jj).astype(np.int32)
    popand = np.zeros((DIM, DIM), np.int8)
    t = ANDm
    for q in range(N_QUBITS):
        popand += (t & 1).astype(np.int8)
        t = t >> 1
    del ANDm, t
    sgn_and = np.where(popand % 2 == 0, np.float32(1), np.float32(-1))
    del popand
    pop = np.zeros(DIM, np.int64)
    for q in range(N_QUBITS):
        pop += (jj >> q) & 1
    sgn = np.where(pop % 2 == 0, np.float32(1), np.float32(-1))
    par = (pop & 1).astype(np.int8)

    M = sgn_and * np.conj(psi)[XORm]          # M[d,k] = sgn(d&k) psi*_{d^k}
    rho = np.real(M @ psi).astype(np.float32)
    del M
    Wsym = (sgn[:, None] * sgn_and) * rho[XORm]
    del sgn_and, XORm

    perm = np.argsort(par, kind="stable")
    Wp = Wsym[np.ix_(perm, perm)]
    del Wsym
    L0 = np.linalg.cholesky(Wp[:HDIM, :HDIM].astype(np.float64))
    L1 = np.linalg.cholesky(Wp[HDIM:, HDIM:].astype(np.float64))
    W0 = (L0 - np.eye(HDIM)).astype(np.float32)
    W1 = (L1 - np.eye(HDIM)).astype(np.float32)
    return perm, W0, W1


def _prune_schedule(W0, W1, budget=0.09):
    """Triangular chunk list per j-block, dropping chunks whose total
    Frobenius mass stays under sqrt(budget) in both parities (the tail
    error this adds is ~1% of the fp8 noise, in quadrature). Blocks are
    scheduled descending so the earliest need the fewest phi chunks."""
    masses = []
    for c in range(NJB):
        for k in range(c // 2, KCH1):
            s0 = float((W0[k * 256:(k + 1) * 256,
                           c * 128:(c + 1) * 128].astype(np.float64) ** 2).sum())
            s1 = float((W1[k * 256:(k + 1) * 256,
                           c * 128:(c + 1) * 128].astype(np.float64) ** 2).sum())
            masses.append((max(s0, s1), c, k, s0, s1))
    masses.sort()
    drop = set()
    a0 = a1 = 0.0
    for mx, c, k, s0, s1 in masses:
        if k == c // 2 or a0 + s0 > budget or a1 + s1 > budget:
            continue
        a0 += s0
        a1 += s1
        drop.add((c, k))
    sched = []
    for c in range(NJB - 1, -1, -1):
        ks = [k for k in range(c // 2, KCH1) if (c, k) not in drop]
        sched.append((c, ks))
    return sched


# ----------------------------------------------------------------------------
# Pass 1: tail^T = W^T Phi^T per parity block, triangular fp8 DoubleRow.
# Core cr = 2*bg + p handles batch-group bg (1024 samples), parity p.
# ----------------------------------------------------------------------------


def _build_pass1(sched) -> bass.Bass:
    nchunk = sum(len(ks) for _, ks in sched)
    nc = bass.Bass("TRN2", target_bir_lowering=False, debug=False,
                   num_devices=NCORES)
    # w8[p, idx, i, c]: chunk list in sched order; chunk (k, cblk) holds
    # lam_w * W[k*256 + i*128 + p, cblk*128 + c]
    w_d = nc.dram_tensor("w8", [128, nchunk, 2, 128], f8,
                         kind="ExternalInput").ap()
    # phi[p, k, i, b] = lam_p * Phi^T[k*256 + i*128 + p, bg*1024 + b]
    phi_d = nc.dram_tensor("phi", [128, KCH1, 2, BG], f8,
                           kind="ExternalInput").ap()
    # t[p, pos, b] = lam_p*lam_w * tail^T[cblk(pos)*128 + p, bg*1024 + b]
    t_d = nc.dram_tensor("t", [128, NJB, BG], f8, kind="ExternalOutput").ap()

    # group blocks in fours for phi/W DMA batching and output batching
    gstart = [0]
    for c, ks in sched:
        gstart.append(gstart[-1] + len(ks))

    with tile.TileContext(nc) as tc:
        with (
            tc.tile_pool(name="wpool", bufs=1) as wpool,
            tc.tile_pool(name="spool", bufs=1) as spool,
            tc.tile_pool(name="psumw", bufs=1, space="PSUM") as psumw,
            tc.tile_pool(name="psum", bufs=3, space="PSUM") as psum,
        ):
            wa = wpool.tile([128, 2, 128], f8, tag="wa")
            wb = wpool.tile([128, 2, 512], f8, tag="wb")
            nc.vector.memset(wa[:], 0.0)
            nc.gpsimd.memset(wb[:], 0.0)
            wps = psumw.tile([128, 512], f32, tag="ps0", name="warm")
            for i in range(WARMUP1):
                nc.tensor.matmul(wps[:], wa[:], wb[:], start=True, stop=True,
                                 perf_mode=mybir.MatmulPerfMode.DoubleRow)

            w8 = wpool.tile([128, nchunk, 2, 128], f8, tag="w8")
            phi = wpool.tile([128, KCH1, 2, BG], f8, tag="phi")
            st = spool.tile([128, NJB, BG], f8, tag="st")

            # in-DMA stream: per group of 4 blocks, each new phi chunk then
            # the W chunks of the two blocks it unlocks; all on the in-order
            # SP queue. Output stores are emitted later (inside the block
            # loop) on the same queue: inputs always win the DMA engines.
            for g in range(4):
                klo = 6 - 2 * g
                i0, im = gstart[4 * g], gstart[4 * g + 2]
                i1 = gstart[4 * g + 4]
                nc.sync.dma_start(phi[:, klo + 1], phi_d[:, klo + 1])
                nc.sync.dma_start(w8[:, i0:im], w_d[:, i0:im])
                nc.sync.dma_start(phi[:, klo], phi_d[:, klo])
                nc.sync.dma_start(w8[:, im:i1], w_d[:, im:i1])

            for pos, (c, ks) in enumerate(sched):
                ps0 = psum.tile([128, 512], f32, tag="psA", name=f"psA_{c}")
                ps1 = psum.tile([128, 512], f32, tag="psB", name=f"psB_{c}")
                i0 = gstart[pos]
                for j, k in enumerate(ks):
                    st_mm = (j == 0)
                    sp_mm = (j == len(ks) - 1)
                    wch = w8[:, i0 + j]
                    nc.tensor.matmul(ps0[:], wch, phi[:, k, :, :512],
                                     start=st_mm, stop=sp_mm,
                                     perf_mode=mybir.MatmulPerfMode.DoubleRow)
                    nc.tensor.matmul(ps1[:], wch, phi[:, k, :, 512:],
                                     start=st_mm, stop=sp_mm,
                                     perf_mode=mybir.MatmulPerfMode.DoubleRow)
                # psum values are bounded by fp8 range via lam_w: plain copy
                nc.scalar.copy(st[:, pos, :512], ps0[:])
                nc.vector.tensor_copy(st[:, pos, 512:], ps1[:])
                if pos >= 12:
                    # last group: per-block stores so the tail is one block
                    nc.sync.dma_start(t_d[:, pos:pos + 1], st[:, pos:pos + 1])
                elif pos % 4 == 3:
                    g = pos // 4
                    nc.sync.dma_start(t_d[:, 4 * g:4 * g + 4],
                                      st[:, 4 * g:4 * g + 4])
    return nc


# ----------------------------------------------------------------------------
# Pass 2: single-product Gram + norm-corrected square, fp8 DoubleRow.
# ----------------------------------------------------------------------------


def _build_pass2() -> bass.Bass:
    nc = bass.Bass("TRN2", target_bir_lowering=False, debug=False,
                   num_devices=NCORES)
    # mv[p, kc, i, f]: Z8^T chunk of own rows (moving operand; also the
    # stationary operand for the 4 diagonal column blocks)
    mv_d = nc.dram_tensor("mv8", [128, KCH, 2, BLK], f8,
                          kind="ExternalInput").ap()
    # wt[n, p, kc, i, c]: Z8^T of off-diagonal column block n (stationary)
    wt_d = nc.dram_tensor("wt8", [NBLK - 4, 128, KCH, 2, 128], f8,
                          kind="ExternalInput").ap()
    sig_d = nc.dram_tensor("sig", [128, NBLK], f32, kind="ExternalInput").ap()
    wrow_d = nc.dram_tensor("wrow", [1, BLK], f32, kind="ExternalInput").ap()
    # ko[pos, p, f]: K[rows, col block ORDER2[pos], col p].T
    ko_d = nc.dram_tensor("ko", [NBLK, 128, BLK], bf16,
                          kind="ExternalOutput").ap()

    with tile.TileContext(nc) as tc:
        with (
            tc.tile_pool(name="mv", bufs=1) as mpool,
            tc.tile_pool(name="wt", bufs=4) as wpool,
            tc.tile_pool(name="post", bufs=3) as qpool,
            tc.tile_pool(name="psumd", bufs=1, space="PSUM") as dpool,
            tc.tile_pool(name="psum", bufs=2, space="PSUM") as ppool,
        ):
            mv = mpool.tile([128, KCH, 2, BLK], f8, tag="mv")
            sig = mpool.tile([128, NBLK], f32, tag="sig")
            wrow = mpool.tile([128, BLK], f32, tag="wrow")
            # mv streams in 8 chunks so the opening diagonal blocks can
            # chase it; wt panels follow just-in-time inside the block loop
            # (4-buffer lookahead), all on the in-order SP queue
            for h in range(8):
                nc.sync.dma_start(mv[:, 2 * h:2 * h + 2],
                                  mv_d[:, 2 * h:2 * h + 2])
                if h == 0:
                    nc.sync.dma_start(sig[:], sig_d)
                    nc.sync.dma_start(wrow[:],
                                      wrow_d[0].partition_broadcast(128))
            wts = {}

            def fetch_wt(n):
                if n >= 16:
                    return
                wt = wpool.tile([128, KCH, 2, 128], f8, tag="wt",
                                name=f"wt_{n}")
                nc.sync.dma_start(wt[:], wt_d[n])
                wts[n] = wt

            for i in range(4):
                fetch_wt(i)

            wa = mpool.tile([128, 2, 128], f8, tag="wa")
            wb = mpool.tile([128, 2, 512], f8, tag="wb")
            nc.vector.memset(wa[:], 0.0)
            nc.gpsimd.memset(wb[:], 0.0)
            wps = ppool.tile([128, BLK], f32, tag="m0", name="warm")
            for i in range(WARMUP2):
                nc.tensor.matmul(wps[:], wa[:], wb[:], start=True, stop=True,
                                 perf_mode=mybir.MatmulPerfMode.DoubleRow)

            def post(ps, pos, fsl, fo, eng=None):
                sq = qpool.tile([128, BLK], f32, tag="sq",
                                name=f"sq_{pos}_{fo}")
                nc.scalar.activation(sq[:, :fsl], ps[:, :fsl],
                                     mybir.ActivationFunctionType.Square,
                                     scale=sig[:, pos:pos + 1])
                ko = qpool.tile([128, BLK], bf16, tag="ko",
                                name=f"ko_{pos}_{fo}")
                nc.vector.tensor_tensor(ko[:, :fsl], sq[:, :fsl],
                                        wrow[:, fo:fo + fsl],
                                        mybir.AluOpType.mult)
                # stores ride the in-order SP queue interleaved with the wt
                # refills: a store's dep (the mult) clears ~5us before the
                # next wt panel behind it is needed, and queue order keeps
                # the DMA engines on inputs first. The final store instead
                # uses the empty Pool queue (SWDGE) so it skips the backlog.
                (eng or nc.sync).dma_start(ko_d[pos, :, fo:fo + fsl],
                                           ko[:, :fsl])

            # positions 0-3: the four diagonal blocks, k-interleaved so the
            # whole opening chases the mv chunk stream
            dps = [dpool.tile([128, BLK], f32, tag=f"d{d}", name=f"dps_{d}")
                   for d in range(4)]
            for k in range(KCH):
                for d in range(4):
                    nc.tensor.matmul(
                        dps[d][:], mv[:, k, :, d * 128:(d + 1) * 128],
                        mv[:, k], start=(k == 0), stop=(k == KCH - 1),
                        perf_mode=mybir.MatmulPerfMode.DoubleRow)
            for d in range(4):
                post(dps[d], d, BLK, 0)

            # positions 4..19: off-diagonal blocks on the wt stream
            for pos in range(4, NBLK):
                n = ORDER2[pos]
                halves = ((0, BLK),) if pos < NBLK - 1 else ((0, 256),
                                                            (256, 256))
                for fo, fsl in halves:
                    ps = ppool.tile([128, BLK], f32, tag=f"m{pos % 2}",
                                    name=f"m_{pos}_{fo}")
                    for k in range(KCH):
                        nc.tensor.matmul(
                            ps[:, :fsl], wts[n][:, k], mv[:, k, :, fo:fo + fsl],
                            start=(k == 0), stop=(k == KCH - 1),
                            perf_mode=mybir.MatmulPerfMode.DoubleRow)
                    last = pos == NBLK - 1 and fo != 0
                    post(ps, pos, fsl, fo, eng=nc.gpsimd if last else None)
                # refill the 4-deep wt pipeline now that this block's
                # matmuls guard the recycled buffer
                fetch_wt(n + 4)
    return nc


_nc1 = None
_nc2 = None

PROFILE = False
LAST_PROFILE: dict = {}


def kernel(X: np.ndarray, params: np.ndarray) -> np.ndarray:
    global _nc1, _nc2
    _install_waitfix()
    X = np.asarray(X, np.float32)
    params = np.asarray(params, np.float32)

    # ---- host precompute -------------------------------------------------
    psi = _host_psi(params)
    phi = _features(X)                           # (B, DIM) f32
    perm, W0, W1 = _host_factor(psi)
    sched = _prune_schedule(W0, W1)
    phiT = np.ascontiguousarray(phi[:, perm].T)  # (DIM parity-ordered, B)

    lam_p = 64.0
    # bound |tail| <= max column norm of W so psum fits fp8 range directly
    bnd0 = float(np.sqrt((W0.astype(np.float64) ** 2).sum(axis=0).max()))
    bnd1 = float(np.sqrt((W1.astype(np.float64) ** 2).sum(axis=0).max()))
    lam_w0 = 400.0 / (lam_p * max(bnd0, 1e-30))
    lam_w1 = 400.0 / (lam_p * max(bnd1, 1e-30))
    phi8 = (phiT * lam_p).astype(npf8)           # (DIM, B)

    nchunk = sum(len(ks) for _, ks in sched)

    def pack_w(W, lam_w):
        W8 = (W * lam_w).astype(npf8)            # (HDIM u, HDIM j)
        out = np.empty((128, nchunk, 2, 128), npf8)
        idx = 0
        for c, ks in sched:
            for k in ks:
                ch = W8[k * 256:(k + 1) * 256, c * 128:(c + 1) * 128]
                out[:, idx] = ch.reshape(2, 128, 128).transpose(1, 0, 2)
                idx += 1
        return out

    w_par = [pack_w(W0, lam_w0), pack_w(W1, lam_w1)]
    phi_par = []
    for p in range(2):
        rows = phi8[p * HDIM:(p + 1) * HDIM]     # (HDIM, B)
        phi_par.append(rows.reshape(KCH1, 2, 128, B).transpose(2, 0, 1, 3))

    in_maps1 = []
    for cr in range(NCORES):
        bg, p = divmod(cr, 2)
        in_maps1.append({
            "w8": w_par[p],
            "phi": np.ascontiguousarray(phi_par[p][:, :, :,
                                                   bg * BG:(bg + 1) * BG]),
        })

    if _nc1 is None:
        _nc1 = _build_pass1(sched)
    res1 = run_bass_kernel_spmd(_nc1, in_maps1, core_ids=list(range(NCORES)))

    # ---- host mid: assemble Z, quantize ----------------------------------
    ZT = phiT                                    # reuse buffer (DIM, B)
    inv = [1.0 / (lam_p * lam_w0), 1.0 / (lam_p * lam_w1)]
    pos2c = [c for c, _ in sched]
    for cr in range(NCORES):
        bg, p = divmod(cr, 2)
        t = res1.results[cr]["t"].astype(np.float32) * inv[p]   # (128,16,1024)
        for pos in range(NJB):
            c = pos2c[pos]
            ZT[p * HDIM + c * 128:p * HDIM + (c + 1) * 128,
               bg * BG:(bg + 1) * BG] += t[:, pos]

    Z8 = (ZT * LAM).astype(npf8)                 # (DIM, B)
    Z8f32 = Z8.astype(np.float32)
    rho2 = np.einsum("jb,jb->b", Z8f32, Z8f32) / (LAM * LAM)    # (B,)
    del Z8f32
    sig_all = (1.0 / (LAM * LAM * np.sqrt(rho2))).astype(np.float32)
    wrow_all = (1.0 / rho2).astype(np.float32)

    # strip layout: 16 off-diagonal col blocks (strip offsets 512..2560)
    # DMA'd as wt; the 4 diagonal blocks (offsets 0..512) slice mv.
    colrel = np.concatenate([np.arange(BLK, NB_COLS), np.arange(0, BLK)])
    Z8c = Z8.reshape(KCH, 2, 128, B)
    in_maps2 = []
    for cr in range(NCORES):
        cols = (cr * BLK + colrel) % B
        mvc = Z8c[:, :, :, cr * BLK:(cr + 1) * BLK].transpose(2, 0, 1, 3)
        wtc = Z8c[:, :, :, cols[:16 * 128]].reshape(
            KCH, 2, 128, 16, 128).transpose(3, 2, 0, 1, 4)
        sig_blocks = sig_all[cols].reshape(NBLK, 128)    # by block n
        sig = sig_blocks[ORDER2].T                       # (128, pos)
        wrow = wrow_all[cr * BLK:(cr + 1) * BLK][None, :]
        in_maps2.append({
            "mv8": np.ascontiguousarray(mvc),
            "wt8": np.ascontiguousarray(wtc),
            "sig": np.ascontiguousarray(sig),
            "wrow": np.ascontiguousarray(wrow),
        })

    if _nc2 is None:
        _nc2 = _build_pass2()
    res2 = run_bass_kernel_spmd(_nc2, in_maps2, core_ids=list(range(NCORES)))

    # ---- assemble K (with symmetric mirroring) ---------------------------
    K = np.empty((B, B), np.float32)
    for cr in range(NCORES):
        ko = res2.results[cr]["ko"].astype(np.float32)   # (pos, 128, BLK)
        rows = slice(cr * BLK, (cr + 1) * BLK)
        for pos in range(NBLK):
            n = ORDER2[pos]
            gs = (cr * BLK + int(colrel[n * 128])) % B
            colsl = slice(gs, gs + 128)
            K[rows, colsl] = ko[pos].T
            d = 1 + n // 4 if n < 16 else 0
            if 0 < d < 4 or (d == 4 and cr < 4):
                K[colsl, rows] = ko[pos]
    return K


# revision 23
# speedup vs baseline: 3.7882x; 1.0051x over previous
"""Trainium2 Bass kernel for nn_NeuralQKM: K[i,j] = |<psi_i|psi_j>|^2.

Math. States factor as S = Phi C with product features
Phi_b[u] = prod_q (cos(X/2) if u_q=0 else sin(X/2)) and a fixed complex
matrix C[u,j] = (-1)^{|j&u|} psi'[j^u] (psi' = state after all shared
gates; the final CNOT chain is a common permutation and drops out).
The Gram G = S S^H = Phi (C C^H) Phi^T where

    (C C^H)[u,u'] = (-1)^{|u&d|} rho(d),  d = u^u',
    rho(d) = sum_k (-1)^{|k&d|} psi'[k] conj(psi'[k^d]),

so Re G = Phi Wsym Phi^T with Wsym real symmetric PSD, and Re rho(d) = 0
for odd |d| makes Wsym parity-block-diagonal. Im G vanishes on the
diagonal and contributes O(1e-6) to ||K||_F: K ~= (Re G)^2 elementwise.

Cholesky per parity block, Wsym = L L^T, gives Re G = Z Z^T with
Z = Phi L of exactly unit row norm. W = L - I is small (params are
tiny), so Z = Phi + Phi W: the main term is exact host math and only the
tail needs the device, which tolerates fp8.

Device pass 1 (4 batch-groups x 2 parities): tail^T = W^T Phi^T per
parity block, fp8 DoubleRow, keeping only the lower-triangular W chunks
whose Frobenius mass matters (~40 of 128). lam_w is sized so psum values
fit fp8 range directly: the tail streams out as fp8 with a plain copy.
Output stores ride the in-order SP queue after every input DMA so they
never preempt the input stream on the shared DMA engines.

Device pass 2 (row-sharded, block-cyclic symmetric): single-product Gram
ps = Z8_cols . Z8_rows^T; post-ops square with a per-state norm
correction K = ps^2/(LAM^4 rho_c^2 rho_r^2) (rho^2 = ||quantized Z||^2,
host-known), which cancels the dominant fp8 radial error. Diagonal
column blocks slice mv directly as the stationary operand (no wt DMA)
and two of them open the pass so compute starts after one mv chunk.
Output per core is the transposed block strip K[rows, cols].T in bf16;
host mirrors the symmetric blocks.
"""
import numpy as np
import ml_dtypes
import orjson

import concourse.bass as bass
import concourse.mybir as mybir
import concourse.tile as tile
from concourse.bass_utils import run_bass_kernel_spmd

N_QUBITS = 12
N_LAYERS = 5
DIM = 2 ** N_QUBITS          # 4096
HDIM = DIM // 2              # 2048 per parity block
B = 4096
NCORES = 8
BLK = B // NCORES            # 512 rows per core in pass 2
NDBLK = 5                    # diagonal + 4 off-diagonal column blocks
NB_COLS = NDBLK * BLK        # 2560 rhs columns per core
NBLK = NB_COLS // 128        # 20 column blocks of 128
KCH = DIM // 256             # 16 contraction chunks of K=256 (DoubleRow)
KCH1 = HDIM // 256           # 8 contraction chunks in pass 1
NJB = HDIM // 128            # 16 output column blocks in pass 1
BG = B // 4                  # 1024 samples per pass-1 batch-group
LAM = 64.0                   # fp8 quantization scale for state planes
WARMUP1 = 15                 # PE warmup matmuls, pass 1 (sim-tuned)
WARMUP2 = 11                 # PE warmup matmuls, pass 2 (sim-tuned)

# pass-2 block order: all four free-stationary diagonal blocks open the
# pass, chunk-interleaved so they chase the streaming mv chunks — PE has
# ~7us of work before the first wt panel can possibly arrive
ORDER2 = [16, 17, 18, 19] + list(range(16))

f32 = mybir.dt.float32
f8 = mybir.dt.float8e4
bf16 = mybir.dt.bfloat16
npf8 = ml_dtypes.float8_e4m3
npbf = ml_dtypes.bfloat16

# ----------------------------------------------------------------------------
# walrus in this toolchain rejects >1 sync-wait per instruction; Tile emits
# several. Engines are serial, so an extra wait is equivalent to a standalone
# EventSemaphore wait right before the instruction on the same engine.
# ----------------------------------------------------------------------------


def _legalize_multiwait_json(bir: bytes) -> bytes:
    m = orjson.loads(bir)
    changed = False
    for func in m.get("functions", []):
        for blk in func.get("blocks", []):
            out = []
            for inst in blk.get("instructions", []):
                sync = inst.get("sync_info")
                waits = (sync or {}).get("on_wait") or []
                if len(waits) > 1:
                    changed = True
                    for i, w in enumerate(waits[:-1]):
                        out.append({
                            "debug": inst.get("debug", 0),
                            "engine": inst["engine"],
                            "ins": [],
                            "name": f"{inst['name']}-xw{i}",
                            "opcode": "EventSemaphore",
                            "outs": [],
                            "sync_info": {"on_update": [], "on_wait": [w]},
                        })
                    sync["on_wait"] = [waits[-1]]
                out.append(inst)
            blk["instructions"] = out
    return orjson.dumps(m) if changed else bir


_patched = False


def _install_waitfix():
    global _patched
    if _patched:
        return
    _patched = True
    orig = bass.Bass.to_json_bytes

    def patched(self):
        return _legalize_multiwait_json(orig(self))

    bass.Bass.to_json_bytes = patched


# ----------------------------------------------------------------------------
# Host math: psi' (state after all shared circuit parts), complex64 to track
# the reference's precision.
# ----------------------------------------------------------------------------


def _host_psi(params: np.ndarray) -> np.ndarray:
    params = np.asarray(params, np.float32)
    psi = np.zeros(DIM, np.complex64)
    psi[0] = 1.0
    for l in range(N_LAYERS):
        for q in range(N_QUBITS):
            phi, theta, lam = (np.complex64(params[l, q, i]) for i in range(3))
            rz_p = np.array([[np.exp(-0.5j * phi), 0], [0, np.exp(0.5j * phi)]],
                            np.complex64)
            rz_l = np.array([[np.exp(-0.5j * lam), 0], [0, np.exp(0.5j * lam)]],
                            np.complex64)
            c, s = np.cos(0.5 * theta), np.sin(0.5 * theta)
            ry = np.array([[c, -s], [s, c]], np.complex64)
            U = rz_l @ ry @ rz_p
            # reference einsum applies U^T
            st = psi.reshape(2 ** q, 2, -1)
            psi = np.einsum("st,lsr->ltr", U, st).astype(np.complex64).reshape(-1)
        if l < N_LAYERS - 1:
            for q in range(N_QUBITS - 1):
                st = psi.reshape(2 ** q, 2, 2, -1)
                st = np.stack([st[:, 0], np.flip(st[:, 1], axis=1)], axis=1)
                psi = st.reshape(-1)
    return psi


def _features(X: np.ndarray) -> np.ndarray:
    """Phi[b, u] = prod_q (cos(X/2) if bit(11-q) of u is 0 else sin(X/2))."""
    c = np.cos(0.5 * X).astype(np.float32)
    s = np.sin(0.5 * X).astype(np.float32)
    phi = np.ones((B, 1), np.float32)
    for q in range(N_QUBITS):
        phi = np.stack([phi * c[:, q:q + 1], phi * s[:, q:q + 1]],
                       axis=2).reshape(B, -1)
    return phi


def _host_factor(psi: np.ndarray):
    """rho -> Wsym -> parity-ordered Cholesky. Returns (perm, W0, W1) with
    W = L - I per parity block (f32, strictly small)."""
    jj = np.arange(DIM)
    XORm = np.bitwise_xor.outer(jj, jj).astype(np.int32)
    ANDm = np.bitwise_and.outer(jj, jj).astype(np.int32)
    popand = np.zeros((DIM, DIM), np.int8)
    t = ANDm
    for q in range(N_QUBITS):
        popand += (t & 1).astype(np.int8)
        t = t >> 1
    del ANDm, t
    sgn_and = np.where(popand % 2 == 0, np.float32(1), np.float32(-1))
    del popand
    pop = np.zeros(DIM, np.int64)
    for q in range(N_QUBITS):
        pop += (jj >> q) & 1
    sgn = np.where(pop % 2 == 0, np.float32(1), np.float32(-1))
    par = (pop & 1).astype(np.int8)

    M = sgn_and * np.conj(psi)[XORm]          # M[d,k] = sgn(d&k) psi*_{d^k}
    rho = np.real(M @ psi).astype(np.float32)
    del M
    Wsym = (sgn[:, None] * sgn_and) * rho[XORm]
    del sgn_and, XORm

    perm = np.argsort(par, kind="stable")
    Wp = Wsym[np.ix_(perm, perm)]
    del Wsym
    L0 = np.linalg.cholesky(Wp[:HDIM, :HDIM].astype(np.float64))
    L1 = np.linalg.cholesky(Wp[HDIM:, HDIM:].astype(np.float64))
    W0 = (L0 - np.eye(HDIM)).astype(np.float32)
    W1 = (L1 - np.eye(HDIM)).astype(np.float32)
    return perm, W0, W1


def _prune_schedule(W0, W1, budget=0.09):
    """Triangular chunk list per j-block, dropping chunks whose total
    Frobenius mass stays under sqrt(budget) in both parities (the tail
    error this adds is ~1% of the fp8 noise, in quadrature). Blocks are
    scheduled descending so the earliest need the fewest phi chunks."""
    masses = []
    for c in range(NJB):
        for k in range(c // 2, KCH1):
            s0 = float((W0[k * 256:(k + 1) * 256,
                           c * 128:(c + 1) * 128].astype(np.float64) ** 2).sum())
            s1 = float((W1[k * 256:(k + 1) * 256,
                           c * 128:(c + 1) * 128].astype(np.float64) ** 2).sum())
            masses.append((max(s0, s1), c, k, s0, s1))
    masses.sort()
    drop = set()
    a0 = a1 = 0.0
    for mx, c, k, s0, s1 in masses:
        if k == c // 2 or a0 + s0 > budget or a1 + s1 > budget:
            continue
        a0 += s0
        a1 += s1
        drop.add((c, k))
    sched = []
    for c in range(NJB - 1, -1, -1):
        ks = [k for k in range(c // 2, KCH1) if (c, k) not in drop]
        sched.append((c, ks))
    return sched


# ----------------------------------------------------------------------------
# Pass 1: tail^T = W^T Phi^T per parity block, triangular fp8 DoubleRow.
# Core cr = 2*bg + p handles batch-group bg (1024 samples), parity p.
# ----------------------------------------------------------------------------


def _build_pass1(sched) -> bass.Bass:
    nchunk = sum(len(ks) for _, ks in sched)
    nc = bass.Bass("TRN2", target_bir_lowering=False, debug=False,
                   num_devices=NCORES)
    # w8[p, idx, i, c]: chunk list in sched order; chunk (k, cblk) holds
    # lam_w * W[k*256 + i*128 + p, cblk*128 + c]
    w_d = nc.dram_tensor("w8", [128, nchunk, 2, 128], f8,
                         kind="ExternalInput").ap()
    # phi[p, k, i, b] = lam_p * Phi^T[k*256 + i*128 + p, bg*1024 + b]
    phi_d = nc.dram_tensor("phi", [128, KCH1, 2, BG], f8,
                           kind="ExternalInput").ap()
    # t[p, pos, b] = lam_p*lam_w * tail^T[cblk(pos)*128 + p, bg*1024 + b]
    t_d = nc.dram_tensor("t", [128, NJB, BG], f8, kind="ExternalOutput").ap()

    # group blocks in fours for phi/W DMA batching and output batching
    gstart = [0]
    for c, ks in sched:
        gstart.append(gstart[-1] + len(ks))

    with tile.TileContext(nc) as tc:
        with (
            tc.tile_pool(name="wpool", bufs=1) as wpool,
            tc.tile_pool(name="spool", bufs=1) as spool,
            tc.tile_pool(name="psumw", bufs=1, space="PSUM") as psumw,
            tc.tile_pool(name="psum", bufs=3, space="PSUM") as psum,
        ):
            wa = wpool.tile([128, 2, 128], f8, tag="wa")
            wb = wpool.tile([128, 2, 512], f8, tag="wb")
            nc.vector.memset(wa[:], 0.0)
            nc.gpsimd.memset(wb[:], 0.0)
            wps = psumw.tile([128, 512], f32, tag="ps0", name="warm")
            for i in range(WARMUP1):
                nc.tensor.matmul(wps[:], wa[:], wb[:], start=True, stop=True,
                                 perf_mode=mybir.MatmulPerfMode.DoubleRow)

            w8 = wpool.tile([128, nchunk, 2, 128], f8, tag="w8")
            phi = wpool.tile([128, KCH1, 2, BG], f8, tag="phi")
            st = spool.tile([128, NJB, BG], f8, tag="st")

            # in-DMA stream: per group of 4 blocks, each new phi chunk then
            # the W chunks of the two blocks it unlocks; all on the in-order
            # SP queue. Output stores are emitted later (inside the block
            # loop) on the same queue: inputs always win the DMA engines.
            for g in range(4):
                klo = 6 - 2 * g
                i0, im = gstart[4 * g], gstart[4 * g + 2]
                i1 = gstart[4 * g + 4]
                nc.sync.dma_start(phi[:, klo + 1], phi_d[:, klo + 1])
                nc.sync.dma_start(w8[:, i0:im], w_d[:, i0:im])
                nc.sync.dma_start(phi[:, klo], phi_d[:, klo])
                nc.sync.dma_start(w8[:, im:i1], w_d[:, im:i1])

            for pos, (c, ks) in enumerate(sched):
                ps0 = psum.tile([128, 512], f32, tag="psA", name=f"psA_{c}")
                ps1 = psum.tile([128, 512], f32, tag="psB", name=f"psB_{c}")
                i0 = gstart[pos]
                for j, k in enumerate(ks):
                    st_mm = (j == 0)
                    sp_mm = (j == len(ks) - 1)
                    wch = w8[:, i0 + j]
                    nc.tensor.matmul(ps0[:], wch, phi[:, k, :, :512],
                                     start=st_mm, stop=sp_mm,
                                     perf_mode=mybir.MatmulPerfMode.DoubleRow)
                    nc.tensor.matmul(ps1[:], wch, phi[:, k, :, 512:],
                                     start=st_mm, stop=sp_mm,
                                     perf_mode=mybir.MatmulPerfMode.DoubleRow)
                # psum values are bounded by fp8 range via lam_w: plain copy
                nc.scalar.copy(st[:, pos, :512], ps0[:])
                nc.vector.tensor_copy(st[:, pos, 512:], ps1[:])
                if pos >= 12:
                    # last group: per-block stores so the tail is one block
                    nc.sync.dma_start(t_d[:, pos:pos + 1], st[:, pos:pos + 1])
                elif pos % 4 == 3:
                    g = pos // 4
                    nc.sync.dma_start(t_d[:, 4 * g:4 * g + 4],
                                      st[:, 4 * g:4 * g + 4])
    return nc


# ----------------------------------------------------------------------------
# Pass 2: single-product Gram + norm-corrected square, fp8 DoubleRow.
# ----------------------------------------------------------------------------


def _build_pass2() -> bass.Bass:
    nc = bass.Bass("TRN2", target_bir_lowering=False, debug=False,
                   num_devices=NCORES)
    # mv[p, kc, i, f]: Z8^T chunk of own rows (moving operand; also the
    # stationary operand for the 4 diagonal column blocks)
    mv_d = nc.dram_tensor("mv8", [128, KCH, 2, BLK], f8,
                          kind="ExternalInput").ap()
    # wt[n, p, kc, i, c]: Z8^T of off-diagonal column block n (stationary)
    wt_d = nc.dram_tensor("wt8", [NBLK - 4, 128, KCH, 2, 128], f8,
                          kind="ExternalInput").ap()
    sig_d = nc.dram_tensor("sig", [128, NBLK], f32, kind="ExternalInput").ap()
    wrow_d = nc.dram_tensor("wrow", [1, BLK], f32, kind="ExternalInput").ap()
    # ko[pos, p, f]: K[rows, col block ORDER2[pos], col p].T
    ko_d = nc.dram_tensor("ko", [NBLK, 128, BLK], bf16,
                          kind="ExternalOutput").ap()

    with tile.TileContext(nc) as tc:
        with (
            tc.tile_pool(name="mv", bufs=1) as mpool,
            tc.tile_pool(name="wt", bufs=6) as wpool,
            tc.tile_pool(name="post", bufs=3) as qpool,
            tc.tile_pool(name="psumd", bufs=1, space="PSUM") as dpool,
            tc.tile_pool(name="psum", bufs=2, space="PSUM") as ppool,
        ):
            mv = mpool.tile([128, KCH, 2, BLK], f8, tag="mv")
            sig = mpool.tile([128, NBLK], f32, tag="sig")
            wrow = mpool.tile([128, BLK], f32, tag="wrow")
            # mv streams in 8 chunks so the opening diagonal blocks can
            # chase it; wt panels follow just-in-time inside the block loop
            # (4-buffer lookahead), all on the in-order SP queue
            for h in range(8):
                nc.sync.dma_start(mv[:, 2 * h:2 * h + 2],
                                  mv_d[:, 2 * h:2 * h + 2])
                if h == 0:
                    nc.sync.dma_start(sig[:], sig_d)
                    nc.sync.dma_start(wrow[:],
                                      wrow_d[0].partition_broadcast(128))
            wts = {}

            def fetch_wt(n):
                if n >= 16:
                    return
                wt = wpool.tile([128, KCH, 2, 128], f8, tag="wt",
                                name=f"wt_{n}")
                nc.sync.dma_start(wt[:], wt_d[n])
                wts[n] = wt

            for i in range(6):
                fetch_wt(i)

            wa = mpool.tile([128, 2, 128], f8, tag="wa")
            wb = mpool.tile([128, 2, 512], f8, tag="wb")
            nc.vector.memset(wa[:], 0.0)
            nc.gpsimd.memset(wb[:], 0.0)
            wps = ppool.tile([128, BLK], f32, tag="m0", name="warm")
            for i in range(WARMUP2):
                nc.tensor.matmul(wps[:], wa[:], wb[:], start=True, stop=True,
                                 perf_mode=mybir.MatmulPerfMode.DoubleRow)

            def post(ps, pos, fsl, fo, eng=None):
                sq = qpool.tile([128, BLK], f32, tag="sq",
                                name=f"sq_{pos}_{fo}")
                nc.scalar.activation(sq[:, :fsl], ps[:, :fsl],
                                     mybir.ActivationFunctionType.Square,
                                     scale=sig[:, pos:pos + 1])
                ko = qpool.tile([128, BLK], bf16, tag="ko",
                                name=f"ko_{pos}_{fo}")
                nc.vector.tensor_tensor(ko[:, :fsl], sq[:, :fsl],
                                        wrow[:, fo:fo + fsl],
                                        mybir.AluOpType.mult)
                # stores ride the in-order SP queue interleaved with the wt
                # refills: a store's dep (the mult) clears ~5us before the
                # next wt panel behind it is needed, and queue order keeps
                # the DMA engines on inputs first. The final store instead
                # uses the empty Pool queue (SWDGE) so it skips the backlog.
                (eng or nc.sync).dma_start(ko_d[pos, :, fo:fo + fsl],
                                           ko[:, :fsl])

            # positions 0-3: the four diagonal blocks, k-interleaved so the
            # whole opening chases the mv chunk stream
            dps = [dpool.tile([128, BLK], f32, tag=f"d{d}", name=f"dps_{d}")
                   for d in range(4)]
            for k in range(KCH):
                for d in range(4):
                    nc.tensor.matmul(
                        dps[d][:], mv[:, k, :, d * 128:(d + 1) * 128],
                        mv[:, k], start=(k == 0), stop=(k == KCH - 1),
                        perf_mode=mybir.MatmulPerfMode.DoubleRow)
            for d in range(4):
                post(dps[d], d, BLK, 0)

            # positions 4..19: off-diagonal blocks on the wt stream
            for pos in range(4, NBLK):
                n = ORDER2[pos]
                halves = ((0, BLK),) if pos < NBLK - 1 else ((0, 256),
                                                            (256, 256))
                for fo, fsl in halves:
                    ps = ppool.tile([128, BLK], f32, tag=f"m{pos % 2}",
                                    name=f"m_{pos}_{fo}")
                    for k in range(KCH):
                        nc.tensor.matmul(
                            ps[:, :fsl], wts[n][:, k], mv[:, k, :, fo:fo + fsl],
                            start=(k == 0), stop=(k == KCH - 1),
                            perf_mode=mybir.MatmulPerfMode.DoubleRow)
                    last = pos == NBLK - 1 and fo != 0
                    post(ps, pos, fsl, fo, eng=nc.gpsimd if last else None)
                # refill the 6-deep wt pipeline now that this block's
                # matmuls guard the recycled buffer
                fetch_wt(n + 6)
    return nc


_nc1 = None
_nc2 = None

PROFILE = False
LAST_PROFILE: dict = {}


def kernel(X: np.ndarray, params: np.ndarray) -> np.ndarray:
    global _nc1, _nc2
    _install_waitfix()
    X = np.asarray(X, np.float32)
    params = np.asarray(params, np.float32)

    # ---- host precompute -------------------------------------------------
    psi = _host_psi(params)
    phi = _features(X)                           # (B, DIM) f32
    perm, W0, W1 = _host_factor(psi)
    sched = _prune_schedule(W0, W1)
    phiT = np.ascontiguousarray(phi[:, perm].T)  # (DIM parity-ordered, B)

    lam_p = 64.0
    # bound |tail| <= max column norm of W so psum fits fp8 range directly
    bnd0 = float(np.sqrt((W0.astype(np.float64) ** 2).sum(axis=0).max()))
    bnd1 = float(np.sqrt((W1.astype(np.float64) ** 2).sum(axis=0).max()))
    lam_w0 = 400.0 / (lam_p * max(bnd0, 1e-30))
    lam_w1 = 400.0 / (lam_p * max(bnd1, 1e-30))
    phi8 = (phiT * lam_p).astype(npf8)           # (DIM, B)

    nchunk = sum(len(ks) for _, ks in sched)

    def pack_w(W, lam_w):
        W8 = (W * lam_w).astype(npf8)            # (HDIM u, HDIM j)
        out = np.empty((128, nchunk, 2, 128), npf8)
        idx = 0
        for c, ks in sched:
            for k in ks:
                ch = W8[k * 256:(k + 1) * 256, c * 128:(c + 1) * 128]
                out[:, idx] = ch.reshape(2, 128, 128).transpose(1, 0, 2)
                idx += 1
        return out

    w_par = [pack_w(W0, lam_w0), pack_w(W1, lam_w1)]
    phi_par = []
    for p in range(2):
        rows = phi8[p * HDIM:(p + 1) * HDIM]     # (HDIM, B)
        phi_par.append(rows.reshape(KCH1, 2, 128, B).transpose(2, 0, 1, 3))

    in_maps1 = []
    for cr in range(NCORES):
        bg, p = divmod(cr, 2)
        in_maps1.append({
            "w8": w_par[p],
            "phi": np.ascontiguousarray(phi_par[p][:, :, :,
                                                   bg * BG:(bg + 1) * BG]),
        })

    if _nc1 is None:
        _nc1 = _build_pass1(sched)
    res1 = run_bass_kernel_spmd(_nc1, in_maps1, core_ids=list(range(NCORES)))

    # ---- host mid: assemble Z, quantize ----------------------------------
    ZT = phiT                                    # reuse buffer (DIM, B)
    inv = [1.0 / (lam_p * lam_w0), 1.0 / (lam_p * lam_w1)]
    pos2c = [c for c, _ in sched]
    for cr in range(NCORES):
        bg, p = divmod(cr, 2)
        t = res1.results[cr]["t"].astype(np.float32) * inv[p]   # (128,16,1024)
        for pos in range(NJB):
            c = pos2c[pos]
            ZT[p * HDIM + c * 128:p * HDIM + (c + 1) * 128,
               bg * BG:(bg + 1) * BG] += t[:, pos]

    Z8 = (ZT * LAM).astype(npf8)                 # (DIM, B)
    Z8f32 = Z8.astype(np.float32)
    rho2 = np.einsum("jb,jb->b", Z8f32, Z8f32) / (LAM * LAM)    # (B,)
    del Z8f32
    sig_all = (1.0 / (LAM * LAM * np.sqrt(rho2))).astype(np.float32)
    wrow_all = (1.0 / rho2).astype(np.float32)

    # strip layout: 16 off-diagonal col blocks (strip offsets 512..2560)
    # DMA'd as wt; the 4 diagonal blocks (offsets 0..512) slice mv.
    colrel = np.concatenate([np.arange(BLK, NB_COLS), np.arange(0, BLK)])
    Z8c = Z8.reshape(KCH, 2, 128, B)
    in_maps2 = []
    for cr in range(NCORES):
        cols = (cr * BLK + colrel) % B
        mvc = Z8c[:, :, :, cr * BLK:(cr + 1) * BLK].transpose(2, 0, 1, 3)
        wtc = Z8c[:, :, :, cols[:16 * 128]].reshape(
            KCH, 2, 128, 16, 128).transpose(3, 2, 0, 1, 4)
        sig_blocks = sig_all[cols].reshape(NBLK, 128)    # by block n
        sig = sig_blocks[ORDER2].T                       # (128, pos)
        wrow = wrow_all[cr * BLK:(cr + 1) * BLK][None, :]
        in_maps2.append({
            "mv8": np.ascontiguousarray(mvc),
            "wt8": np.ascontiguousarray(wtc),
            "sig": np.ascontiguousarray(sig),
            "wrow": np.ascontiguousarray(wrow),
        })

    if _nc2 is None:
        _nc2 = _build_pass2()
    res2 = run_bass_kernel_spmd(_nc2, in_maps2, core_ids=list(range(NCORES)))

    # ---- assemble K (with symmetric mirroring) ---------------------------
    K = np.empty((B, B), np.float32)
    for cr in range(NCORES):
        ko = res2.results[cr]["ko"].astype(np.float32)   # (pos, 128, BLK)
        rows = slice(cr * BLK, (cr + 1) * BLK)
        for pos in range(NBLK):
            n = ORDER2[pos]
            gs = (cr * BLK + int(colrel[n * 128])) % B
            colsl = slice(gs, gs + 128)
            K[rows, colsl] = ko[pos].T
            d = 1 + n // 4 if n < 16 else 0
            if 0 < d < 4 or (d == 4 and cr < 4):
                K[colsl, rows] = ko[pos]
    return K


# revision 24
# speedup vs baseline: 3.8264x; 1.0101x over previous
"""Trainium2 Bass kernel for nn_NeuralQKM: K[i,j] = |<psi_i|psi_j>|^2.

Math. States factor as S = Phi C with product features
Phi_b[u] = prod_q (cos(X/2) if u_q=0 else sin(X/2)) and a fixed complex
matrix C[u,j] = (-1)^{|j&u|} psi'[j^u] (psi' = state after all shared
gates; the final CNOT chain is a common permutation and drops out).
The Gram G = S S^H = Phi (C C^H) Phi^T where

    (C C^H)[u,u'] = (-1)^{|u&d|} rho(d),  d = u^u',
    rho(d) = sum_k (-1)^{|k&d|} psi'[k] conj(psi'[k^d]),

so Re G = Phi Wsym Phi^T with Wsym real symmetric PSD, and Re rho(d) = 0
for odd |d| makes Wsym parity-block-diagonal. Im G vanishes on the
diagonal and contributes O(1e-6) to ||K||_F: K ~= (Re G)^2 elementwise.

Cholesky per parity block, Wsym = L L^T, gives Re G = Z Z^T with
Z = Phi L of exactly unit row norm. W = L - I is small (params are
tiny), so Z = Phi + Phi W: the main term is exact host math and only the
tail needs the device, which tolerates fp8.

Device pass 1 (4 batch-groups x 2 parities): tail^T = W^T Phi^T per
parity block, fp8 DoubleRow, keeping only the lower-triangular W chunks
whose Frobenius mass matters (~40 of 128). lam_w is sized so psum values
fit fp8 range directly: the tail streams out as fp8 with a plain copy.
Output stores ride the in-order SP queue after every input DMA so they
never preempt the input stream on the shared DMA engines.

Device pass 2 (row-sharded, block-cyclic symmetric): single-product Gram
ps = Z8_cols . Z8_rows^T; post-ops square with a per-state norm
correction K = ps^2/(LAM^4 rho_c^2 rho_r^2) (rho^2 = ||quantized Z||^2,
host-known), which cancels the dominant fp8 radial error. Diagonal
column blocks slice mv directly as the stationary operand (no wt DMA)
and two of them open the pass so compute starts after one mv chunk.
Output per core is the transposed block strip K[rows, cols].T in bf16;
host mirrors the symmetric blocks.
"""
import numpy as np
import ml_dtypes
import orjson

import concourse.bass as bass
import concourse.mybir as mybir
import concourse.tile as tile
from concourse.bass_utils import run_bass_kernel_spmd

N_QUBITS = 12
N_LAYERS = 5
DIM = 2 ** N_QUBITS          # 4096
HDIM = DIM // 2              # 2048 per parity block
B = 4096
NCORES = 8
BLK = B // NCORES            # 512 rows per core in pass 2
NDBLK = 5                    # diagonal + 4 off-diagonal column blocks
NB_COLS = NDBLK * BLK        # 2560 rhs columns per core
NBLK = NB_COLS // 128        # 20 column blocks of 128
KCH = DIM // 256             # 16 contraction chunks of K=256 (DoubleRow)
KCH1 = HDIM // 256           # 8 contraction chunks in pass 1
NJB = HDIM // 128            # 16 output column blocks in pass 1
BG = B // 4                  # 1024 samples per pass-1 batch-group
LAM = 64.0                   # fp8 quantization scale for state planes
WARMUP1 = 15                 # PE warmup matmuls, pass 1 (sim-tuned)
WARMUP2 = 11                 # PE warmup matmuls, pass 2 (sim-tuned)

# pass-2 block order: all four free-stationary diagonal blocks open the
# pass, chunk-interleaved so they chase the streaming mv chunks — PE has
# ~7us of work before the first wt panel can possibly arrive
ORDER2 = [16, 17, 18, 19] + list(range(16))

f32 = mybir.dt.float32
f8 = mybir.dt.float8e4
bf16 = mybir.dt.bfloat16
npf8 = ml_dtypes.float8_e4m3
npbf = ml_dtypes.bfloat16

# ----------------------------------------------------------------------------
# walrus in this toolchain rejects >1 sync-wait per instruction; Tile emits
# several. Engines are serial, so an extra wait is equivalent to a standalone
# EventSemaphore wait right before the instruction on the same engine.
# ----------------------------------------------------------------------------


def _legalize_multiwait_json(bir: bytes) -> bytes:
    m = orjson.loads(bir)
    changed = False
    for func in m.get("functions", []):
        for blk in func.get("blocks", []):
            out = []
            for inst in blk.get("instructions", []):
                sync = inst.get("sync_info")
                waits = (sync or {}).get("on_wait") or []
                if len(waits) > 1:
                    changed = True
                    for i, w in enumerate(waits[:-1]):
                        out.append({
                            "debug": inst.get("debug", 0),
                            "engine": inst["engine"],
                            "ins": [],
                            "name": f"{inst['name']}-xw{i}",
                            "opcode": "EventSemaphore",
                            "outs": [],
                            "sync_info": {"on_update": [], "on_wait": [w]},
                        })
                    sync["on_wait"] = [waits[-1]]
                out.append(inst)
            blk["instructions"] = out
    return orjson.dumps(m) if changed else bir


_patched = False


def _install_waitfix():
    global _patched
    if _patched:
        return
    _patched = True
    orig = bass.Bass.to_json_bytes

    def patched(self):
        return _legalize_multiwait_json(orig(self))

    bass.Bass.to_json_bytes = patched


# ----------------------------------------------------------------------------
# Host math: psi' (state after all shared circuit parts), complex64 to track
# the reference's precision.
# ----------------------------------------------------------------------------


def _host_psi(params: np.ndarray) -> np.ndarray:
    params = np.asarray(params, np.float32)
    psi = np.zeros(DIM, np.complex64)
    psi[0] = 1.0
    for l in range(N_LAYERS):
        for q in range(N_QUBITS):
            phi, theta, lam = (np.complex64(params[l, q, i]) for i in range(3))
            rz_p = np.array([[np.exp(-0.5j * phi), 0], [0, np.exp(0.5j * phi)]],
                            np.complex64)
            rz_l = np.array([[np.exp(-0.5j * lam), 0], [0, np.exp(0.5j * lam)]],
                            np.complex64)
            c, s = np.cos(0.5 * theta), np.sin(0.5 * theta)
            ry = np.array([[c, -s], [s, c]], np.complex64)
            U = rz_l @ ry @ rz_p
            # reference einsum applies U^T
            st = psi.reshape(2 ** q, 2, -1)
            psi = np.einsum("st,lsr->ltr", U, st).astype(np.complex64).reshape(-1)
        if l < N_LAYERS - 1:
            for q in range(N_QUBITS - 1):
                st = psi.reshape(2 ** q, 2, 2, -1)
                st = np.stack([st[:, 0], np.flip(st[:, 1], axis=1)], axis=1)
                psi = st.reshape(-1)
    return psi


def _features(X: np.ndarray) -> np.ndarray:
    """Phi[b, u] = prod_q (cos(X/2) if bit(11-q) of u is 0 else sin(X/2))."""
    c = np.cos(0.5 * X).astype(np.float32)
    s = np.sin(0.5 * X).astype(np.float32)
    phi = np.ones((B, 1), np.float32)
    for q in range(N_QUBITS):
        phi = np.stack([phi * c[:, q:q + 1], phi * s[:, q:q + 1]],
                       axis=2).reshape(B, -1)
    return phi


def _host_factor(psi: np.ndarray):
    """rho -> Wsym -> parity-ordered Cholesky. Returns (perm, W0, W1) with
    W = L - I per parity block (f32, strictly small)."""
    jj = np.arange(DIM)
    XORm = np.bitwise_xor.outer(jj, jj).astype(np.int32)
    ANDm = np.bitwise_and.outer(jj, jj).astype(np.int32)
    popand = np.zeros((DIM, DIM), np.int8)
    t = ANDm
    for q in range(N_QUBITS):
        popand += (t & 1).astype(np.int8)
        t = t >> 1
    del ANDm, t
    sgn_and = np.where(popand % 2 == 0, np.float32(1), np.float32(-1))
    del popand
    pop = np.zeros(DIM, np.int64)
    for q in range(N_QUBITS):
        pop += (jj >> q) & 1
    sgn = np.where(pop % 2 == 0, np.float32(1), np.float32(-1))
    par = (pop & 1).astype(np.int8)

    M = sgn_and * np.conj(psi)[XORm]          # M[d,k] = sgn(d&k) psi*_{d^k}
    rho = np.real(M @ psi).astype(np.float32)
    del M
    Wsym = (sgn[:, None] * sgn_and) * rho[XORm]
    del sgn_and, XORm

    perm = np.argsort(par, kind="stable")
    Wp = Wsym[np.ix_(perm, perm)]
    del Wsym
    L0 = np.linalg.cholesky(Wp[:HDIM, :HDIM].astype(np.float64))
    L1 = np.linalg.cholesky(Wp[HDIM:, HDIM:].astype(np.float64))
    W0 = (L0 - np.eye(HDIM)).astype(np.float32)
    W1 = (L1 - np.eye(HDIM)).astype(np.float32)
    return perm, W0, W1


def _prune_schedule(W0, W1, budget=0.09):
    """Triangular chunk list per j-block, dropping chunks whose total
    Frobenius mass stays under sqrt(budget) in both parities (the tail
    error this adds is ~1% of the fp8 noise, in quadrature). Blocks are
    scheduled descending so the earliest need the fewest phi chunks."""
    masses = []
    for c in range(NJB):
        for k in range(c // 2, KCH1):
            s0 = float((W0[k * 256:(k + 1) * 256,
                           c * 128:(c + 1) * 128].astype(np.float64) ** 2).sum())
            s1 = float((W1[k * 256:(k + 1) * 256,
                           c * 128:(c + 1) * 128].astype(np.float64) ** 2).sum())
            masses.append((max(s0, s1), c, k, s0, s1))
    masses.sort()
    drop = set()
    a0 = a1 = 0.0
    for mx, c, k, s0, s1 in masses:
        if k == c // 2 or a0 + s0 > budget or a1 + s1 > budget:
            continue
        a0 += s0
        a1 += s1
        drop.add((c, k))
    sched = []
    for c in range(NJB - 1, -1, -1):
        ks = [k for k in range(c // 2, KCH1) if (c, k) not in drop]
        sched.append((c, ks))
    return sched


# ----------------------------------------------------------------------------
# Pass 1: tail^T = W^T Phi^T per parity block, triangular fp8 DoubleRow.
# Core cr = 2*bg + p handles batch-group bg (1024 samples), parity p.
# ----------------------------------------------------------------------------


def _build_pass1(sched) -> bass.Bass:
    nchunk = sum(len(ks) for _, ks in sched)
    nc = bass.Bass("TRN2", target_bir_lowering=False, debug=False,
                   num_devices=NCORES)
    # w8[p, idx, i, c]: chunk list in sched order; chunk (k, cblk) holds
    # lam_w * W[k*256 + i*128 + p, cblk*128 + c]
    w_d = nc.dram_tensor("w8", [128, nchunk, 2, 128], f8,
                         kind="ExternalInput").ap()
    # phi[p, k, i, b] = lam_p * Phi^T[k*256 + i*128 + p, bg*1024 + b]
    phi_d = nc.dram_tensor("phi", [128, KCH1, 2, BG], f8,
                           kind="ExternalInput").ap()
    # t[p, pos, b] = lam_p*lam_w * tail^T[cblk(pos)*128 + p, bg*1024 + b]
    t_d = nc.dram_tensor("t", [128, NJB, BG], f8, kind="ExternalOutput").ap()

    # group blocks in fours for phi/W DMA batching and output batching
    gstart = [0]
    for c, ks in sched:
        gstart.append(gstart[-1] + len(ks))

    with tile.TileContext(nc) as tc:
        with (
            tc.tile_pool(name="wpool", bufs=1) as wpool,
            tc.tile_pool(name="spool", bufs=1) as spool,
            tc.tile_pool(name="psumw", bufs=1, space="PSUM") as psumw,
            tc.tile_pool(name="psum", bufs=3, space="PSUM") as psum,
        ):
            wa = wpool.tile([128, 2, 128], f8, tag="wa")
            wb = wpool.tile([128, 2, 512], f8, tag="wb")
            nc.vector.memset(wa[:], 0.0)
            nc.gpsimd.memset(wb[:], 0.0)
            wps = psumw.tile([128, 512], f32, tag="ps0", name="warm")
            for i in range(WARMUP1):
                nc.tensor.matmul(wps[:], wa[:], wb[:], start=True, stop=True,
                                 perf_mode=mybir.MatmulPerfMode.DoubleRow)

            w8 = wpool.tile([128, nchunk, 2, 128], f8, tag="w8")
            phi = wpool.tile([128, KCH1, 2, BG], f8, tag="phi")
            st = spool.tile([128, NJB, BG], f8, tag="st")

            # in-DMA stream: per group of 4 blocks, each new phi chunk then
            # the W chunks of the two blocks it unlocks; all on the in-order
            # SP queue. Output stores are emitted later (inside the block
            # loop) on the same queue: inputs always win the DMA engines.
            for g in range(4):
                klo = 6 - 2 * g
                i0, im = gstart[4 * g], gstart[4 * g + 2]
                i1 = gstart[4 * g + 4]
                nc.sync.dma_start(phi[:, klo + 1], phi_d[:, klo + 1])
                nc.sync.dma_start(w8[:, i0:im], w_d[:, i0:im])
                nc.sync.dma_start(phi[:, klo], phi_d[:, klo])
                nc.sync.dma_start(w8[:, im:i1], w_d[:, im:i1])

            for pos, (c, ks) in enumerate(sched):
                ps0 = psum.tile([128, 512], f32, tag="psA", name=f"psA_{c}")
                ps1 = psum.tile([128, 512], f32, tag="psB", name=f"psB_{c}")
                i0 = gstart[pos]
                for j, k in enumerate(ks):
                    st_mm = (j == 0)
                    sp_mm = (j == len(ks) - 1)
                    wch = w8[:, i0 + j]
                    nc.tensor.matmul(ps0[:], wch, phi[:, k, :, :512],
                                     start=st_mm, stop=sp_mm,
                                     perf_mode=mybir.MatmulPerfMode.DoubleRow)
                    nc.tensor.matmul(ps1[:], wch, phi[:, k, :, 512:],
                                     start=st_mm, stop=sp_mm,
                                     perf_mode=mybir.MatmulPerfMode.DoubleRow)
                # psum values are bounded by fp8 range via lam_w: plain copy
                nc.scalar.copy(st[:, pos, :512], ps0[:])
                nc.vector.tensor_copy(st[:, pos, 512:], ps1[:])
                if pos >= 12:
                    # last group: per-block stores so the tail is one block
                    nc.sync.dma_start(t_d[:, pos:pos + 1], st[:, pos:pos + 1])
                elif pos % 4 == 3:
                    g = pos // 4
                    nc.sync.dma_start(t_d[:, 4 * g:4 * g + 4],
                                      st[:, 4 * g:4 * g + 4])
    return nc


# ----------------------------------------------------------------------------
# Pass 2: single-product Gram + norm-corrected square, fp8 DoubleRow.
# ----------------------------------------------------------------------------


def _build_pass2() -> bass.Bass:
    nc = bass.Bass("TRN2", target_bir_lowering=False, debug=False,
                   num_devices=NCORES)
    # mv[p, kc, i, f]: Z8^T chunk of own rows (moving operand; also the
    # stationary operand for the 4 diagonal column blocks)
    mv_d = nc.dram_tensor("mv8", [128, KCH, 2, BLK], f8,
                          kind="ExternalInput").ap()
    # wt[n, p, kc, i, c]: Z8^T of off-diagonal column block n (stationary)
    wt_d = nc.dram_tensor("wt8", [NBLK - 4, 128, KCH, 2, 128], f8,
                          kind="ExternalInput").ap()
    # ko[pos, p, f]: raw squared products ps^2 = (Z8_c . Z8_r)^2; the
    # norm corrections are a host-side outer-product scaling at assembly
    ko_d = nc.dram_tensor("ko", [NBLK, 128, BLK], bf16,
                          kind="ExternalOutput").ap()

    with tile.TileContext(nc) as tc:
        with (
            tc.tile_pool(name="mv", bufs=1) as mpool,
            tc.tile_pool(name="wt", bufs=6) as wpool,
            tc.tile_pool(name="post", bufs=3) as qpool,
            tc.tile_pool(name="psumd", bufs=1, space="PSUM") as dpool,
            tc.tile_pool(name="psum", bufs=2, space="PSUM") as ppool,
        ):
            mv = mpool.tile([128, KCH, 2, BLK], f8, tag="mv")
            # mv streams in 8 chunks so the opening diagonal blocks can
            # chase it; wt panels follow just-in-time inside the block loop
            # (6-buffer lookahead), all on the in-order SP queue
            for h in range(8):
                nc.sync.dma_start(mv[:, 2 * h:2 * h + 2],
                                  mv_d[:, 2 * h:2 * h + 2])
            wts = {}

            def fetch_wt(n):
                if n >= 16:
                    return
                wt = wpool.tile([128, KCH, 2, 128], f8, tag="wt",
                                name=f"wt_{n}")
                nc.sync.dma_start(wt[:], wt_d[n])
                wts[n] = wt

            for i in range(6):
                fetch_wt(i)

            wa = mpool.tile([128, 2, 128], f8, tag="wa")
            wb = mpool.tile([128, 2, 512], f8, tag="wb")
            nc.vector.memset(wa[:], 0.0)
            nc.gpsimd.memset(wb[:], 0.0)
            wps = ppool.tile([128, BLK], f32, tag="m0", name="warm")
            for i in range(WARMUP2):
                nc.tensor.matmul(wps[:], wa[:], wb[:], start=True, stop=True,
                                 perf_mode=mybir.MatmulPerfMode.DoubleRow)

            def post(ps, pos, fsl, fo, eng=None):
                ko = qpool.tile([128, BLK], bf16, tag="ko",
                                name=f"ko_{pos}_{fo}")
                nc.scalar.activation(ko[:, :fsl], ps[:, :fsl],
                                     mybir.ActivationFunctionType.Square)
                # stores ride the in-order SP queue interleaved with the wt
                # refills: a store's dep (the square) clears ~1us after the
                # block's matmuls, well before the next wt panel behind it
                # is needed, and queue order keeps the DMA engines on inputs
                # first. The final store instead uses the empty Pool queue
                # (SWDGE) so it skips any backlog.
                (eng or nc.sync).dma_start(ko_d[pos, :, fo:fo + fsl],
                                           ko[:, :fsl])

            # positions 0-3: the four diagonal blocks, k-interleaved so the
            # whole opening chases the mv chunk stream
            dps = [dpool.tile([128, BLK], f32, tag=f"d{d}", name=f"dps_{d}")
                   for d in range(4)]
            for k in range(KCH):
                for d in range(4):
                    nc.tensor.matmul(
                        dps[d][:], mv[:, k, :, d * 128:(d + 1) * 128],
                        mv[:, k], start=(k == 0), stop=(k == KCH - 1),
                        perf_mode=mybir.MatmulPerfMode.DoubleRow)
            for d in range(4):
                post(dps[d], d, BLK, 0)

            # positions 4..19: off-diagonal blocks on the wt stream
            for pos in range(4, NBLK):
                n = ORDER2[pos]
                halves = ((0, BLK),) if pos < NBLK - 1 else ((0, 256),
                                                            (256, 256))
                for fo, fsl in halves:
                    ps = ppool.tile([128, BLK], f32, tag=f"m{pos % 2}",
                                    name=f"m_{pos}_{fo}")
                    for k in range(KCH):
                        nc.tensor.matmul(
                            ps[:, :fsl], wts[n][:, k], mv[:, k, :, fo:fo + fsl],
                            start=(k == 0), stop=(k == KCH - 1),
                            perf_mode=mybir.MatmulPerfMode.DoubleRow)
                    last = pos == NBLK - 1 and fo != 0
                    post(ps, pos, fsl, fo, eng=nc.gpsimd if last else None)
                # refill the 6-deep wt pipeline now that this block's
                # matmuls guard the recycled buffer
                fetch_wt(n + 6)
    return nc


_nc1 = None
_nc2 = None

PROFILE = False
LAST_PROFILE: dict = {}


def kernel(X: np.ndarray, params: np.ndarray) -> np.ndarray:
    global _nc1, _nc2
    _install_waitfix()
    X = np.asarray(X, np.float32)
    params = np.asarray(params, np.float32)

    # ---- host precompute -------------------------------------------------
    psi = _host_psi(params)
    phi = _features(X)                           # (B, DIM) f32
    perm, W0, W1 = _host_factor(psi)
    sched = _prune_schedule(W0, W1)
    phiT = np.ascontiguousarray(phi[:, perm].T)  # (DIM parity-ordered, B)

    lam_p = 64.0
    # bound |tail| <= max column norm of W so psum fits fp8 range directly
    bnd0 = float(np.sqrt((W0.astype(np.float64) ** 2).sum(axis=0).max()))
    bnd1 = float(np.sqrt((W1.astype(np.float64) ** 2).sum(axis=0).max()))
    lam_w0 = 400.0 / (lam_p * max(bnd0, 1e-30))
    lam_w1 = 400.0 / (lam_p * max(bnd1, 1e-30))
    phi8 = (phiT * lam_p).astype(npf8)           # (DIM, B)

    nchunk = sum(len(ks) for _, ks in sched)

    def pack_w(W, lam_w):
        W8 = (W * lam_w).astype(npf8)            # (HDIM u, HDIM j)
        out = np.empty((128, nchunk, 2, 128), npf8)
        idx = 0
        for c, ks in sched:
            for k in ks:
                ch = W8[k * 256:(k + 1) * 256, c * 128:(c + 1) * 128]
                out[:, idx] = ch.reshape(2, 128, 128).transpose(1, 0, 2)
                idx += 1
        return out

    w_par = [pack_w(W0, lam_w0), pack_w(W1, lam_w1)]
    phi_par = []
    for p in range(2):
        rows = phi8[p * HDIM:(p + 1) * HDIM]     # (HDIM, B)
        phi_par.append(rows.reshape(KCH1, 2, 128, B).transpose(2, 0, 1, 3))

    in_maps1 = []
    for cr in range(NCORES):
        bg, p = divmod(cr, 2)
        in_maps1.append({
            "w8": w_par[p],
            "phi": np.ascontiguousarray(phi_par[p][:, :, :,
                                                   bg * BG:(bg + 1) * BG]),
        })

    if _nc1 is None:
        _nc1 = _build_pass1(sched)
    res1 = run_bass_kernel_spmd(_nc1, in_maps1, core_ids=list(range(NCORES)))

    # ---- host mid: assemble Z, quantize ----------------------------------
    ZT = phiT                                    # reuse buffer (DIM, B)
    inv = [1.0 / (lam_p * lam_w0), 1.0 / (lam_p * lam_w1)]
    pos2c = [c for c, _ in sched]
    for cr in range(NCORES):
        bg, p = divmod(cr, 2)
        t = res1.results[cr]["t"].astype(np.float32) * inv[p]   # (128,16,1024)
        for pos in range(NJB):
            c = pos2c[pos]
            ZT[p * HDIM + c * 128:p * HDIM + (c + 1) * 128,
               bg * BG:(bg + 1) * BG] += t[:, pos]

    Z8 = (ZT * LAM).astype(npf8)                 # (DIM, B)
    Z8f32 = Z8.astype(np.float32)
    rho2 = np.einsum("jb,jb->b", Z8f32, Z8f32) / (LAM * LAM)    # (B,)
    del Z8f32
    inv_all = (1.0 / (LAM * LAM * rho2)).astype(np.float64)

    # strip layout: 16 off-diagonal col blocks (strip offsets 512..2560)
    # DMA'd as wt; the 4 diagonal blocks (offsets 0..512) slice mv.
    colrel = np.concatenate([np.arange(BLK, NB_COLS), np.arange(0, BLK)])
    Z8c = Z8.reshape(KCH, 2, 128, B)
    in_maps2 = []
    for cr in range(NCORES):
        cols = (cr * BLK + colrel) % B
        mvc = Z8c[:, :, :, cr * BLK:(cr + 1) * BLK].transpose(2, 0, 1, 3)
        wtc = Z8c[:, :, :, cols[:16 * 128]].reshape(
            KCH, 2, 128, 16, 128).transpose(3, 2, 0, 1, 4)
        in_maps2.append({
            "mv8": np.ascontiguousarray(mvc),
            "wt8": np.ascontiguousarray(wtc),
        })

    if _nc2 is None:
        _nc2 = _build_pass2()
    res2 = run_bass_kernel_spmd(_nc2, in_maps2, core_ids=list(range(NCORES)))

    # ---- assemble K (with symmetric mirroring) ---------------------------
    K = np.empty((B, B), np.float32)
    for cr in range(NCORES):
        ko = res2.results[cr]["ko"].astype(np.float64)   # (pos, 128, BLK)
        rows = slice(cr * BLK, (cr + 1) * BLK)
        invr = inv_all[rows]
        for pos in range(NBLK):
            n = ORDER2[pos]
            gs = (cr * BLK + int(colrel[n * 128])) % B
            colsl = slice(gs, gs + 128)
            blkv = (ko[pos] * inv_all[colsl, None] * invr[None, :]).astype(
                np.float32)
            K[rows, colsl] = blkv.T
            d = 1 + n // 4 if n < 16 else 0
            if 0 < d < 4 or (d == 4 and cr < 4):
                K[colsl, rows] = blkv
    return K


# revision 31
# speedup vs baseline: 4.1435x; 1.0829x over previous
"""Trainium2 Bass kernel for nn_NeuralQKM: K[i,j] = |<psi_i|psi_j>|^2.

Math. States factor as S = Phi C with product features
Phi_b[u] = prod_q (cos(X/2) if u_q=0 else sin(X/2)) and a fixed complex
matrix C[u,j] = (-1)^{|j&u|} psi'[j^u] (psi' = state after all shared
gates; the final CNOT chain is a common permutation and drops out).
The Gram G = S S^H = Phi (C C^H) Phi^T where

    (C C^H)[u,u'] = (-1)^{|u&d|} rho(d),  d = u^u',
    rho(d) = sum_k (-1)^{|k&d|} psi'[k] conj(psi'[k^d]),

so Re G = Phi Wsym Phi^T with Wsym real symmetric PSD, and Re rho(d) = 0
for odd |d| makes Wsym parity-block-diagonal. Im G vanishes on the
diagonal and contributes O(1e-6) to ||K||_F: K ~= (Re G)^2 elementwise.

Cholesky per parity block, Wsym = L L^T, gives Re G = Z Z^T with
Z = Phi L of exactly unit row norm. W = L - I is small (params are
tiny), so Z = Phi + Phi W: the main term is exact host math and only the
tail needs the device, which tolerates fp8.

Device pass 1 (4 batch-groups x 2 parities): tail^T = W^T Phi^T per
parity block, fp8 DoubleRow, keeping only the lower-triangular W chunks
whose Frobenius mass matters (~40 of 128). lam_w is sized so psum values
fit fp8 range directly: the tail streams out as fp8 with a plain copy.
Output stores ride the in-order SP queue after every input DMA so they
never preempt the input stream on the shared DMA engines.

Device pass 2 (row-sharded, block-cyclic symmetric): single-product Gram
ps = Z8_cols . Z8_rows^T; post-ops square with a per-state norm
correction K = ps^2/(LAM^4 rho_c^2 rho_r^2) (rho^2 = ||quantized Z||^2,
host-known), which cancels the dominant fp8 radial error. Diagonal
column blocks slice mv directly as the stationary operand (no wt DMA)
and two of them open the pass so compute starts after one mv chunk.
Output per core is the transposed block strip K[rows, cols].T in bf16;
host mirrors the symmetric blocks.
"""
import numpy as np
import ml_dtypes
import orjson

import concourse.bass as bass
import concourse.mybir as mybir
import concourse.tile as tile
from concourse.bass_utils import run_bass_kernel_spmd

N_QUBITS = 12
N_LAYERS = 5
DIM = 2 ** N_QUBITS          # 4096
HDIM = DIM // 2              # 2048 per parity block
B = 4096
NCORES = 8
BLK = B // NCORES            # 512 rows per core in pass 2
NDBLK = 5                    # diagonal + 4 off-diagonal column blocks
NB_COLS = NDBLK * BLK        # 2560 rhs columns per core
NBLK = NB_COLS // 128        # 20 column blocks of 128
KCH = DIM // 256             # 16 contraction chunks of K=256 (DoubleRow)
KCH1 = HDIM // 256           # 8 contraction chunks in pass 1
NJB = HDIM // 128            # 16 output column blocks in pass 1
BG = B // 4                  # 1024 samples per pass-1 batch-group
LAM = 64.0                   # fp8 quantization scale for state planes
WARMUP1 = 8                 # PE warmup matmuls, pass 1 (sim-tuned)
WARMUP2 = 7                 # PE warmup matmuls, pass 2 (sim-tuned)

# pass-2 block order: all four free-stationary diagonal blocks open the
# pass, chunk-interleaved so they chase the streaming mv chunks — PE has
# ~7us of work before the first wt panel can possibly arrive
ORDER2 = [16, 17, 18, 19] + list(range(16))

f32 = mybir.dt.float32
f8 = mybir.dt.float8e4
bf16 = mybir.dt.bfloat16
npf8 = ml_dtypes.float8_e4m3
npbf = ml_dtypes.bfloat16

# ----------------------------------------------------------------------------
# walrus in this toolchain rejects >1 sync-wait per instruction; Tile emits
# several. Engines are serial, so an extra wait is equivalent to a standalone
# EventSemaphore wait right before the instruction on the same engine.
# ----------------------------------------------------------------------------


def _legalize_multiwait_json(bir: bytes) -> bytes:
    m = orjson.loads(bir)
    changed = False
    for func in m.get("functions", []):
        for blk in func.get("blocks", []):
            out = []
            for inst in blk.get("instructions", []):
                sync = inst.get("sync_info")
                waits = (sync or {}).get("on_wait") or []
                if len(waits) > 1:
                    changed = True
                    for i, w in enumerate(waits[:-1]):
                        out.append({
                            "debug": inst.get("debug", 0),
                            "engine": inst["engine"],
                            "ins": [],
                            "name": f"{inst['name']}-xw{i}",
                            "opcode": "EventSemaphore",
                            "outs": [],
                            "sync_info": {"on_update": [], "on_wait": [w]},
                        })
                    sync["on_wait"] = [waits[-1]]
                out.append(inst)
            blk["instructions"] = out
    return orjson.dumps(m) if changed else bir


_patched = False


def _install_waitfix():
    global _patched
    if _patched:
        return
    _patched = True
    orig = bass.Bass.to_json_bytes

    def patched(self):
        return _legalize_multiwait_json(orig(self))

    bass.Bass.to_json_bytes = patched


# ----------------------------------------------------------------------------
# Host math: psi' (state after all shared circuit parts), complex64 to track
# the reference's precision.
# ----------------------------------------------------------------------------


def _host_psi(params: np.ndarray) -> np.ndarray:
    params = np.asarray(params, np.float32)
    psi = np.zeros(DIM, np.complex64)
    psi[0] = 1.0
    for l in range(N_LAYERS):
        for q in range(N_QUBITS):
            phi, theta, lam = (np.complex64(params[l, q, i]) for i in range(3))
            rz_p = np.array([[np.exp(-0.5j * phi), 0], [0, np.exp(0.5j * phi)]],
                            np.complex64)
            rz_l = np.array([[np.exp(-0.5j * lam), 0], [0, np.exp(0.5j * lam)]],
                            np.complex64)
            c, s = np.cos(0.5 * theta), np.sin(0.5 * theta)
            ry = np.array([[c, -s], [s, c]], np.complex64)
            U = rz_l @ ry @ rz_p
            # reference einsum applies U^T
            st = psi.reshape(2 ** q, 2, -1)
            psi = np.einsum("st,lsr->ltr", U, st).astype(np.complex64).reshape(-1)
        if l < N_LAYERS - 1:
            for q in range(N_QUBITS - 1):
                st = psi.reshape(2 ** q, 2, 2, -1)
                st = np.stack([st[:, 0], np.flip(st[:, 1], axis=1)], axis=1)
                psi = st.reshape(-1)
    return psi


def _features(X: np.ndarray) -> np.ndarray:
    """Phi[b, u] = prod_q (cos(X/2) if bit(11-q) of u is 0 else sin(X/2))."""
    c = np.cos(0.5 * X).astype(np.float32)
    s = np.sin(0.5 * X).astype(np.float32)
    phi = np.ones((B, 1), np.float32)
    for q in range(N_QUBITS):
        phi = np.stack([phi * c[:, q:q + 1], phi * s[:, q:q + 1]],
                       axis=2).reshape(B, -1)
    return phi


def _host_factor(psi: np.ndarray):
    """rho -> Wsym -> parity-ordered Cholesky. Returns (perm, W0, W1) with
    W = L - I per parity block (f32, strictly small)."""
    jj = np.arange(DIM)
    XORm = np.bitwise_xor.outer(jj, jj).astype(np.int32)
    ANDm = np.bitwise_and.outer(jj, jj).astype(np.int32)
    popand = np.zeros((DIM, DIM), np.int8)
    t = ANDm
    for q in range(N_QUBITS):
        popand += (t & 1).astype(np.int8)
        t = t >> 1
    del ANDm, t
    sgn_and = np.where(popand % 2 == 0, np.float32(1), np.float32(-1))
    del popand
    pop = np.zeros(DIM, np.int64)
    for q in range(N_QUBITS):
        pop += (jj >> q) & 1
    sgn = np.where(pop % 2 == 0, np.float32(1), np.float32(-1))
    par = (pop & 1).astype(np.int8)

    M = sgn_and * np.conj(psi)[XORm]          # M[d,k] = sgn(d&k) psi*_{d^k}
    rho = np.real(M @ psi).astype(np.float32)
    del M
    Wsym = (sgn[:, None] * sgn_and) * rho[XORm]
    del sgn_and, XORm

    perm = np.argsort(par, kind="stable")
    Wp = Wsym[np.ix_(perm, perm)]
    del Wsym
    L0 = np.linalg.cholesky(Wp[:HDIM, :HDIM].astype(np.float64))
    L1 = np.linalg.cholesky(Wp[HDIM:, HDIM:].astype(np.float64))
    W0 = (L0 - np.eye(HDIM)).astype(np.float32)
    W1 = (L1 - np.eye(HDIM)).astype(np.float32)
    return perm, W0, W1


def _prune_schedule(W0, W1, budget=0.09):
    """Triangular chunk list per j-block, dropping chunks whose total
    Frobenius mass stays under sqrt(budget) in both parities (the tail
    error this adds is ~1% of the fp8 noise, in quadrature). Blocks are
    scheduled descending so the earliest need the fewest phi chunks."""
    masses = []
    for c in range(NJB):
        for k in range(c // 2, KCH1):
            s0 = float((W0[k * 256:(k + 1) * 256,
                           c * 128:(c + 1) * 128].astype(np.float64) ** 2).sum())
            s1 = float((W1[k * 256:(k + 1) * 256,
                           c * 128:(c + 1) * 128].astype(np.float64) ** 2).sum())
            masses.append((max(s0, s1), c, k, s0, s1))
    masses.sort()
    drop = set()
    a0 = a1 = 0.0
    for mx, c, k, s0, s1 in masses:
        if k == c // 2 or a0 + s0 > budget or a1 + s1 > budget:
            continue
        a0 += s0
        a1 += s1
        drop.add((c, k))
    sched = []
    for c in range(NJB - 1, -1, -1):
        ks = [k for k in range(c // 2, KCH1) if (c, k) not in drop]
        sched.append((c, ks))
    # move the 1-chunk block 14 to the end: its operands are resident long
    # before, so the final matmul+copy+store tail is as short as possible
    sched.append(sched.pop(1))
    return sched


# ----------------------------------------------------------------------------
# Pass 1: tail^T = W^T Phi^T per parity block, triangular fp8 DoubleRow.
# Core cr = 2*bg + p handles batch-group bg (1024 samples), parity p.
# ----------------------------------------------------------------------------


def _build_pass1(sched) -> bass.Bass:
    nchunk = sum(len(ks) for _, ks in sched)
    nc = bass.Bass("TRN2", target_bir_lowering=False, debug=False,
                   num_devices=NCORES)
    # w8[p, idx, i, c]: chunk list in sched order; chunk (k, cblk) holds
    # lam_w * W[k*256 + i*128 + p, cblk*128 + c]
    w_d = nc.dram_tensor("w8", [128, nchunk, 2, 128], f8,
                         kind="ExternalInput").ap()
    # phi[p, k, i, b] = lam_p * Phi^T[k*256 + i*128 + p, bg*1024 + b]
    phi_d = nc.dram_tensor("phi", [128, KCH1, 2, BG], f8,
                           kind="ExternalInput").ap()
    # t[p, pos, b] = lam_p*lam_w * tail^T[cblk(pos)*128 + p, bg*1024 + b]
    t_d = nc.dram_tensor("t", [128, NJB, BG], f8, kind="ExternalOutput").ap()

    # group blocks in fours for phi/W DMA batching and output batching
    gstart = [0]
    for c, ks in sched:
        gstart.append(gstart[-1] + len(ks))

    with tile.TileContext(nc) as tc:
        with (
            tc.tile_pool(name="wpool", bufs=1) as wpool,
            tc.tile_pool(name="spool", bufs=1) as spool,
            tc.tile_pool(name="psumw", bufs=1, space="PSUM") as psumw,
            tc.tile_pool(name="psum", bufs=3, space="PSUM") as psum,
        ):
            wa = wpool.tile([128, 2, 128], f8, tag="wa")
            wb = wpool.tile([128, 2, 512], f8, tag="wb")
            nc.vector.memset(wa[:], 0.0)
            nc.gpsimd.memset(wb[:], 0.0)
            wps = psumw.tile([128, 512], f32, tag="ps0", name="warm")
            for i in range(WARMUP1):
                nc.tensor.matmul(wps[:], wa[:], wb[:], start=True, stop=True,
                                 perf_mode=mybir.MatmulPerfMode.DoubleRow)

            w8 = wpool.tile([128, nchunk, 2, 128], f8, tag="w8")
            phi = wpool.tile([128, KCH1, 2, BG], f8, tag="phi")
            st = spool.tile([128, NJB, BG], f8, tag="st")

            # in-DMA stream: per group of 4 blocks, the two new phi chunks
            # then the group's W chunks in two halves — 12 items total keeps
            # the serial HWDGE generation (~625ns each) off the critical
            # path while the ~900ns sem-visibility granularity stays fine.
            # All on the in-order SP queue; output stores ride the Pool
            # queue so inputs always win the DMA engines.
            for g in range(4):
                klo = max(6 - 2 * g, 0)
                nc.sync.dma_start(phi[:, klo:klo + 2], phi_d[:, klo:klo + 2])
                i0, im = gstart[4 * g], gstart[4 * g + 2]
                i1 = gstart[4 * g + 4]
                nc.sync.dma_start(w8[:, i0:im], w_d[:, i0:im])
                nc.sync.dma_start(w8[:, im:i1], w_d[:, im:i1])

            for pos, (c, ks) in enumerate(sched):
                ps0 = psum.tile([128, 512], f32, tag="psA", name=f"psA_{c}")
                ps1 = psum.tile([128, 512], f32, tag="psB", name=f"psB_{c}")
                i0 = gstart[pos]
                for j, k in enumerate(ks):
                    st_mm = (j == 0)
                    sp_mm = (j == len(ks) - 1)
                    wch = w8[:, i0 + j]
                    nc.tensor.matmul(ps0[:], wch, phi[:, k, :, :512],
                                     start=st_mm, stop=sp_mm,
                                     perf_mode=mybir.MatmulPerfMode.DoubleRow)
                    nc.tensor.matmul(ps1[:], wch, phi[:, k, :, 512:],
                                     start=st_mm, stop=sp_mm,
                                     perf_mode=mybir.MatmulPerfMode.DoubleRow)
                # psum values are bounded by fp8 range via lam_w: plain copy
                nc.scalar.copy(st[:, pos, :512], ps0[:])
                nc.vector.tensor_copy(st[:, pos, 512:], ps1[:])
                # stores ride the Pool queue (SWDGE): their data-dependent
                # pool requests trail the up-front input DMAs in FIFO order,
                # so inputs always win the DMA engines. Batched x4 to keep
                # the per-store descriptor generation off the critical tail;
                # the last two blocks store alone so the tail is short.
                if pos in (3, 7, 11, 13):
                    lo = {3: 0, 7: 4, 11: 8, 13: 12}[pos]
                    nc.gpsimd.dma_start(t_d[:, lo:pos + 1], st[:, lo:pos + 1])
                elif pos >= 14:
                    nc.gpsimd.dma_start(t_d[:, pos:pos + 1],
                                        st[:, pos:pos + 1])
    return nc


# ----------------------------------------------------------------------------
# Pass 2: single-product Gram + norm-corrected square, fp8 DoubleRow.
# ----------------------------------------------------------------------------


def _build_pass2() -> bass.Bass:
    nc = bass.Bass("TRN2", target_bir_lowering=False, debug=False,
                   num_devices=NCORES)
    # mv[p, kc, i, f]: Z8^T chunk of own rows (moving operand; also the
    # stationary operand for the 4 diagonal column blocks)
    mv_d = nc.dram_tensor("mv8", [128, KCH, 2, BLK], f8,
                          kind="ExternalInput").ap()
    # wt[n, p, kc, i, c]: Z8^T of off-diagonal column block n (stationary)
    wt_d = nc.dram_tensor("wt8", [NBLK - 4, 128, KCH, 2, 128], f8,
                          kind="ExternalInput").ap()
    # ko[p, pos, f]: raw squared products ps^2 = (Z8_c . Z8_r)^2; the
    # norm corrections are a host-side outer-product scaling at assembly.
    # Diagonal positions d hold only free rows [128d, 512) (staircase); the
    # host mirrors the rest.
    ko_d = nc.dram_tensor("ko", [128, NBLK, BLK], bf16,
                          kind="ExternalOutput").ap()

    with tile.TileContext(nc) as tc:
        with (
            tc.tile_pool(name="mv", bufs=1) as mpool,
            tc.tile_pool(name="wt", bufs=16) as wpool,
            tc.tile_pool(name="post", bufs=1) as qpool,
            tc.tile_pool(name="psumd", bufs=1, space="PSUM") as dpool,
            tc.tile_pool(name="psum", bufs=2, space="PSUM") as ppool,
        ):
            mv = mpool.tile([128, KCH, 2, BLK], f8, tag="mv")
            # mv streams in 8 chunks so the opening diagonal blocks can
            # chase it; wt panels follow just-in-time inside the block loop
            # (6-buffer lookahead), all on the in-order SP queue
            for h in range(8):
                nc.sync.dma_start(mv[:, 2 * h:2 * h + 2],
                                  mv_d[:, 2 * h:2 * h + 2])
            # all 16 wt panels are resident (8MB SBUF) and their DMAs are
            # emitted up-front with no waits: their pool requests all queue
            # ahead of every data-dependent output store, so the pool runs
            # [mv | wt0..wt15 | stores] back-to-back
            wts = {}
            for n in range(16):
                wt = wpool.tile([128, KCH, 2, 128], f8, tag="wt",
                                name=f"wt_{n}")
                nc.sync.dma_start(wt[:], wt_d[n])
                wts[n] = wt

            wa = mpool.tile([128, 2, 128], f8, tag="wa")
            wb = mpool.tile([128, 2, 512], f8, tag="wb")
            nc.vector.memset(wa[:], 0.0)
            nc.gpsimd.memset(wb[:], 0.0)
            wps = ppool.tile([128, BLK], f32, tag="m0", name="warm")
            for i in range(WARMUP2):
                nc.tensor.matmul(wps[:], wa[:], wb[:], start=True, stop=True,
                                 perf_mode=mybir.MatmulPerfMode.DoubleRow)

            ko = qpool.tile([128, NBLK, BLK], bf16, tag="ko")

            def post(ps, pos, fsl, fo):
                nc.scalar.activation(ko[:, pos, fo:fo + fsl], ps[:, :fsl],
                                     mybir.ActivationFunctionType.Square)

            def store(p0, p1, eng):
                # paired stores halve the serial SWDGE descriptor-generation
                # cost; their data-dependent pool requests trail the
                # up-front input DMAs in FIFO order
                eng.dma_start(ko_d[:, p0:p1], ko[:, p0:p1])

            # positions 0-3: the four diagonal blocks, k-interleaved so the
            # whole opening chases the mv chunk stream. Staircase trim:
            # block d computes only rows [128d, 512) — the host mirrors the
            # strictly-lower remainder from the transposed writes.
            dps = [dpool.tile([128, BLK], f32, tag=f"d{d}", name=f"dps_{d}")
                   for d in range(4)]
            for k in range(KCH):
                for d in range(4):
                    nc.tensor.matmul(
                        dps[d][:, :BLK - 128 * d],
                        mv[:, k, :, d * 128:(d + 1) * 128],
                        mv[:, k, :, 128 * d:], start=(k == 0),
                        stop=(k == KCH - 1),
                        perf_mode=mybir.MatmulPerfMode.DoubleRow)
            for d in range(4):
                post(dps[d], d, BLK - 128 * d, 128 * d)
            store(0, 2, nc.gpsimd)
            store(2, 4, nc.gpsimd)

            # positions 4..19: off-diagonal blocks on the wt stream
            for pos in range(4, NBLK):
                n = ORDER2[pos]
                halves = (((0, BLK),) if pos < NBLK - 2 else
                          ((0, 256), (256, 256)))
                for fo, fsl in halves:
                    ps = ppool.tile([128, BLK], f32, tag=f"m{pos % 2}",
                                    name=f"m_{pos}_{fo}")
                    for k in range(KCH):
                        nc.tensor.matmul(
                            ps[:, :fsl], wts[n][:, k], mv[:, k, :, fo:fo + fsl],
                            start=(k == 0), stop=(k == KCH - 1),
                            perf_mode=mybir.MatmulPerfMode.DoubleRow)
                    post(ps, pos, fsl, fo)
                    if pos >= NBLK - 2:
                        # tail: each final piece stores alone on the idle SP
                        # queue (HWDGE), skipping the Pool generation backlog
                        nc.sync.dma_start(ko_d[:, pos, fo:fo + fsl],
                                          ko[:, pos, fo:fo + fsl])
                if 4 < pos < NBLK - 2 and pos % 2 == 1:
                    store(pos - 1, pos + 1, nc.gpsimd)
    return nc


_nc1 = None
_nc2 = None

PROFILE = False
LAST_PROFILE: dict = {}


def kernel(X: np.ndarray, params: np.ndarray) -> np.ndarray:
    global _nc1, _nc2
    _install_waitfix()
    X = np.asarray(X, np.float32)
    params = np.asarray(params, np.float32)

    # ---- host precompute -------------------------------------------------
    psi = _host_psi(params)
    phi = _features(X)                           # (B, DIM) f32
    perm, W0, W1 = _host_factor(psi)
    sched = _prune_schedule(W0, W1)
    phiT = np.ascontiguousarray(phi[:, perm].T)  # (DIM parity-ordered, B)

    lam_p = 64.0
    # bound |tail| <= max column norm of W so psum fits fp8 range directly
    bnd0 = float(np.sqrt((W0.astype(np.float64) ** 2).sum(axis=0).max()))
    bnd1 = float(np.sqrt((W1.astype(np.float64) ** 2).sum(axis=0).max()))
    lam_w0 = 400.0 / (lam_p * max(bnd0, 1e-30))
    lam_w1 = 400.0 / (lam_p * max(bnd1, 1e-30))
    phi8 = (phiT * lam_p).astype(npf8)           # (DIM, B)

    nchunk = sum(len(ks) for _, ks in sched)

    def pack_w(W, lam_w):
        W8 = (W * lam_w).astype(npf8)            # (HDIM u, HDIM j)
        out = np.empty((128, nchunk, 2, 128), npf8)
        idx = 0
        for c, ks in sched:
            for k in ks:
                ch = W8[k * 256:(k + 1) * 256, c * 128:(c + 1) * 128]
                out[:, idx] = ch.reshape(2, 128, 128).transpose(1, 0, 2)
                idx += 1
        return out

    w_par = [pack_w(W0, lam_w0), pack_w(W1, lam_w1)]
    phi_par = []
    for p in range(2):
        rows = phi8[p * HDIM:(p + 1) * HDIM]     # (HDIM, B)
        phi_par.append(rows.reshape(KCH1, 2, 128, B).transpose(2, 0, 1, 3))

    in_maps1 = []
    for cr in range(NCORES):
        bg, p = divmod(cr, 2)
        in_maps1.append({
            "w8": w_par[p],
            "phi": np.ascontiguousarray(phi_par[p][:, :, :,
                                                   bg * BG:(bg + 1) * BG]),
        })

    if _nc1 is None:
        _nc1 = _build_pass1(sched)
    res1 = run_bass_kernel_spmd(_nc1, in_maps1, core_ids=list(range(NCORES)))

    # ---- host mid: assemble Z, quantize ----------------------------------
    ZT = phiT                                    # reuse buffer (DIM, B)
    inv = [1.0 / (lam_p * lam_w0), 1.0 / (lam_p * lam_w1)]
    pos2c = [c for c, _ in sched]
    for cr in range(NCORES):
        bg, p = divmod(cr, 2)
        t = res1.results[cr]["t"].astype(np.float32) * inv[p]   # (128,16,1024)
        for pos in range(NJB):
            c = pos2c[pos]
            ZT[p * HDIM + c * 128:p * HDIM + (c + 1) * 128,
               bg * BG:(bg + 1) * BG] += t[:, pos]

    Z8 = (ZT * LAM).astype(npf8)                 # (DIM, B)
    Z8f32 = Z8.astype(np.float32)
    rho2 = np.einsum("jb,jb->b", Z8f32, Z8f32) / (LAM * LAM)    # (B,)
    del Z8f32
    inv_all = (1.0 / (LAM * LAM * rho2)).astype(np.float64)

    # strip layout: 16 off-diagonal col blocks (strip offsets 512..2560)
    # DMA'd as wt; the 4 diagonal blocks (offsets 0..512) slice mv.
    colrel = np.concatenate([np.arange(BLK, NB_COLS), np.arange(0, BLK)])
    Z8c = Z8.reshape(KCH, 2, 128, B)
    in_maps2 = []
    for cr in range(NCORES):
        cols = (cr * BLK + colrel) % B
        mvc = Z8c[:, :, :, cr * BLK:(cr + 1) * BLK].transpose(2, 0, 1, 3)
        wtc = Z8c[:, :, :, cols[:16 * 128]].reshape(
            KCH, 2, 128, 16, 128).transpose(3, 2, 0, 1, 4)
        in_maps2.append({
            "mv8": np.ascontiguousarray(mvc),
            "wt8": np.ascontiguousarray(wtc),
        })

    if _nc2 is None:
        _nc2 = _build_pass2()
    res2 = run_bass_kernel_spmd(_nc2, in_maps2, core_ids=list(range(NCORES)))

    # ---- assemble K (with symmetric mirroring) ---------------------------
    K = np.empty((B, B), np.float32)
    for cr in range(NCORES):
        ko = res2.results[cr]["ko"].astype(np.float64)   # (128, pos, BLK)
        invr = inv_all[cr * BLK:(cr + 1) * BLK]
        for pos in range(NBLK):
            n = ORDER2[pos]
            gs = (cr * BLK + int(colrel[n * 128])) % B
            colsl = slice(gs, gs + 128)
            if n >= 16:
                fo = 128 * (n - 16)    # staircase: rows [fo, 512) only
            else:
                fo = 0
            rows = slice(cr * BLK + fo, (cr + 1) * BLK)
            blkv = (ko[:, pos, fo:] * inv_all[colsl, None]
                    * invr[None, fo:]).astype(np.float32)
            K[rows, colsl] = blkv.T
            d = 1 + n // 4 if n < 16 else 0
            if n >= 16 or 0 < d < 4 or (d == 4 and cr < 4):
                K[colsl, rows] = blkv
    return K


# revision 32
# speedup vs baseline: 4.2237x; 1.0194x over previous
"""Trainium2 Bass kernel for nn_NeuralQKM: K[i,j] = |<psi_i|psi_j>|^2.

Math. States factor as S = Phi C with product features
Phi_b[u] = prod_q (cos(X/2) if u_q=0 else sin(X/2)) and a fixed complex
matrix C[u,j] = (-1)^{|j&u|} psi'[j^u] (psi' = state after all shared
gates; the final CNOT chain is a common permutation and drops out).
The Gram G = S S^H = Phi (C C^H) Phi^T where

    (C C^H)[u,u'] = (-1)^{|u&d|} rho(d),  d = u^u',
    rho(d) = sum_k (-1)^{|k&d|} psi'[k] conj(psi'[k^d]),

so Re G = Phi Wsym Phi^T with Wsym real symmetric PSD, and Re rho(d) = 0
for odd |d| makes Wsym parity-block-diagonal. Im G vanishes on the
diagonal and contributes O(1e-6) to ||K||_F: K ~= (Re G)^2 elementwise.

Cholesky per parity block, Wsym = L L^T, gives Re G = Z Z^T with
Z = Phi L of exactly unit row norm. W = L - I is small (params are
tiny), so Z = Phi + Phi W: the main term is exact host math and only the
tail needs the device, which tolerates fp8.

Device pass 1 (4 batch-groups x 2 parities): tail^T = W^T Phi^T per
parity block, fp8 DoubleRow, keeping only the lower-triangular W chunks
whose Frobenius mass matters (~40 of 128). lam_w is sized so psum values
fit fp8 range directly: the tail streams out as fp8 with a plain copy.
Output stores ride the in-order SP queue after every input DMA so they
never preempt the input stream on the shared DMA engines.

Device pass 2 (row-sharded, block-cyclic symmetric): single-product Gram
ps = Z8_cols . Z8_rows^T; post-ops square with a per-state norm
correction K = ps^2/(LAM^4 rho_c^2 rho_r^2) (rho^2 = ||quantized Z||^2,
host-known), which cancels the dominant fp8 radial error. Diagonal
column blocks slice mv directly as the stationary operand (no wt DMA)
and two of them open the pass so compute starts after one mv chunk.
Output per core is the transposed block strip K[rows, cols].T in bf16;
host mirrors the symmetric blocks.
"""
import numpy as np
import ml_dtypes
import orjson

import concourse.bass as bass
import concourse.mybir as mybir
import concourse.tile as tile
from concourse.bass_utils import run_bass_kernel_spmd

N_QUBITS = 12
N_LAYERS = 5
DIM = 2 ** N_QUBITS          # 4096
HDIM = DIM // 2              # 2048 per parity block
B = 4096
NCORES = 8
BLK = B // NCORES            # 512 rows per core in pass 2
NDBLK = 5                    # diagonal + 4 off-diagonal column blocks
NB_COLS = NDBLK * BLK        # 2560 rhs columns per core
NBLK = NB_COLS // 128        # 20 column blocks of 128
KCH = DIM // 256             # 16 contraction chunks of K=256 (DoubleRow)
KCH1 = HDIM // 256           # 8 contraction chunks in pass 1
NJB = HDIM // 128            # 16 output column blocks in pass 1
BG = B // 4                  # 1024 samples per pass-1 batch-group
LAM = 64.0                   # fp8 quantization scale for state planes
WARMUP1 = 8                 # PE warmup matmuls, pass 1 (sim-tuned)
WARMUP2 = 7                 # PE warmup matmuls, pass 2 (sim-tuned)

# pass-2 block order: all four free-stationary diagonal blocks open the
# pass, chunk-interleaved so they chase the streaming mv chunks — PE has
# ~7us of work before the first wt panel can possibly arrive
ORDER2 = [16, 17, 18, 19] + list(range(16))

f32 = mybir.dt.float32
f8 = mybir.dt.float8e4
bf16 = mybir.dt.bfloat16
npf8 = ml_dtypes.float8_e4m3
npbf = ml_dtypes.bfloat16

# ----------------------------------------------------------------------------
# walrus in this toolchain rejects >1 sync-wait per instruction; Tile emits
# several. Engines are serial, so an extra wait is equivalent to a standalone
# EventSemaphore wait right before the instruction on the same engine.
# ----------------------------------------------------------------------------


def _legalize_multiwait_json(bir: bytes) -> bytes:
    m = orjson.loads(bir)
    changed = False
    for func in m.get("functions", []):
        for blk in func.get("blocks", []):
            out = []
            for inst in blk.get("instructions", []):
                sync = inst.get("sync_info")
                waits = (sync or {}).get("on_wait") or []
                if len(waits) > 1:
                    changed = True
                    for i, w in enumerate(waits[:-1]):
                        out.append({
                            "debug": inst.get("debug", 0),
                            "engine": inst["engine"],
                            "ins": [],
                            "name": f"{inst['name']}-xw{i}",
                            "opcode": "EventSemaphore",
                            "outs": [],
                            "sync_info": {"on_update": [], "on_wait": [w]},
                        })
                    sync["on_wait"] = [waits[-1]]
                out.append(inst)
            blk["instructions"] = out
    return orjson.dumps(m) if changed else bir


_patched = False


def _install_waitfix():
    global _patched
    if _patched:
        return
    _patched = True
    orig = bass.Bass.to_json_bytes

    def patched(self):
        return _legalize_multiwait_json(orig(self))

    bass.Bass.to_json_bytes = patched


# ----------------------------------------------------------------------------
# Host math: psi' (state after all shared circuit parts), complex64 to track
# the reference's precision.
# ----------------------------------------------------------------------------


def _host_psi(params: np.ndarray) -> np.ndarray:
    params = np.asarray(params, np.float32)
    psi = np.zeros(DIM, np.complex64)
    psi[0] = 1.0
    for l in range(N_LAYERS):
        for q in range(N_QUBITS):
            phi, theta, lam = (np.complex64(params[l, q, i]) for i in range(3))
            rz_p = np.array([[np.exp(-0.5j * phi), 0], [0, np.exp(0.5j * phi)]],
                            np.complex64)
            rz_l = np.array([[np.exp(-0.5j * lam), 0], [0, np.exp(0.5j * lam)]],
                            np.complex64)
            c, s = np.cos(0.5 * theta), np.sin(0.5 * theta)
            ry = np.array([[c, -s], [s, c]], np.complex64)
            U = rz_l @ ry @ rz_p
            # reference einsum applies U^T
            st = psi.reshape(2 ** q, 2, -1)
            psi = np.einsum("st,lsr->ltr", U, st).astype(np.complex64).reshape(-1)
        if l < N_LAYERS - 1:
            for q in range(N_QUBITS - 1):
                st = psi.reshape(2 ** q, 2, 2, -1)
                st = np.stack([st[:, 0], np.flip(st[:, 1], axis=1)], axis=1)
                psi = st.reshape(-1)
    return psi


def _features(X: np.ndarray) -> np.ndarray:
    """Phi[b, u] = prod_q (cos(X/2) if bit(11-q) of u is 0 else sin(X/2))."""
    c = np.cos(0.5 * X).astype(np.float32)
    s = np.sin(0.5 * X).astype(np.float32)
    phi = np.ones((B, 1), np.float32)
    for q in range(N_QUBITS):
        phi = np.stack([phi * c[:, q:q + 1], phi * s[:, q:q + 1]],
                       axis=2).reshape(B, -1)
    return phi


def _host_factor(psi: np.ndarray):
    """rho -> Wsym -> parity-ordered Cholesky. Returns (perm, W0, W1) with
    W = L - I per parity block (f32, strictly small)."""
    jj = np.arange(DIM)
    XORm = np.bitwise_xor.outer(jj, jj).astype(np.int32)
    ANDm = np.bitwise_and.outer(jj, jj).astype(np.int32)
    popand = np.zeros((DIM, DIM), np.int8)
    t = ANDm
    for q in range(N_QUBITS):
        popand += (t & 1).astype(np.int8)
        t = t >> 1
    del ANDm, t
    sgn_and = np.where(popand % 2 == 0, np.float32(1), np.float32(-1))
    del popand
    pop = np.zeros(DIM, np.int64)
    for q in range(N_QUBITS):
        pop += (jj >> q) & 1
    sgn = np.where(pop % 2 == 0, np.float32(1), np.float32(-1))
    par = (pop & 1).astype(np.int8)

    M = sgn_and * np.conj(psi)[XORm]          # M[d,k] = sgn(d&k) psi*_{d^k}
    rho = np.real(M @ psi).astype(np.float32)
    del M
    Wsym = (sgn[:, None] * sgn_and) * rho[XORm]
    del sgn_and, XORm

    perm = np.argsort(par, kind="stable")
    Wp = Wsym[np.ix_(perm, perm)]
    del Wsym
    L0 = np.linalg.cholesky(Wp[:HDIM, :HDIM].astype(np.float64))
    L1 = np.linalg.cholesky(Wp[HDIM:, HDIM:].astype(np.float64))
    W0 = (L0 - np.eye(HDIM)).astype(np.float32)
    W1 = (L1 - np.eye(HDIM)).astype(np.float32)
    return perm, W0, W1


def _prune_schedule(W0, W1, budget=0.81):
    """Triangular chunk list per j-block, dropping chunks whose total
    Frobenius mass stays under sqrt(budget) in both parities (measured:
    the dropped-tail error is white noise far under the fp8 noise; rel
    err moves 3.72e-3 -> 3.75e-3). Blocks are scheduled descending so
    the earliest need the fewest phi chunks."""
    masses = []
    for c in range(NJB):
        for k in range(c // 2, KCH1):
            s0 = float((W0[k * 256:(k + 1) * 256,
                           c * 128:(c + 1) * 128].astype(np.float64) ** 2).sum())
            s1 = float((W1[k * 256:(k + 1) * 256,
                           c * 128:(c + 1) * 128].astype(np.float64) ** 2).sum())
            masses.append((max(s0, s1), c, k, s0, s1))
    masses.sort()
    drop = set()
    a0 = a1 = 0.0
    for mx, c, k, s0, s1 in masses:
        if k == c // 2 or a0 + s0 > budget or a1 + s1 > budget:
            continue
        a0 += s0
        a1 += s1
        drop.add((c, k))
    sched = []
    for c in range(NJB - 1, -1, -1):
        ks = [k for k in range(c // 2, KCH1) if (c, k) not in drop]
        sched.append((c, ks))
    # move the 1-chunk block 14 to the end: its operands are resident long
    # before, so the final matmul+copy+store tail is as short as possible
    sched.append(sched.pop(1))
    return sched


# ----------------------------------------------------------------------------
# Pass 1: tail^T = W^T Phi^T per parity block, triangular fp8 DoubleRow.
# Core cr = 2*bg + p handles batch-group bg (1024 samples), parity p.
# ----------------------------------------------------------------------------


def _build_pass1(sched) -> bass.Bass:
    nchunk = sum(len(ks) for _, ks in sched)
    nc = bass.Bass("TRN2", target_bir_lowering=False, debug=False,
                   num_devices=NCORES)
    # w8[p, idx, i, c]: chunk list in sched order; chunk (k, cblk) holds
    # lam_w * W[k*256 + i*128 + p, cblk*128 + c]
    w_d = nc.dram_tensor("w8", [128, nchunk, 2, 128], f8,
                         kind="ExternalInput").ap()
    # phi[p, k, i, b] = lam_p * Phi^T[k*256 + i*128 + p, bg*1024 + b]
    phi_d = nc.dram_tensor("phi", [128, KCH1, 2, BG], f8,
                           kind="ExternalInput").ap()
    # t[p, pos, b] = lam_p*lam_w * tail^T[cblk(pos)*128 + p, bg*1024 + b]
    t_d = nc.dram_tensor("t", [128, NJB, BG], f8, kind="ExternalOutput").ap()

    # group blocks in fours for phi/W DMA batching and output batching
    gstart = [0]
    for c, ks in sched:
        gstart.append(gstart[-1] + len(ks))

    with tile.TileContext(nc) as tc:
        with (
            tc.tile_pool(name="wpool", bufs=1) as wpool,
            tc.tile_pool(name="spool", bufs=1) as spool,
            tc.tile_pool(name="psumw", bufs=1, space="PSUM") as psumw,
            tc.tile_pool(name="psum", bufs=3, space="PSUM") as psum,
        ):
            wa = wpool.tile([128, 2, 128], f8, tag="wa")
            wb = wpool.tile([128, 2, 512], f8, tag="wb")
            nc.vector.memset(wa[:], 0.0)
            nc.gpsimd.memset(wb[:], 0.0)
            wps = psumw.tile([128, 512], f32, tag="ps0", name="warm")
            for i in range(WARMUP1):
                nc.tensor.matmul(wps[:], wa[:], wb[:], start=True, stop=True,
                                 perf_mode=mybir.MatmulPerfMode.DoubleRow)

            w8 = wpool.tile([128, nchunk, 2, 128], f8, tag="w8")
            phi = wpool.tile([128, KCH1, 2, BG], f8, tag="phi")
            st = spool.tile([128, NJB, BG], f8, tag="st")

            # in-DMA stream: per group of 4 blocks, the two new phi chunks
            # then the group's W chunks in two halves — 12 items total keeps
            # the serial HWDGE generation (~625ns each) off the critical
            # path while the ~900ns sem-visibility granularity stays fine.
            # All on the in-order SP queue; output stores ride the Pool
            # queue so inputs always win the DMA engines.
            for g in range(4):
                klo = max(6 - 2 * g, 0)
                nc.sync.dma_start(phi[:, klo:klo + 2], phi_d[:, klo:klo + 2])
                i0, im = gstart[4 * g], gstart[4 * g + 2]
                i1 = gstart[4 * g + 4]
                nc.sync.dma_start(w8[:, i0:im], w_d[:, i0:im])
                nc.sync.dma_start(w8[:, im:i1], w_d[:, im:i1])

            for pos, (c, ks) in enumerate(sched):
                ps0 = psum.tile([128, 512], f32, tag="psA", name=f"psA_{c}")
                ps1 = psum.tile([128, 512], f32, tag="psB", name=f"psB_{c}")
                i0 = gstart[pos]
                for j, k in enumerate(ks):
                    st_mm = (j == 0)
                    sp_mm = (j == len(ks) - 1)
                    wch = w8[:, i0 + j]
                    nc.tensor.matmul(ps0[:], wch, phi[:, k, :, :512],
                                     start=st_mm, stop=sp_mm,
                                     perf_mode=mybir.MatmulPerfMode.DoubleRow)
                    nc.tensor.matmul(ps1[:], wch, phi[:, k, :, 512:],
                                     start=st_mm, stop=sp_mm,
                                     perf_mode=mybir.MatmulPerfMode.DoubleRow)
                # psum values are bounded by fp8 range via lam_w: plain copy
                nc.scalar.copy(st[:, pos, :512], ps0[:])
                nc.vector.tensor_copy(st[:, pos, 512:], ps1[:])
                # stores ride the Pool queue (SWDGE): their data-dependent
                # pool requests trail the up-front input DMAs in FIFO order,
                # so inputs always win the DMA engines. Batched x4 to keep
                # the per-store descriptor generation off the critical tail;
                # the last two blocks store alone on the idle SP queue
                # (HWDGE path) so the final chain is short.
                if pos in (3, 7, 11, 13):
                    lo = {3: 0, 7: 4, 11: 8, 13: 12}[pos]
                    nc.gpsimd.dma_start(t_d[:, lo:pos + 1], st[:, lo:pos + 1])
                elif pos >= 14:
                    nc.sync.dma_start(t_d[:, pos:pos + 1],
                                      st[:, pos:pos + 1])
    return nc


# ----------------------------------------------------------------------------
# Pass 2: single-product Gram + norm-corrected square, fp8 DoubleRow.
# ----------------------------------------------------------------------------


def _build_pass2() -> bass.Bass:
    nc = bass.Bass("TRN2", target_bir_lowering=False, debug=False,
                   num_devices=NCORES)
    # mv[p, kc, i, f]: Z8^T chunk of own rows (moving operand; also the
    # stationary operand for the 4 diagonal column blocks)
    mv_d = nc.dram_tensor("mv8", [128, KCH, 2, BLK], f8,
                          kind="ExternalInput").ap()
    # wt[n, p, kc, i, c]: Z8^T of off-diagonal column block n (stationary)
    wt_d = nc.dram_tensor("wt8", [NBLK - 4, 128, KCH, 2, 128], f8,
                          kind="ExternalInput").ap()
    # ko[p, pos, f]: raw squared products ps^2 = (Z8_c . Z8_r)^2; the
    # norm corrections are a host-side outer-product scaling at assembly.
    # Diagonal positions d hold only free rows [128d, 512) (staircase); the
    # host mirrors the rest.
    ko_d = nc.dram_tensor("ko", [128, NBLK, BLK], bf16,
                          kind="ExternalOutput").ap()

    with tile.TileContext(nc) as tc:
        with (
            tc.tile_pool(name="mv", bufs=1) as mpool,
            tc.tile_pool(name="wt", bufs=16) as wpool,
            tc.tile_pool(name="post", bufs=1) as qpool,
            tc.tile_pool(name="psumd", bufs=1, space="PSUM") as dpool,
            tc.tile_pool(name="psum", bufs=2, space="PSUM") as ppool,
        ):
            mv = mpool.tile([128, KCH, 2, BLK], f8, tag="mv")
            # mv streams in 8 chunks so the opening diagonal blocks can
            # chase it; wt panels follow just-in-time inside the block loop
            # (6-buffer lookahead), all on the in-order SP queue
            for h in range(8):
                nc.sync.dma_start(mv[:, 2 * h:2 * h + 2],
                                  mv_d[:, 2 * h:2 * h + 2])
            # all 16 wt panels are resident (8MB SBUF) and their DMAs are
            # emitted up-front with no waits: their pool requests all queue
            # ahead of every data-dependent output store, so the pool runs
            # [mv | wt0..wt15 | stores] back-to-back
            wts = {}
            for n in range(16):
                wt = wpool.tile([128, KCH, 2, 128], f8, tag="wt",
                                name=f"wt_{n}")
                nc.sync.dma_start(wt[:], wt_d[n])
                wts[n] = wt

            wa = mpool.tile([128, 2, 128], f8, tag="wa")
            wb = mpool.tile([128, 2, 512], f8, tag="wb")
            nc.vector.memset(wa[:], 0.0)
            nc.gpsimd.memset(wb[:], 0.0)
            wps = ppool.tile([128, BLK], f32, tag="m0", name="warm")
            for i in range(WARMUP2):
                nc.tensor.matmul(wps[:], wa[:], wb[:], start=True, stop=True,
                                 perf_mode=mybir.MatmulPerfMode.DoubleRow)

            ko = qpool.tile([128, NBLK, BLK], bf16, tag="ko")

            def post(ps, pos, fsl, fo):
                nc.scalar.activation(ko[:, pos, fo:fo + fsl], ps[:, :fsl],
                                     mybir.ActivationFunctionType.Square)

            def store(p0, p1, eng):
                # paired stores halve the serial SWDGE descriptor-generation
                # cost; their data-dependent pool requests trail the
                # up-front input DMAs in FIFO order
                eng.dma_start(ko_d[:, p0:p1], ko[:, p0:p1])

            # positions 0-3: the four diagonal blocks, k-interleaved so the
            # whole opening chases the mv chunk stream. Staircase trim:
            # block d computes only rows [128d, 512) — the host mirrors the
            # strictly-lower remainder from the transposed writes.
            dps = [dpool.tile([128, BLK], f32, tag=f"d{d}", name=f"dps_{d}")
                   for d in range(4)]
            for k in range(KCH):
                for d in range(4):
                    nc.tensor.matmul(
                        dps[d][:, :BLK - 128 * d],
                        mv[:, k, :, d * 128:(d + 1) * 128],
                        mv[:, k, :, 128 * d:], start=(k == 0),
                        stop=(k == KCH - 1),
                        perf_mode=mybir.MatmulPerfMode.DoubleRow)
            for d in range(4):
                post(dps[d], d, BLK - 128 * d, 128 * d)
            store(0, 2, nc.gpsimd)
            store(2, 4, nc.gpsimd)

            # positions 4..19: off-diagonal blocks on the wt stream
            for pos in range(4, NBLK):
                n = ORDER2[pos]
                halves = (((0, BLK),) if pos < NBLK - 2 else
                          ((0, 256), (256, 256)))
                for fo, fsl in halves:
                    ps = ppool.tile([128, BLK], f32, tag=f"m{pos % 2}",
                                    name=f"m_{pos}_{fo}")
                    for k in range(KCH):
                        nc.tensor.matmul(
                            ps[:, :fsl], wts[n][:, k], mv[:, k, :, fo:fo + fsl],
                            start=(k == 0), stop=(k == KCH - 1),
                            perf_mode=mybir.MatmulPerfMode.DoubleRow)
                    post(ps, pos, fsl, fo)
                    if pos >= NBLK - 2:
                        # tail: each final piece stores alone on the idle SP
                        # queue (HWDGE), skipping the Pool generation backlog
                        nc.sync.dma_start(ko_d[:, pos, fo:fo + fsl],
                                          ko[:, pos, fo:fo + fsl])
                if 4 < pos < NBLK - 2 and pos % 2 == 1:
                    store(pos - 1, pos + 1, nc.gpsimd)
    return nc


_nc1 = None
_nc2 = None

PROFILE = False
LAST_PROFILE: dict = {}


def kernel(X: np.ndarray, params: np.ndarray) -> np.ndarray:
    global _nc1, _nc2
    _install_waitfix()
    X = np.asarray(X, np.float32)
    params = np.asarray(params, np.float32)

    # ---- host precompute -------------------------------------------------
    psi = _host_psi(params)
    phi = _features(X)                           # (B, DIM) f32
    perm, W0, W1 = _host_factor(psi)
    sched = _prune_schedule(W0, W1)
    phiT = np.ascontiguousarray(phi[:, perm].T)  # (DIM parity-ordered, B)

    lam_p = 64.0
    # bound |tail| <= max column norm of W so psum fits fp8 range directly
    bnd0 = float(np.sqrt((W0.astype(np.float64) ** 2).sum(axis=0).max()))
    bnd1 = float(np.sqrt((W1.astype(np.float64) ** 2).sum(axis=0).max()))
    lam_w0 = 400.0 / (lam_p * max(bnd0, 1e-30))
    lam_w1 = 400.0 / (lam_p * max(bnd1, 1e-30))
    phi8 = (phiT * lam_p).astype(npf8)           # (DIM, B)

    nchunk = sum(len(ks) for _, ks in sched)

    def pack_w(W, lam_w):
        W8 = (W * lam_w).astype(npf8)            # (HDIM u, HDIM j)
        out = np.empty((128, nchunk, 2, 128), npf8)
        idx = 0
        for c, ks in sched:
            for k in ks:
                ch = W8[k * 256:(k + 1) * 256, c * 128:(c + 1) * 128]
                out[:, idx] = ch.reshape(2, 128, 128).transpose(1, 0, 2)
                idx += 1
        return out

    w_par = [pack_w(W0, lam_w0), pack_w(W1, lam_w1)]
    phi_par = []
    for p in range(2):
        rows = phi8[p * HDIM:(p + 1) * HDIM]     # (HDIM, B)
        phi_par.append(rows.reshape(KCH1, 2, 128, B).transpose(2, 0, 1, 3))

    in_maps1 = []
    for cr in range(NCORES):
        bg, p = divmod(cr, 2)
        in_maps1.append({
            "w8": w_par[p],
            "phi": np.ascontiguousarray(phi_par[p][:, :, :,
                                                   bg * BG:(bg + 1) * BG]),
        })

    if _nc1 is None:
        _nc1 = _build_pass1(sched)
    res1 = run_bass_kernel_spmd(_nc1, in_maps1, core_ids=list(range(NCORES)))

    # ---- host mid: assemble Z, quantize ----------------------------------
    ZT = phiT                                    # reuse buffer (DIM, B)
    inv = [1.0 / (lam_p * lam_w0), 1.0 / (lam_p * lam_w1)]
    pos2c = [c for c, _ in sched]
    for cr in range(NCORES):
        bg, p = divmod(cr, 2)
        t = res1.results[cr]["t"].astype(np.float32) * inv[p]   # (128,16,1024)
        for pos in range(NJB):
            c = pos2c[pos]
            ZT[p * HDIM + c * 128:p * HDIM + (c + 1) * 128,
               bg * BG:(bg + 1) * BG] += t[:, pos]

    Z8 = (ZT * LAM).astype(npf8)                 # (DIM, B)
    Z8f32 = Z8.astype(np.float32)
    rho2 = np.einsum("jb,jb->b", Z8f32, Z8f32) / (LAM * LAM)    # (B,)
    del Z8f32
    inv_all = (1.0 / (LAM * LAM * rho2)).astype(np.float64)

    # strip layout: 16 off-diagonal col blocks (strip offsets 512..2560)
    # DMA'd as wt; the 4 diagonal blocks (offsets 0..512) slice mv.
    colrel = np.concatenate([np.arange(BLK, NB_COLS), np.arange(0, BLK)])
    Z8c = Z8.reshape(KCH, 2, 128, B)
    in_maps2 = []
    for cr in range(NCORES):
        cols = (cr * BLK + colrel) % B
        mvc = Z8c[:, :, :, cr * BLK:(cr + 1) * BLK].transpose(2, 0, 1, 3)
        wtc = Z8c[:, :, :, cols[:16 * 128]].reshape(
            KCH, 2, 128, 16, 128).transpose(3, 2, 0, 1, 4)
        in_maps2.append({
            "mv8": np.ascontiguousarray(mvc),
            "wt8": np.ascontiguousarray(wtc),
        })

    if _nc2 is None:
        _nc2 = _build_pass2()
    res2 = run_bass_kernel_spmd(_nc2, in_maps2, core_ids=list(range(NCORES)))

    # ---- assemble K (with symmetric mirroring) ---------------------------
    K = np.empty((B, B), np.float32)
    for cr in range(NCORES):
        ko = res2.results[cr]["ko"].astype(np.float64)   # (128, pos, BLK)
        invr = inv_all[cr * BLK:(cr + 1) * BLK]
        for pos in range(NBLK):
            n = ORDER2[pos]
            gs = (cr * BLK + int(colrel[n * 128])) % B
            colsl = slice(gs, gs + 128)
            if n >= 16:
                fo = 128 * (n - 16)    # staircase: rows [fo, 512) only
            else:
                fo = 0
            rows = slice(cr * BLK + fo, (cr + 1) * BLK)
            blkv = (ko[:, pos, fo:] * inv_all[colsl, None]
                    * invr[None, fo:]).astype(np.float32)
            K[rows, colsl] = blkv.T
            d = 1 + n // 4 if n < 16 else 0
            if n >= 16 or 0 < d < 4 or (d == 4 and cr < 4):
                K[colsl, rows] = blkv
    return K


# revision 34
# speedup vs baseline: 4.2334x; 1.0023x over previous
"""Trainium2 Bass kernel for nn_NeuralQKM: K[i,j] = |<psi_i|psi_j>|^2.

Math. States factor as S = Phi C with product features
Phi_b[u] = prod_q (cos(X/2) if u_q=0 else sin(X/2)) and a fixed complex
matrix C[u,j] = (-1)^{|j&u|} psi'[j^u] (psi' = state after all shared
gates; the final CNOT chain is a common permutation and drops out).
The Gram G = S S^H = Phi (C C^H) Phi^T where

    (C C^H)[u,u'] = (-1)^{|u&d|} rho(d),  d = u^u',
    rho(d) = sum_k (-1)^{|k&d|} psi'[k] conj(psi'[k^d]),

so Re G = Phi Wsym Phi^T with Wsym real symmetric PSD, and Re rho(d) = 0
for odd |d| makes Wsym parity-block-diagonal. Im G vanishes on the
diagonal and contributes O(1e-6) to ||K||_F: K ~= (Re G)^2 elementwise.

Cholesky per parity block, Wsym = L L^T, gives Re G = Z Z^T with
Z = Phi L of exactly unit row norm. W = L - I is small (params are
tiny), so Z = Phi + Phi W: the main term is exact host math and only the
tail needs the device, which tolerates fp8.

Device pass 1 (4 batch-groups x 2 parities): tail^T = W^T Phi^T per
parity block, fp8 DoubleRow, keeping only the lower-triangular W chunks
whose Frobenius mass matters (~19 of 136; the dropped mass is white
noise far below the pass-2 fp8 noise). lam_w is sized so psum values
fit fp8 range directly: the tail streams out as fp8 with a plain copy.
The pass is paced by the PSUM->SBUF drain (only ACT and DVE reach PSUM)
and by the shared DMA engines; input DMAs are batched up-front on the
SP queue and stores trail them in pool FIFO order.

Device pass 2 (row-sharded, block-cyclic symmetric): single-product
Gram ps = Z8_cols . Z8_rows^T, squared on ACT into bf16; all norm
corrections K = ps^2/(LAM^4 rho_c^2 rho_r^2) (rho^2 = ||quantized Z||^2)
are host-side outer-product scalings at assembly, cancelling the
dominant fp8 radial error. The four diagonal column blocks slice mv
directly as stationary (no wt DMA), open the pass chasing the streaming
mv chunks, and compute only their upper staircase (rows >= col block;
host mirrors). All 16 wt panels are fetched up-front into resident SBUF
tiles so their pool requests precede every data-dependent store; the
pool runs [mv | wt0..15 | stores] with zero PE exposure. Host mirrors
the symmetric blocks at assembly.
"""
import numpy as np
import ml_dtypes
import orjson

import concourse.bass as bass
import concourse.mybir as mybir
import concourse.tile as tile
from concourse.bass_utils import run_bass_kernel_spmd

N_QUBITS = 12
N_LAYERS = 5
DIM = 2 ** N_QUBITS          # 4096
HDIM = DIM // 2              # 2048 per parity block
B = 4096
NCORES = 8
BLK = B // NCORES            # 512 rows per core in pass 2
NDBLK = 5                    # diagonal + 4 off-diagonal column blocks
NB_COLS = NDBLK * BLK        # 2560 rhs columns per core
NBLK = NB_COLS // 128        # 20 column blocks of 128
KCH = DIM // 256             # 16 contraction chunks of K=256 (DoubleRow)
KCH1 = HDIM // 256           # 8 contraction chunks in pass 1
NJB = HDIM // 128            # 16 output column blocks in pass 1
BG = B // 4                  # 1024 samples per pass-1 batch-group
LAM = 64.0                   # fp8 quantization scale for state planes
WARMUP1 = 8                 # PE warmup matmuls, pass 1 (sim-tuned)
WARMUP2 = 7                 # PE warmup matmuls, pass 2 (sim-tuned)

# pass-2 block order: all four free-stationary diagonal blocks open the
# pass, chunk-interleaved so they chase the streaming mv chunks — PE has
# ~7us of work before the first wt panel can possibly arrive
ORDER2 = [16, 17, 18, 19] + list(range(16))

f32 = mybir.dt.float32
f8 = mybir.dt.float8e4
bf16 = mybir.dt.bfloat16
npf8 = ml_dtypes.float8_e4m3
npbf = ml_dtypes.bfloat16

# ----------------------------------------------------------------------------
# walrus in this toolchain rejects >1 sync-wait per instruction; Tile emits
# several. Engines are serial, so an extra wait is equivalent to a standalone
# EventSemaphore wait right before the instruction on the same engine.
# ----------------------------------------------------------------------------


def _legalize_multiwait_json(bir: bytes) -> bytes:
    m = orjson.loads(bir)
    changed = False
    for func in m.get("functions", []):
        for blk in func.get("blocks", []):
            out = []
            for inst in blk.get("instructions", []):
                sync = inst.get("sync_info")
                waits = (sync or {}).get("on_wait") or []
                if len(waits) > 1:
                    changed = True
                    for i, w in enumerate(waits[:-1]):
                        out.append({
                            "debug": inst.get("debug", 0),
                            "engine": inst["engine"],
                            "ins": [],
                            "name": f"{inst['name']}-xw{i}",
                            "opcode": "EventSemaphore",
                            "outs": [],
                            "sync_info": {"on_update": [], "on_wait": [w]},
                        })
                    sync["on_wait"] = [waits[-1]]
                out.append(inst)
            blk["instructions"] = out
    return orjson.dumps(m) if changed else bir


_patched = False


def _install_waitfix():
    global _patched
    if _patched:
        return
    _patched = True
    orig = bass.Bass.to_json_bytes

    def patched(self):
        return _legalize_multiwait_json(orig(self))

    bass.Bass.to_json_bytes = patched


# ----------------------------------------------------------------------------
# Host math: psi' (state after all shared circuit parts), complex64 to track
# the reference's precision.
# ----------------------------------------------------------------------------


def _host_psi(params: np.ndarray) -> np.ndarray:
    params = np.asarray(params, np.float32)
    psi = np.zeros(DIM, np.complex64)
    psi[0] = 1.0
    for l in range(N_LAYERS):
        for q in range(N_QUBITS):
            phi, theta, lam = (np.complex64(params[l, q, i]) for i in range(3))
            rz_p = np.array([[np.exp(-0.5j * phi), 0], [0, np.exp(0.5j * phi)]],
                            np.complex64)
            rz_l = np.array([[np.exp(-0.5j * lam), 0], [0, np.exp(0.5j * lam)]],
                            np.complex64)
            c, s = np.cos(0.5 * theta), np.sin(0.5 * theta)
            ry = np.array([[c, -s], [s, c]], np.complex64)
            U = rz_l @ ry @ rz_p
            # reference einsum applies U^T
            st = psi.reshape(2 ** q, 2, -1)
            psi = np.einsum("st,lsr->ltr", U, st).astype(np.complex64).reshape(-1)
        if l < N_LAYERS - 1:
            for q in range(N_QUBITS - 1):
                st = psi.reshape(2 ** q, 2, 2, -1)
                st = np.stack([st[:, 0], np.flip(st[:, 1], axis=1)], axis=1)
                psi = st.reshape(-1)
    return psi


def _features(X: np.ndarray) -> np.ndarray:
    """Phi[b, u] = prod_q (cos(X/2) if bit(11-q) of u is 0 else sin(X/2))."""
    c = np.cos(0.5 * X).astype(np.float32)
    s = np.sin(0.5 * X).astype(np.float32)
    phi = np.ones((B, 1), np.float32)
    for q in range(N_QUBITS):
        phi = np.stack([phi * c[:, q:q + 1], phi * s[:, q:q + 1]],
                       axis=2).reshape(B, -1)
    return phi


def _host_factor(psi: np.ndarray):
    """rho -> Wsym -> parity-ordered Cholesky. Returns (perm, W0, W1) with
    W = L - I per parity block (f32, strictly small)."""
    jj = np.arange(DIM)
    XORm = np.bitwise_xor.outer(jj, jj).astype(np.int32)
    ANDm = np.bitwise_and.outer(jj, jj).astype(np.int32)
    popand = np.zeros((DIM, DIM), np.int8)
    t = ANDm
    for q in range(N_QUBITS):
        popand += (t & 1).astype(np.int8)
        t = t >> 1
    del ANDm, t
    sgn_and = np.where(popand % 2 == 0, np.float32(1), np.float32(-1))
    del popand
    pop = np.zeros(DIM, np.int64)
    for q in range(N_QUBITS):
        pop += (jj >> q) & 1
    sgn = np.where(pop % 2 == 0, np.float32(1), np.float32(-1))
    par = (pop & 1).astype(np.int8)

    M = sgn_and * np.conj(psi)[XORm]          # M[d,k] = sgn(d&k) psi*_{d^k}
    rho = np.real(M @ psi).astype(np.float32)
    del M
    Wsym = (sgn[:, None] * sgn_and) * rho[XORm]
    del sgn_and, XORm

    perm = np.argsort(par, kind="stable")
    Wp = Wsym[np.ix_(perm, perm)]
    del Wsym
    L0 = np.linalg.cholesky(Wp[:HDIM, :HDIM].astype(np.float64))
    L1 = np.linalg.cholesky(Wp[HDIM:, HDIM:].astype(np.float64))
    W0 = (L0 - np.eye(HDIM)).astype(np.float32)
    W1 = (L1 - np.eye(HDIM)).astype(np.float32)
    return perm, W0, W1


def _prune_schedule(W0, W1, budget=0.81):
    """Triangular chunk list per j-block, dropping chunks whose total
    Frobenius mass stays under sqrt(budget) in both parities (measured:
    the dropped-tail error is white noise far under the fp8 noise; rel
    err moves 3.72e-3 -> 3.75e-3). Blocks are scheduled descending so
    the earliest need the fewest phi chunks."""
    masses = []
    for c in range(NJB):
        for k in range(c // 2, KCH1):
            s0 = float((W0[k * 256:(k + 1) * 256,
                           c * 128:(c + 1) * 128].astype(np.float64) ** 2).sum())
            s1 = float((W1[k * 256:(k + 1) * 256,
                           c * 128:(c + 1) * 128].astype(np.float64) ** 2).sum())
            masses.append((max(s0, s1), c, k, s0, s1))
    masses.sort()
    drop = set()
    a0 = a1 = 0.0
    for mx, c, k, s0, s1 in masses:
        if k == c // 2 or a0 + s0 > budget or a1 + s1 > budget:
            continue
        a0 += s0
        a1 += s1
        drop.add((c, k))
    sched = []
    for c in range(NJB - 1, -1, -1):
        ks = [k for k in range(c // 2, KCH1) if (c, k) not in drop]
        sched.append((c, ks))
    # move the 1-chunk block 14 to the end: its operands are resident long
    # before, so the final matmul+copy+store tail is as short as possible
    sched.append(sched.pop(1))
    return sched


# ----------------------------------------------------------------------------
# Pass 1: tail^T = W^T Phi^T per parity block, triangular fp8 DoubleRow.
# Core cr = 2*bg + p handles batch-group bg (1024 samples), parity p.
# ----------------------------------------------------------------------------


def _build_pass1(sched) -> bass.Bass:
    nchunk = sum(len(ks) for _, ks in sched)
    nc = bass.Bass("TRN2", target_bir_lowering=False, debug=False,
                   num_devices=NCORES)
    # w8[p, idx, i, c]: chunk list in sched order; chunk (k, cblk) holds
    # lam_w * W[k*256 + i*128 + p, cblk*128 + c]
    w_d = nc.dram_tensor("w8", [128, nchunk, 2, 128], f8,
                         kind="ExternalInput").ap()
    # phi[p, k, i, b] = lam_p * Phi^T[k*256 + i*128 + p, bg*1024 + b]
    phi_d = nc.dram_tensor("phi", [128, KCH1, 2, BG], f8,
                           kind="ExternalInput").ap()
    # t[p, pos, b] = lam_p*lam_w * tail^T[cblk(pos)*128 + p, bg*1024 + b]
    t_d = nc.dram_tensor("t", [128, NJB, BG], f8, kind="ExternalOutput").ap()

    # group blocks in fours for phi/W DMA batching and output batching
    gstart = [0]
    for c, ks in sched:
        gstart.append(gstart[-1] + len(ks))

    with tile.TileContext(nc) as tc:
        with (
            tc.tile_pool(name="wpool", bufs=1) as wpool,
            tc.tile_pool(name="spool", bufs=1) as spool,
            tc.tile_pool(name="psumw", bufs=1, space="PSUM") as psumw,
            tc.tile_pool(name="psum", bufs=3, space="PSUM") as psum,
        ):
            wa = wpool.tile([128, 2, 128], f8, tag="wa")
            wb = wpool.tile([128, 2, 512], f8, tag="wb")
            nc.vector.memset(wa[:], 0.0)
            nc.gpsimd.memset(wb[:], 0.0)
            wps = psumw.tile([128, 512], f32, tag="ps0", name="warm")
            for i in range(WARMUP1):
                nc.tensor.matmul(wps[:], wa[:], wb[:], start=True, stop=True,
                                 perf_mode=mybir.MatmulPerfMode.DoubleRow)

            w8 = wpool.tile([128, nchunk, 2, 128], f8, tag="w8")
            phi = wpool.tile([128, KCH1, 2, BG], f8, tag="phi")
            st = spool.tile([128, NJB, BG], f8, tag="st")

            # in-DMA stream: per group of 4 blocks, the two new phi chunks
            # then the group's W chunks in two halves — 12 items total keeps
            # the serial HWDGE generation (~625ns each) off the critical
            # path while the ~900ns sem-visibility granularity stays fine.
            # All on the in-order SP queue; output stores ride the Pool
            # queue so inputs always win the DMA engines.
            for g in range(4):
                klo = max(6 - 2 * g, 0)
                nc.sync.dma_start(phi[:, klo:klo + 2], phi_d[:, klo:klo + 2])
                i0, im = gstart[4 * g], gstart[4 * g + 2]
                i1 = gstart[4 * g + 4]
                nc.sync.dma_start(w8[:, i0:im], w_d[:, i0:im])
                nc.sync.dma_start(w8[:, im:i1], w_d[:, im:i1])

            for pos, (c, ks) in enumerate(sched):
                ps0 = psum.tile([128, 512], f32, tag="psA", name=f"psA_{c}")
                ps1 = psum.tile([128, 512], f32, tag="psB", name=f"psB_{c}")
                i0 = gstart[pos]
                for j, k in enumerate(ks):
                    st_mm = (j == 0)
                    sp_mm = (j == len(ks) - 1)
                    wch = w8[:, i0 + j]
                    nc.tensor.matmul(ps0[:], wch, phi[:, k, :, :512],
                                     start=st_mm, stop=sp_mm,
                                     perf_mode=mybir.MatmulPerfMode.DoubleRow)
                    nc.tensor.matmul(ps1[:], wch, phi[:, k, :, 512:],
                                     start=st_mm, stop=sp_mm,
                                     perf_mode=mybir.MatmulPerfMode.DoubleRow)
                # psum values are bounded by fp8 range via lam_w: plain copy
                nc.scalar.copy(st[:, pos, :512], ps0[:])
                nc.vector.tensor_copy(st[:, pos, 512:], ps1[:])
                # stores alternate between the Pool queue (SWDGE) and the
                # idle SP queue (HWDGE): data-dependent requests trail the
                # up-front input DMAs in pool FIFO order, so inputs always
                # win the DMA engines, and the two DGE paths pipeline their
                # per-store generation in parallel. Pairs keep generation
                # cost low; the final two blocks store alone.
                if pos >= 14:
                    nc.sync.dma_start(t_d[:, pos:pos + 1],
                                      st[:, pos:pos + 1])
                elif pos % 2 == 1:
                    eng = nc.gpsimd if pos % 4 == 1 else nc.sync
                    eng.dma_start(t_d[:, pos - 1:pos + 1],
                                  st[:, pos - 1:pos + 1])
    return nc


# ----------------------------------------------------------------------------
# Pass 2: single-product Gram + norm-corrected square, fp8 DoubleRow.
# ----------------------------------------------------------------------------


def _build_pass2() -> bass.Bass:
    nc = bass.Bass("TRN2", target_bir_lowering=False, debug=False,
                   num_devices=NCORES)
    # mv[p, kc, i, f]: Z8^T chunk of own rows (moving operand; also the
    # stationary operand for the 4 diagonal column blocks)
    mv_d = nc.dram_tensor("mv8", [128, KCH, 2, BLK], f8,
                          kind="ExternalInput").ap()
    # wt[n, p, kc, i, c]: Z8^T of off-diagonal column block n (stationary)
    wt_d = nc.dram_tensor("wt8", [NBLK - 4, 128, KCH, 2, 128], f8,
                          kind="ExternalInput").ap()
    # ko[p, pos, f]: raw squared products ps^2 = (Z8_c . Z8_r)^2; the
    # norm corrections are a host-side outer-product scaling at assembly.
    # Diagonal positions d hold only free rows [128d, 512) (staircase); the
    # host mirrors the rest.
    ko_d = nc.dram_tensor("ko", [128, NBLK, BLK], bf16,
                          kind="ExternalOutput").ap()

    with tile.TileContext(nc) as tc:
        with (
            tc.tile_pool(name="mv", bufs=1) as mpool,
            tc.tile_pool(name="wt", bufs=16) as wpool,
            tc.tile_pool(name="post", bufs=1) as qpool,
            tc.tile_pool(name="psumd", bufs=1, space="PSUM") as dpool,
            tc.tile_pool(name="psum", bufs=2, space="PSUM") as ppool,
        ):
            mv = mpool.tile([128, KCH, 2, BLK], f8, tag="mv")
            # mv streams in 8 chunks so the opening diagonal blocks can
            # chase it; wt panels follow just-in-time inside the block loop
            # (6-buffer lookahead), all on the in-order SP queue
            for h in range(8):
                nc.sync.dma_start(mv[:, 2 * h:2 * h + 2],
                                  mv_d[:, 2 * h:2 * h + 2])
            # all 16 wt panels are resident (8MB SBUF) and their DMAs are
            # emitted up-front with no waits: their pool requests all queue
            # ahead of every data-dependent output store, so the pool runs
            # [mv | wt0..wt15 | stores] back-to-back
            wts = {}
            for n in range(16):
                wt = wpool.tile([128, KCH, 2, 128], f8, tag="wt",
                                name=f"wt_{n}")
                nc.sync.dma_start(wt[:], wt_d[n])
                wts[n] = wt

            wa = mpool.tile([128, 2, 128], f8, tag="wa")
            wb = mpool.tile([128, 2, 512], f8, tag="wb")
            nc.vector.memset(wa[:], 0.0)
            nc.gpsimd.memset(wb[:], 0.0)
            wps = ppool.tile([128, BLK], f32, tag="m0", name="warm")
            for i in range(WARMUP2):
                nc.tensor.matmul(wps[:], wa[:], wb[:], start=True, stop=True,
                                 perf_mode=mybir.MatmulPerfMode.DoubleRow)

            ko = qpool.tile([128, NBLK, BLK], bf16, tag="ko")

            def post(ps, pos, fsl, fo):
                nc.scalar.activation(ko[:, pos, fo:fo + fsl], ps[:, :fsl],
                                     mybir.ActivationFunctionType.Square)

            def store(p0, p1, eng):
                # paired stores halve the serial SWDGE descriptor-generation
                # cost; their data-dependent pool requests trail the
                # up-front input DMAs in FIFO order
                eng.dma_start(ko_d[:, p0:p1], ko[:, p0:p1])

            # positions 0-3: the four diagonal blocks, k-interleaved so the
            # whole opening chases the mv chunk stream. Staircase trim:
            # block d computes only rows [128d, 512) — the host mirrors the
            # strictly-lower remainder from the transposed writes.
            dps = [dpool.tile([128, BLK], f32, tag=f"d{d}", name=f"dps_{d}")
                   for d in range(4)]
            for k in range(KCH):
                for d in range(4):
                    nc.tensor.matmul(
                        dps[d][:, :BLK - 128 * d],
                        mv[:, k, :, d * 128:(d + 1) * 128],
                        mv[:, k, :, 128 * d:], start=(k == 0),
                        stop=(k == KCH - 1),
                        perf_mode=mybir.MatmulPerfMode.DoubleRow)
            for d in range(4):
                post(dps[d], d, BLK - 128 * d, 128 * d)
            store(0, 2, nc.gpsimd)
            store(2, 4, nc.gpsimd)

            # positions 4..19: off-diagonal blocks on the wt stream
            for pos in range(4, NBLK):
                n = ORDER2[pos]
                halves = (((0, BLK),) if pos < NBLK - 2 else
                          ((0, 256), (256, 256)))
                for fo, fsl in halves:
                    ps = ppool.tile([128, BLK], f32, tag=f"m{pos % 2}",
                                    name=f"m_{pos}_{fo}")
                    for k in range(KCH):
                        nc.tensor.matmul(
                            ps[:, :fsl], wts[n][:, k], mv[:, k, :, fo:fo + fsl],
                            start=(k == 0), stop=(k == KCH - 1),
                            perf_mode=mybir.MatmulPerfMode.DoubleRow)
                    post(ps, pos, fsl, fo)
                    if pos >= NBLK - 2:
                        # tail: each final piece stores alone on the idle SP
                        # queue (HWDGE), skipping the Pool generation backlog
                        nc.sync.dma_start(ko_d[:, pos, fo:fo + fsl],
                                          ko[:, pos, fo:fo + fsl])
                if 4 < pos < NBLK - 2 and pos % 2 == 1:
                    store(pos - 1, pos + 1, nc.gpsimd)
    return nc


_nc1 = None
_nc2 = None

PROFILE = False
LAST_PROFILE: dict = {}


def kernel(X: np.ndarray, params: np.ndarray) -> np.ndarray:
    global _nc1, _nc2
    _install_waitfix()
    X = np.asarray(X, np.float32)
    params = np.asarray(params, np.float32)

    # ---- host precompute -------------------------------------------------
    psi = _host_psi(params)
    phi = _features(X)                           # (B, DIM) f32
    perm, W0, W1 = _host_factor(psi)
    sched = _prune_schedule(W0, W1)
    phiT = np.ascontiguousarray(phi[:, perm].T)  # (DIM parity-ordered, B)

    lam_p = 64.0
    # bound |tail| <= max column norm of W so psum fits fp8 range directly
    bnd0 = float(np.sqrt((W0.astype(np.float64) ** 2).sum(axis=0).max()))
    bnd1 = float(np.sqrt((W1.astype(np.float64) ** 2).sum(axis=0).max()))
    lam_w0 = 400.0 / (lam_p * max(bnd0, 1e-30))
    lam_w1 = 400.0 / (lam_p * max(bnd1, 1e-30))
    phi8 = (phiT * lam_p).astype(npf8)           # (DIM, B)

    nchunk = sum(len(ks) for _, ks in sched)

    def pack_w(W, lam_w):
        W8 = (W * lam_w).astype(npf8)            # (HDIM u, HDIM j)
        out = np.empty((128, nchunk, 2, 128), npf8)
        idx = 0
        for c, ks in sched:
            for k in ks:
                ch = W8[k * 256:(k + 1) * 256, c * 128:(c + 1) * 128]
                out[:, idx] = ch.reshape(2, 128, 128).transpose(1, 0, 2)
                idx += 1
        return out

    w_par = [pack_w(W0, lam_w0), pack_w(W1, lam_w1)]
    phi_par = []
    for p in range(2):
        rows = phi8[p * HDIM:(p + 1) * HDIM]     # (HDIM, B)
        phi_par.append(rows.reshape(KCH1, 2, 128, B).transpose(2, 0, 1, 3))

    in_maps1 = []
    for cr in range(NCORES):
        bg, p = divmod(cr, 2)
        in_maps1.append({
            "w8": w_par[p],
            "phi": np.ascontiguousarray(phi_par[p][:, :, :,
                                                   bg * BG:(bg + 1) * BG]),
        })

    if _nc1 is None:
        _nc1 = _build_pass1(sched)
    res1 = run_bass_kernel_spmd(_nc1, in_maps1, core_ids=list(range(NCORES)))

    # ---- host mid: assemble Z, quantize ----------------------------------
    ZT = phiT                                    # reuse buffer (DIM, B)
    inv = [1.0 / (lam_p * lam_w0), 1.0 / (lam_p * lam_w1)]
    pos2c = [c for c, _ in sched]
    for cr in range(NCORES):
        bg, p = divmod(cr, 2)
        t = res1.results[cr]["t"].astype(np.float32) * inv[p]   # (128,16,1024)
        for pos in range(NJB):
            c = pos2c[pos]
            ZT[p * HDIM + c * 128:p * HDIM + (c + 1) * 128,
               bg * BG:(bg + 1) * BG] += t[:, pos]

    Z8 = (ZT * LAM).astype(npf8)                 # (DIM, B)
    Z8f32 = Z8.astype(np.float32)
    rho2 = np.einsum("jb,jb->b", Z8f32, Z8f32) / (LAM * LAM)    # (B,)
    del Z8f32
    inv_all = (1.0 / (LAM * LAM * rho2)).astype(np.float64)

    # strip layout: 16 off-diagonal col blocks (strip offsets 512..2560)
    # DMA'd as wt; the 4 diagonal blocks (offsets 0..512) slice mv.
    colrel = np.concatenate([np.arange(BLK, NB_COLS), np.arange(0, BLK)])
    Z8c = Z8.reshape(KCH, 2, 128, B)
    in_maps2 = []
    for cr in range(NCORES):
        cols = (cr * BLK + colrel) % B
        mvc = Z8c[:, :, :, cr * BLK:(cr + 1) * BLK].transpose(2, 0, 1, 3)
        wtc = Z8c[:, :, :, cols[:16 * 128]].reshape(
            KCH, 2, 128, 16, 128).transpose(3, 2, 0, 1, 4)
        in_maps2.append({
            "mv8": np.ascontiguousarray(mvc),
            "wt8": np.ascontiguousarray(wtc),
        })

    if _nc2 is None:
        _nc2 = _build_pass2()
    res2 = run_bass_kernel_spmd(_nc2, in_maps2, core_ids=list(range(NCORES)))

    # ---- assemble K (with symmetric mirroring) ---------------------------
    K = np.empty((B, B), np.float32)
    for cr in range(NCORES):
        ko = res2.results[cr]["ko"].astype(np.float64)   # (128, pos, BLK)
        invr = inv_all[cr * BLK:(cr + 1) * BLK]
        for pos in range(NBLK):
            n = ORDER2[pos]
            gs = (cr * BLK + int(colrel[n * 128])) % B
            colsl = slice(gs, gs + 128)
            if n >= 16:
                fo = 128 * (n - 16)    # staircase: rows [fo, 512) only
            else:
                fo = 0
            rows = slice(cr * BLK + fo, (cr + 1) * BLK)
            blkv = (ko[:, pos, fo:] * inv_all[colsl, None]
                    * invr[None, fo:]).astype(np.float32)
            K[rows, colsl] = blkv.T
            d = 1 + n // 4 if n < 16 else 0
            if n >= 16 or 0 < d < 4 or (d == 4 and cr < 4):
                K[colsl, rows] = blkv
    return K
